# revision 1
# baseline (speedup 1.0000x reference)
"""Trainium2 Bass kernel for nn_BertCounterFactTransformer.

Contract: kernel(**inputs) takes FULL unsharded numpy inputs (as produced by
reference.setup_inputs()) and returns the FULL [32, 1024] float32 output.

Strategy (data-parallel over batch, 8 cores x 4 samples):
  - Host: compute false/option masks + per-sample-slot tile bounds from x_ids,
    transpose x to xT, shard over cores. The program is specialized to the
    bounds (max over cores per slot -> one SPMD program) and cached per
    bounds tuple; masks keep any extra computed tiles harmless.
  - Device, per sample (F = #false row tiles, OJ = first option col):
      qT projections      only cols [0, 128F)        (bf16 matmuls)
      kT projections      only cols [OJ, 512)
      gate                g = exp(al)*fmask / max(sum, 1e-8)
      scores              [128F, 512-OJ] block only   (3 types)
      E_sup = exp(S_sup/32 + obias), E_rep = exp(S_rep/32 + tanh(S_con/32) + obias)
      coeff_t = gate / rowsum(E_t);  r_t = coeff_t^T @ E_t
      pooled  = x^T @ [gate, r_rep, r_sup]  -> fused^T columns   (f32 matmuls)
  - Device, batched tail in f32: h=relu(W1^T fused + b1), y=W2^T h + b2, LN.

Key identity: gate @ (attn @ x) == (gate @ attn) @ x, so [L,D] attention
outputs are never materialized. Column masking is injected into PSUM via K=1
matmul bias rows (obias = -960 raw -> -30 after the 1/32 scale).
"""

import sys

if "/opt/trn_rl_repo" not in sys.path:
    sys.path.insert(0, "/opt/trn_rl_repo")

import numpy as np
import ml_dtypes
from contextlib import ExitStack

np_bf16 = ml_dtypes.bfloat16

import concourse.bacc as bacc
import concourse.bass as bass
import concourse.mybir as mybir
import concourse.tile as tile
from concourse import bass_utils

f32 = mybir.dt.float32
bf16 = mybir.dt.bfloat16
AF = mybir.ActivationFunctionType
ALU = mybir.AluOpType

B, L, D = 32, 512, 1024
NCORES = 8
BC = B // NCORES          # samples per core
NL = L // 128             # 4 L-tiles
ND = D // 128             # 8 D-tiles
NC3 = 3 * D // 128        # 24 tiles of the 3D fused dim
SCALE = 1.0 / 32.0        # 1/sqrt(D)
OBIAS_RAW = -960.0        # -30 after * SCALE
LN_EPS = 1e-5

PROJ_NAMES = ["w_sq", "w_sk", "w_cq", "w_ck", "w_rq", "w_rk"]
PBIAS_NAMES = ["b_sq", "b_sk", "b_cq", "b_ck", "b_rq", "b_rk"]
QS, KS, QC, KC, QR, KR = range(6)
QPROJ = (QS, QC, QR)

_PROGRAM_CACHE = {}
_M_CACHE = {}


def _m_matrix(wq, wk):
    import hashlib
    wq = np.asarray(wq, dtype=np.float32)
    wk = np.asarray(wk, dtype=np.float32)
    key = hashlib.blake2b(wq.tobytes() + wk.tobytes(), digest_size=16).digest()
    if key not in _M_CACHE:
        _M_CACHE[key] = np.ascontiguousarray(wq @ wk.T).astype(np_bf16)
    return _M_CACHE[key]


def build_program(bounds=((2, 2),) * BC, use_m=True, enable_asserts=False):
    """bounds[s] = (F, J0): false rows live in tiles [0,F), option cols in
    [128*J0, 512). Computing a superset is always correct (masks zero it)."""
    nc = bacc.Bacc(
        "TRN2",
        target_bir_lowering=False,
        debug=False,
        enable_asserts=enable_asserts,
        num_devices=NCORES,
    )

    xT_d = nc.dram_tensor("xT", [BC, D, L], bf16, kind="ExternalInput").ap()
    x_d = nc.dram_tensor("x", [BC, L, D], f32, kind="ExternalInput").ap()
    fmask_d = nc.dram_tensor("fmask", [BC, L], f32, kind="ExternalInput").ap()
    obias_d = nc.dram_tensor("obias", [BC, L], bf16, kind="ExternalInput").ap()

    if use_m:
        W_d = {p: nc.dram_tensor(n, [D, D], bf16, kind="ExternalInput").ap()
               for p, n in ((QS, "m_sup"), (QC, "m_con"), (QR, "m_rep"))}
    else:
        W_d = {p: nc.dram_tensor(PROJ_NAMES[p], [D, D], bf16, kind="ExternalInput").ap()
               for p in range(6)}
    Brow_d = {} if use_m else {
        p: nc.dram_tensor(PBIAS_NAMES[p], [1, D], bf16, kind="ExternalInput").ap()
        for p in range(6)}
    wanom_d = nc.dram_tensor("w_anom", [D, 1], bf16, kind="ExternalInput").ap()
    wf1_d = nc.dram_tensor("w_f1", [ND, 128, NC3 * 128], bf16, kind="ExternalInput").ap()
    wf2_d = nc.dram_tensor("w_f2", [ND, 128, ND * 128], bf16, kind="ExternalInput").ap()
    bf1_d = nc.dram_tensor("b_f1", [128, ND], f32, kind="ExternalInput").ap()
    bf2_d = nc.dram_tensor("b_f2", [128, ND], f32, kind="ExternalInput").ap()
    lng_d = nc.dram_tensor("ln_g", [128, ND], f32, kind="ExternalInput").ap()
    lnb_d = nc.dram_tensor("ln_b", [128, ND], f32, kind="ExternalInput").ap()

    out_d = nc.dram_tensor("out", [BC, D], f32, kind="ExternalOutput").ap()

    with tile.TileContext(nc) as tc, ExitStack() as ctx:
        const_p = ctx.enter_context(tc.tile_pool(name="const", bufs=1))
        tmp_p = ctx.enter_context(tc.tile_pool(name="tmp", bufs=2))
        sm_p = ctx.enter_context(tc.tile_pool(name="small", bufs=3))
        tail_p = ctx.enter_context(tc.tile_pool(name="tail", bufs=1))
        ps_big = ctx.enter_context(tc.tile_pool(name="psb", bufs=4, space="PSUM"))
        ps_s = ctx.enter_context(tc.tile_pool(name="pss", bufs=4, space="PSUM"))
        es2 = ExitStack()   # closed after phase C: x, E
        x_p = es2.enter_context(tc.tile_pool(name="x", bufs=3))
        e_p = es2.enter_context(tc.tile_pool(name="emat", bufs=2))
        es1 = ExitStack()   # closed after phase B: xT, W, proj
        xT_p = es1.enter_context(tc.tile_pool(name="xT", bufs=1))
        w_p = es1.enter_context(tc.tile_pool(name="w", bufs=2))
        proj_p = es1.enter_context(tc.tile_pool(name="proj", bufs=1))

        # ---- constants ----
        ones_row = const_p.tile([1, L], bf16)
        nc.vector.memset(ones_row[:], 1.0)
        ones_f = const_p.tile([1, 128], f32)
        nc.vector.memset(ones_f[:], 1.0)
        ones_col = const_p.tile([128, 1], f32)
        nc.vector.memset(ones_col[:], 1.0)
        iot_t = const_p.tile([128, 128], mybir.dt.int32)
        nc.gpsimd.iota(iot_t[:], pattern=[[1, 128]], base=0, channel_multiplier=-1)
        ident_t = const_p.tile([128, 128], f32)
        nc.vector.tensor_scalar(ident_t[:], iot_t[:], scalar1=0, scalar2=None,
                                op0=ALU.is_equal)

        wanom_t = const_p.tile([128, ND], bf16)
        nc.scalar.dma_start(wanom_t[:], wanom_d[:, 0].rearrange("(k p) -> p k", p=128))
        brow_t = {}
        for p in Brow_d:
            brow_t[p] = const_p.tile([1, D], bf16, name=f"brow{p}")
            nc.sync.dma_start(brow_t[p][:], Brow_d[p][:])
        bf1_t = const_p.tile([128, ND], f32)
        nc.scalar.dma_start(bf1_t[:], bf1_d[:])
        bf2_t = const_p.tile([128, ND], f32)
        nc.scalar.dma_start(bf2_t[:], bf2_d[:])
        lng_t = const_p.tile([128, ND], f32)
        nc.scalar.dma_start(lng_t[:], lng_d[:])
        lnb_t = const_p.tile([128, ND], f32)
        nc.scalar.dma_start(lnb_t[:], lnb_d[:])

        fusedT = tail_p.tile([128, NC3, BC], bf16)

        # per-slot geometry
        geo = []
        for s in range(BC):
            F, J0 = bounds[s]
            geo.append((F, J0, F * 128, J0 * 128, L - J0 * 128,
                        F > 0 and L - J0 * 128 > 0))

        # ---- Phase A: xT resident + gates; M weights via one DMA each ----
        xT_t = xT_p.tile([128, BC * ND, L], bf16)
        fm_ts, ob_ts, x_ts = [], [], []
        for s in range(BC):
            nc.sync.dma_start(
                xT_t[:, s * ND : (s + 1) * ND, :],
                xT_d[s].rearrange("(k p) i -> p k i", p=128),
            )
            fm_t = sm_p.tile([128, NL], f32, tag="fm", bufs=BC, name=f"fm{s}")
            nc.scalar.dma_start(fm_t[:], fmask_d[s].rearrange("(t p) -> p t", p=128))
            fm_ts.append(fm_t)
            ob_t = sm_p.tile([1, L], bf16, tag="ob", bufs=2, name=f"ob{s}")
            nc.scalar.dma_start(ob_t[:], obias_d[s : s + 1, :])
            ob_ts.append(ob_t)

        gate_ts = []
        for s in range(BC):
            F, J0, CQ, OJ, NO, have_attn = geo[s]
            gate_t = sm_p.tile([128, NL], f32, tag="gate", bufs=BC, name=f"gate{s}")
            gate_ts.append(gate_t)
            if F == 0:
                continue
            ghat_t = sm_p.tile([128, NL], f32, tag="ghat")
            for it in range(F):
                al_ps = ps_s.tile([128, 1], f32, tag="pss")
                for k in range(ND):
                    nc.tensor.matmul(
                        al_ps[:],
                        lhsT=xT_t[:, s * ND + k, it * 128 : (it + 1) * 128],
                        rhs=wanom_t[:, k : k + 1],
                        start=(k == 0), stop=(k == ND - 1),
                    )
                eg_t = sm_p.tile([128, 1], f32, tag="eg")
                nc.scalar.activation(eg_t[:], al_ps[:], AF.Exp)
                nc.vector.tensor_mul(
                    ghat_t[:, it : it + 1], eg_t[:], fm_ts[s][:, it : it + 1]
                )
            gsum_t = sm_p.tile([128, 1], f32, tag="gsum")
            nc.vector.tensor_reduce(
                gsum_t[:], ghat_t[:, 0:F], axis=mybir.AxisListType.X, op=ALU.add
            )
            S_ps = ps_s.tile([1, 1], f32, tag="pss")
            nc.tensor.matmul(S_ps[:], lhsT=gsum_t[:], rhs=ones_col[:],
                             start=True, stop=True)
            Smax_t = sm_p.tile([1, 1], f32, tag="Smax")
            nc.vector.tensor_scalar_max(Smax_t[:], S_ps[:], 1e-8)
            Sb_ps = ps_s.tile([128, 1], f32, tag="pss")
            nc.tensor.matmul(Sb_ps[:], lhsT=ones_f[:], rhs=Smax_t[:],
                             start=True, stop=True)
            recipS_t = sm_p.tile([128, 1], f32, tag="recipS")
            nc.vector.reciprocal(recipS_t[:], Sb_ps[:])
            nc.vector.tensor_scalar_mul(gate_t[:, 0:F], ghat_t[:, 0:F],
                                        recipS_t[:])

        # ---- projections: one gpsimd DMA per M matrix, all samples inner ----
        projs = [[None] * BC for _ in range(6)]
        proj_list = list(QPROJ) if use_m else list(range(6))
        for p in proj_list:
            qside = p in QPROJ
            widths = [
                ((g[2] if qside else g[4]) if g[5] else 0) for g in geo
            ]
            wmax = max(widths)
            if wmax == 0:
                continue
            wt = w_p.tile([128, ND, D], bf16, tag="w", name=f"w{p}")
            nc.gpsimd.dma_start(wt[:], W_d[p].rearrange("(k p) c -> p k c", p=128))
            pt = proj_p.tile([128, BC, ND, wmax], bf16, tag=f"proj{p}")
            for m in range(ND):
                for s in range(BC):
                    width = widths[s]
                    if width == 0:
                        continue
                    lo = 0 if qside else geo[s][3]
                    ps = ps_big.tile([128, width], f32, tag="ps")
                    for k in range(ND):
                        nc.tensor.matmul(
                            ps[:], lhsT=wt[:, k, m * 128 : (m + 1) * 128],
                            rhs=xT_t[:, s * ND + k, lo : lo + width],
                            start=(k == 0), stop=(use_m and k == ND - 1),
                        )
                    if not use_m:
                        nc.tensor.matmul(
                            ps[:], lhsT=brow_t[p][:, m * 128 : (m + 1) * 128],
                            rhs=ones_row[:, 0:width], start=False, stop=True,
                        )
                    nc.vector.tensor_copy(pt[:, s, m, :], ps[:])
            for s in range(BC):
                if widths[s]:
                    projs[p][s] = pt

        for s in range(BC):
            x_t = x_p.tile([128, NL, D], f32, tag="x", name=f"x{s}")
            nc.sync.dma_start(x_t[:], x_d[s].rearrange("(t p) d -> p t d", p=128))
            x_ts.append(x_t)

        # ---- Phase B: scores -> E, coeffs (all samples) ----
        E_sups, E_reps, co_sups, co_reps = {}, {}, {}, {}
        for s in range(BC):
            F, J0, CQ, OJ, NO, have_attn = geo[s]
            if not have_attn:
                continue
            E_sup = e_p.tile([128, max(F, 1), NO], f32, tag="esup", bufs=BC,
                             name=f"esup{s}")
            E_rep = e_p.tile([128, max(F, 1), NO], f32, tag="erep", bufs=BC,
                             name=f"erep{s}")
            co_sup = sm_p.tile([128, NL], f32, tag="cosup", bufs=BC,
                               name=f"cosup{s}")
            co_rep = sm_p.tile([128, NL], f32, tag="corep", bufs=BC,
                               name=f"corep{s}")
            E_sups[s], E_reps[s] = E_sup, E_rep
            co_sups[s], co_reps[s] = co_sup, co_rep
            gate_t = gate_ts[s]
            ob_t = ob_ts[s]
            for it in range(F):
                isl = slice(it * 128, (it + 1) * 128)
                ps_sup = ps_big.tile([128, NO], f32, tag="ps")
                for k in range(ND):
                    nc.tensor.matmul(
                        ps_sup[:], lhsT=projs[QS][s][:, s, k, isl],
                        rhs=(xT_t[:, s * ND + k, OJ:L] if use_m
                             else projs[KS][s][:, s, k, 0:NO]),
                        start=(k == 0), stop=False,
                    )
                nc.tensor.matmul(ps_sup[:], lhsT=ones_row[:, 0:128],
                                 rhs=ob_t[:, OJ:L], start=False, stop=True)
                ps_con = ps_big.tile([128, NO], f32, tag="ps")
                for k in range(ND):
                    nc.tensor.matmul(
                        ps_con[:], lhsT=projs[QC][s][:, s, k, isl],
                        rhs=(xT_t[:, s * ND + k, OJ:L] if use_m
                             else projs[KC][s][:, s, k, 0:NO]),
                        start=(k == 0), stop=(k == ND - 1),
                    )
                ps_rep = ps_big.tile([128, NO], f32, tag="ps")
                for k in range(ND):
                    nc.tensor.matmul(
                        ps_rep[:], lhsT=projs[QR][s][:, s, k, isl],
                        rhs=(xT_t[:, s * ND + k, OJ:L] if use_m
                             else projs[KR][s][:, s, k, 0:NO]),
                        start=(k == 0), stop=False,
                    )
                nc.tensor.matmul(ps_rep[:], lhsT=ones_row[:, 0:128],
                                 rhs=ob_t[:, OJ:L], start=False, stop=True)

                T_t = tmp_p.tile([128, NO], f32, tag="T")
                nc.scalar.activation(T_t[:], ps_con[:], AF.Tanh, scale=SCALE)
                A_t = tmp_p.tile([128, NO], f32, tag="A")
                nc.vector.scalar_tensor_tensor(
                    A_t[:], in0=ps_rep[:], scalar=SCALE, in1=T_t[:],
                    op0=ALU.mult, op1=ALU.add,
                )
                rs_sup = sm_p.tile([128, 1], f32, tag="rssup")
                nc.scalar.activation(E_sup[:, it, :], ps_sup[:], AF.Exp,
                                     scale=SCALE, accum_out=rs_sup[:])
                rs_rep = sm_p.tile([128, 1], f32, tag="rsrep")
                nc.scalar.activation(E_rep[:, it, :], A_t[:], AF.Exp,
                                     accum_out=rs_rep[:])
                rc_sup = sm_p.tile([128, 1], f32, tag="rcsup")
                nc.vector.reciprocal(rc_sup[:], rs_sup[:])
                nc.vector.tensor_mul(co_sup[:, it : it + 1],
                                     gate_t[:, it : it + 1], rc_sup[:])
                rc_rep = sm_p.tile([128, 1], f32, tag="rcrep")
                nc.vector.reciprocal(rc_rep[:], rs_rep[:])
                nc.vector.tensor_mul(co_rep[:, it : it + 1],
                                     gate_t[:, it : it + 1], rc_rep[:])

        es1.close()

        # ---- Phase C: r vectors, G, pooled (all samples) ----
        for s in range(BC):
            F, J0, CQ, OJ, NO, have_attn = geo[s]
            x_t = x_ts[s]

            G_t = sm_p.tile([128, NL, 3], f32, tag="G")
            nc.vector.memset(G_t[:], 0.0)
            if F > 0:
                for it in range(F):
                    nc.vector.tensor_copy(G_t[:, it, 0:1],
                                          gate_ts[s][:, it : it + 1])
            if have_attn:
                E_sup, E_rep = E_sups[s], E_reps[s]
                co_sup, co_rep = co_sups[s], co_reps[s]
                for jt in range(J0, NL):
                    jsl = slice(jt * 128 - OJ, jt * 128 - OJ + 128)
                    r_ps = ps_s.tile([128, 2], f32, tag="pss")
                    for it in range(F):
                        nc.tensor.matmul(
                            r_ps[:, 0:1], lhsT=E_rep[:, it, jsl],
                            rhs=co_rep[:, it : it + 1],
                            start=(it == 0), stop=(it == F - 1),
                        )
                    for it in range(F):
                        nc.tensor.matmul(
                            r_ps[:, 1:2], lhsT=E_sup[:, it, jsl],
                            rhs=co_sup[:, it : it + 1],
                            start=(it == 0), stop=(it == F - 1),
                        )
                    nc.vector.tensor_copy(G_t[:, jt, 1:3], r_ps[:, 0:2])

            rts = sorted(set(range(F)) | (set(range(J0, NL)) if have_attn else set()))
            if not rts:
                rts = [0]
            for m in range(ND):
                pool_ps = ps_s.tile([128, 3], f32, tag="pss")
                for i, rt in enumerate(rts):
                    nc.tensor.matmul(
                        pool_ps[:], lhsT=x_t[:, rt, m * 128 : (m + 1) * 128],
                        rhs=G_t[:, rt, :],
                        start=(i == 0), stop=(i == len(rts) - 1),
                    )
                for t in range(3):
                    nc.vector.tensor_copy(
                        fusedT[:, t * ND + m, s : s + 1], pool_ps[:, t : t + 1]
                    )

        es2.close()

        # ---- batched MLP tail ----
        wf1_p = ctx.enter_context(tc.tile_pool(name="wf1", bufs=8))
        hT_t = tail_p.tile([128, ND, BC], bf16)
        for m in range(ND):
            wt = wf1_p.tile([128, NC3, 128], bf16, tag="wf1")
            nc.gpsimd.dma_start(wt[:], wf1_d[m].rearrange("p (k c) -> p k c", c=128))
            h_ps = ps_s.tile([128, BC], f32, tag="pss")
            for k in range(NC3):
                nc.tensor.matmul(h_ps[:], lhsT=wt[:, k, :], rhs=fusedT[:, k, :],
                                 start=(k == 0), stop=(k == NC3 - 1))
            nc.scalar.activation(hT_t[:, m, :], h_ps[:], AF.Relu,
                                 bias=bf1_t[:, m : m + 1])

        yT_t = tail_p.tile([128, ND, BC], f32)
        sq_t = tail_p.tile([128, ND, BC], f32)
        for m in range(ND):
            wt = wf1_p.tile([128, ND, 128], bf16, tag="wf2")
            nc.gpsimd.dma_start(wt[:], wf2_d[m].rearrange("p (k c) -> p k c", c=128))
            y_ps = ps_s.tile([128, BC], f32, tag="pss")
            for k in range(ND):
                nc.tensor.matmul(y_ps[:], lhsT=wt[:, k, :], rhs=hT_t[:, k, :],
                                 start=(k == 0), stop=(k == ND - 1))
            nc.vector.tensor_scalar_add(yT_t[:, m, :], y_ps[:], bf2_t[:, m : m + 1])
            nc.scalar.square(sq_t[:, m, :], yT_t[:, m, :])

        sum_ps = ps_s.tile([1, BC], f32, tag="pss")
        for m in range(ND):
            nc.tensor.matmul(sum_ps[:], lhsT=ones_col[:], rhs=yT_t[:, m, :],
                             start=(m == 0), stop=(m == ND - 1))
        ssq_ps = ps_s.tile([1, BC], f32, tag="pss")
        for m in range(ND):
            nc.tensor.matmul(ssq_ps[:], lhsT=ones_col[:], rhs=sq_t[:, m, :],
                             start=(m == 0), stop=(m == ND - 1))
        mean_t = sm_p.tile([1, BC], f32, tag="mean")
        nc.scalar.mul(mean_t[:], sum_ps[:], 1.0 / D)
        msq_t = sm_p.tile([1, BC], f32, tag="msq")
        nc.scalar.mul(msq_t[:], ssq_ps[:], 1.0 / D)
        m2_t = sm_p.tile([1, BC], f32, tag="m2")
        nc.vector.tensor_mul(m2_t[:], mean_t[:], mean_t[:])
        var_t = sm_p.tile([1, BC], f32, tag="var")
        nc.vector.tensor_sub(var_t[:], msq_t[:], m2_t[:])
        nc.vector.tensor_scalar_add(var_t[:], var_t[:], LN_EPS)
        sd_t = sm_p.tile([1, BC], f32, tag="sd")
        nc.scalar.sqrt(sd_t[:], var_t[:])
        rstd_t = sm_p.tile([1, BC], f32, tag="rstd")
        nc.vector.reciprocal(rstd_t[:], sd_t[:])

        mb_ps = ps_s.tile([128, BC], f32, tag="pss")
        nc.tensor.matmul(mb_ps[:], lhsT=ones_f[:], rhs=mean_t[:],
                         start=True, stop=True)
        mb_t = sm_p.tile([128, BC], f32, tag="mbt")
        nc.vector.tensor_copy(mb_t[:], mb_ps[:])
        rb_ps = ps_s.tile([128, BC], f32, tag="pss")
        nc.tensor.matmul(rb_ps[:], lhsT=ones_f[:], rhs=rstd_t[:],
                         start=True, stop=True)
        rb_t = sm_p.tile([128, BC], f32, tag="rbt")
        nc.vector.tensor_copy(rb_t[:], rb_ps[:])

        zrow_t = tail_p.tile([BC, D], f32)
        for m in range(ND):
            z_t = tmp_p.tile([128, BC], f32, tag="z")
            nc.vector.tensor_sub(z_t[:], yT_t[:, m, :], mb_t[:])
            nc.vector.tensor_mul(z_t[:], z_t[:], rb_t[:])
            z2_t = tmp_p.tile([128, BC], f32, tag="z2")
            nc.vector.tensor_scalar(
                z2_t[:], z_t[:], scalar1=lng_t[:, m : m + 1],
                scalar2=lnb_t[:, m : m + 1], op0=ALU.mult, op1=ALU.add,
            )
            tr_ps = ps_s.tile([BC, 128], f32, tag="pss")
            nc.tensor.transpose(tr_ps[:], z2_t[:], ident_t[:])
            nc.vector.tensor_copy(zrow_t[:, m * 128 : (m + 1) * 128], tr_ps[:])
        nc.sync.dma_start(out_d[:, :], zrow_t[:, :])

    nc.compile()
    return nc


def _host_prep(inputs):
    """Returns (in_maps, bounds)."""
    x = np.asarray(inputs["x"], dtype=np.float32)
    x_ids = np.asarray(inputs["x_ids"])
    pad_idx = int(np.asarray(inputs["pad_idx"]))
    sep_idx = int(np.asarray(inputs["sep_idx"]))
    assert x.shape == (B, L, D), x.shape

    valid = x_ids != pad_idx
    sepm = x_ids == sep_idx
    has = sepm.any(axis=1)
    first = sepm.argmax(axis=1)
    vlen = valid.sum(axis=1)
    fb = np.clip(vlen // 2, 1, max(1, L - 2))
    sp = np.where(has, first, fb)
    pos = np.arange(L)
    fmask = ((pos[None, :] < sp[:, None]) & valid).astype(np.float32)
    omask = (pos[None, :] > sp[:, None]) & valid
    obias = np.where(omask, 0.0, OBIAS_RAW).astype(np.float32)

    # per-slot tile bounds: F covers all false rows, J0 covers all option cols
    F_all = np.ceil(sp / 128).astype(int)           # false subset of [0, sep)
    J0_all = np.minimum((sp + 1) // 128, NL)        # option subset of [sep+1, L)
    bounds = tuple(
        (int(F_all.reshape(NCORES, BC)[:, s].max()),
         int(J0_all.reshape(NCORES, BC)[:, s].min()))
        for s in range(BC)
    )

    xT = np.ascontiguousarray(x.transpose(0, 2, 1))

    def w(name):
        return np.ascontiguousarray(np.asarray(inputs[name], dtype=np.float32))

    def ppart(name):
        return np.ascontiguousarray(np.asarray(inputs[name], dtype=np.float32)
                                    .reshape(ND, 128).T)

    use_m = all(not np.any(np.asarray(inputs[n])) for n in PBIAS_NAMES)
    shared = {}
    if use_m:
        for dst, qn, kn in (("m_sup", "w_sq", "w_sk"), ("m_con", "w_cq", "w_ck"),
                            ("m_rep", "w_rq", "w_rk")):
            shared[dst] = _m_matrix(inputs[qn], inputs[kn])
    else:
        for p in range(6):
            shared[PROJ_NAMES[p]] = w(PROJ_NAMES[p]).astype(np_bf16)
            shared[PBIAS_NAMES[p]] = w(PBIAS_NAMES[p]).reshape(1, D).astype(np_bf16)
    shared["w_anom"] = w("w_anom").reshape(D, 1).astype(np_bf16)
    def mpack(name, nk):
        a = w(name)                      # [nk*128, ND*128]
        a = a.reshape(nk, 128, ND, 128).transpose(2, 1, 0, 3).reshape(ND, 128, nk * 128)
        return np.ascontiguousarray(a).astype(np_bf16)

    shared["w_f1"] = mpack("w_f1", NC3)
    shared["w_f2"] = mpack("w_f2", ND)
    shared["b_f1"] = ppart("b_f1")
    shared["b_f2"] = ppart("b_f2")
    shared["ln_g"] = ppart("ln_g")
    shared["ln_b"] = ppart("ln_b")

    in_maps = []
    for c in range(NCORES):
        sl = slice(c * BC, (c + 1) * BC)
        m = dict(shared)
        m["x"] = np.ascontiguousarray(x[sl])
        m["xT"] = np.ascontiguousarray(xT[sl]).astype(np_bf16)
        m["fmask"] = np.ascontiguousarray(fmask[sl])
        m["obias"] = np.ascontiguousarray(obias[sl]).astype(np_bf16)
        in_maps.append(m)
    return in_maps, bounds, use_m


def get_program(bounds, use_m):
    key = (bounds, use_m)
    if key not in _PROGRAM_CACHE:
        _PROGRAM_CACHE[key] = build_program(bounds, use_m=use_m)
    return _PROGRAM_CACHE[key]


def run(trace=False, **inputs):
    in_maps, bounds, use_m = _host_prep(inputs)
    nc = get_program(bounds, use_m)
    res = bass_utils.run_bass_kernel_spmd(
        nc, in_maps, core_ids=list(range(NCORES)), trace=trace
    )
    out = np.concatenate([res.results[c]["out"] for c in range(NCORES)], axis=0)
    return out.astype(np.float32), res


def kernel(**inputs):
    out, _ = run(trace=False, **inputs)
    return out



# revision 8
# speedup vs baseline: 1.5013x; 1.5013x over previous
"""Trainium2 Bass kernel for nn_BertCounterFactTransformer.

Contract: kernel(**inputs) takes FULL unsharded numpy inputs (as produced by
reference.setup_inputs()) and returns the FULL [32, 1024] float32 output.

Strategy (data-parallel over batch, 8 cores x 4 samples):
  - Host: compute sep positions from x_ids, SORT samples by sep position and
    assign sorted rank r -> core (r % 8), slot (r // 8) so the per-slot-pair
    tile bounds are tight and uniform across cores. Precompute
    M_p = W_pq @ W_pk^T (bf16) so scores are x M x^T (no k-side projection).
  - Device, per pair of slots (F tiles of false rows, option cols from OJ):
      gate       all-4-sample anomaly logits in ONE [4, wg] PSUM via
                 block-diagonal embedded w_anom; false-mask folded as a
                 -30 bias row; exp+normalize row-wise; PE-transpose to cols
      proj       qT = (x M_p)^T (or M_p x_opt^T if the option side is
                 smaller), 2 samples batched per matmul (width<=512)
      scores     S = q @ x_opt^T blocks; option mask via -960 bias rows
      E_sup = exp(S_sup/32 + ob), E_rep = exp(S_rep/32 + tanh(S_con/32) + ob)
      coeff_t = gate / rowsum(E_t);  r_t = E_t^T @ coeff_t   (width-1 chains)
  - Pool: ONE [12, 512] PSUM accumulates x_s^T @ [gate|r_rep|r_sup] for all
    4 samples via 12-col G with per-sample zero blocks; PE-transpose the
    [12, 1024] result into fused^T columns.
  - Tail in row form: h = relu(fused @ W1 + b1), y = h @ W2 + b2, LayerNorm
    along the free dim, direct [4, 1024] output DMA.

Key identity: gate @ (attn @ x) == (gate @ attn) @ x, so [L,D] attention
outputs are never materialized.
"""

import sys

if "/opt/trn_rl_repo" not in sys.path:
    sys.path.insert(0, "/opt/trn_rl_repo")

import numpy as np
import ml_dtypes
from contextlib import ExitStack

np_bf16 = ml_dtypes.bfloat16

import concourse.bacc as bacc
import concourse.bass as bass
import concourse.mybir as mybir
import concourse.tile as tile
from concourse import bass_utils

f32 = mybir.dt.float32
bf16 = mybir.dt.bfloat16
AF = mybir.ActivationFunctionType
ALU = mybir.AluOpType

B, L, D = 32, 512, 1024
NCORES = 8
BC = B // NCORES          # samples per core
NL = L // 128             # 4 L-tiles
ND = D // 128             # 8 D-tiles
NC3 = 3 * D // 128        # 24 tiles of the 3D fused dim
SCALE = 1.0 / 32.0        # 1/sqrt(D)
OBIAS_RAW = -960.0        # -30 after * SCALE
FBIAS = -30.0
LN_EPS = 1e-5

_PROGRAM_CACHE = {}
_M_CACHE = {}


def _m_matrix(wq, wk, transposed=False):
    import hashlib
    wq = np.asarray(wq, dtype=np.float32)
    wk = np.asarray(wk, dtype=np.float32)
    key = (hashlib.blake2b(wq.tobytes() + wk.tobytes(), digest_size=16).digest(),
           transposed)
    if key not in _M_CACHE:
        m = wq @ wk.T
        if transposed:
            m = m.T
        _M_CACHE[key] = np.ascontiguousarray(m).astype(np_bf16)
    return _M_CACHE[key]


def _geo(F, J0):
    OJ = J0 * 128
    NO = L - OJ
    CQ = F * 128
    have = NO > 0
    side_q = (CQ <= NO) if have else True
    w = (CQ if side_q else NO) if have else 0
    lo = 0 if side_q else OJ
    return dict(F=F, J0=J0, OJ=OJ, NO=NO, CQ=CQ, have=have,
                side_q=side_q, w=w, lo=lo)


def build_program_fast(pair_geo):
    """pair_geo = ((F0, J0_0), (F1, J0_1)); pair p covers slots {2p, 2p+1}.
    Computing a superset is always correct (bias masks zero it)."""
    nc = bacc.Bacc(
        "TRN2",
        target_bir_lowering=False,
        debug=False,
        enable_asserts=False,
        num_devices=NCORES,
    )

    geos = [_geo(F, J0) for (F, J0) in pair_geo]
    need_mt = any(g["have"] and not g["side_q"] for g in geos)
    wg = max(max(g["CQ"] for g in geos), 128)       # gate width (cols 0..wg)
    maxF = max(g["F"] for g in geos)

    xT_d = nc.dram_tensor("xT", [ND, 128, BC, L], bf16, kind="ExternalInput").ap()
    x_d = nc.dram_tensor("x", [BC, L, D], bf16, kind="ExternalInput").ap()
    fb_d = nc.dram_tensor("fbias", [BC, L], bf16, kind="ExternalInput").ap()
    ob_d = nc.dram_tensor("obias", [BC, L], bf16, kind="ExternalInput").ap()
    m_d = [nc.dram_tensor(f"m{p}", [D, D], bf16, kind="ExternalInput").ap()
           for p in range(3)]
    mt_d = ([nc.dram_tensor(f"mt{p}", [D, D], bf16, kind="ExternalInput").ap()
             for p in range(3)] if need_mt else None)
    emb_d = nc.dram_tensor("wanom_emb", [128, ND, BC, BC], bf16,
                           kind="ExternalInput").ap()
    w1_d = nc.dram_tensor("w_f1", [NC3, 128, D], bf16, kind="ExternalInput").ap()
    w2_d = nc.dram_tensor("w_f2", [ND, 128, D], bf16, kind="ExternalInput").ap()
    b1_d = nc.dram_tensor("b_f1", [1, D], bf16, kind="ExternalInput").ap()
    b2_d = nc.dram_tensor("b_f2", [1, D], bf16, kind="ExternalInput").ap()
    lng_d = nc.dram_tensor("ln_g", [BC, D], f32, kind="ExternalInput").ap()
    lnb_d = nc.dram_tensor("ln_b", [BC, D], f32, kind="ExternalInput").ap()
    out_d = nc.dram_tensor("out", [BC, D], f32, kind="ExternalOutput").ap()

    with tile.TileContext(nc) as tc, ExitStack() as ctx:
        const_p = ctx.enter_context(tc.tile_pool(name="const", bufs=1))
        main_p = ctx.enter_context(tc.tile_pool(name="main", bufs=1))
        sm_p = ctx.enter_context(tc.tile_pool(name="small", bufs=2))
        tmp_p = ctx.enter_context(tc.tile_pool(name="tmp", bufs=2))
        ps_big = ctx.enter_context(tc.tile_pool(name="psb", bufs=4, space="PSUM"))
        ps_med = ctx.enter_context(tc.tile_pool(name="psm", bufs=2, space="PSUM"))
        ps_sm = ctx.enter_context(tc.tile_pool(name="pss", bufs=2, space="PSUM"))
        es2 = ExitStack()   # E matrices; closed after last r
        e_p = es2.enter_context(tc.tile_pool(name="emat", bufs=1))
        es1 = ExitStack()   # W matrices + projT; closed after last scores
        w_p = es1.enter_context(tc.tile_pool(name="w", bufs=1))
        proj_p = es1.enter_context(tc.tile_pool(name="proj", bufs=1))

        # ---- constants ----
        ones_row = const_p.tile([1, 128], bf16)
        nc.vector.memset(ones_row[:], 1.0)
        ones4 = const_p.tile([1, BC], bf16)
        nc.vector.memset(ones4[:], 1.0)
        eyerows = const_p.tile([1, BC, BC], bf16)
        nc.vector.memset(eyerows[:], 0.0)
        for s in range(BC):
            nc.vector.memset(eyerows[:, s, s : s + 1], 1.0)
        iot_t = const_p.tile([128, 128], mybir.dt.int32)
        nc.gpsimd.iota(iot_t[:], pattern=[[1, 128]], base=0, channel_multiplier=-1)
        ident_f = const_p.tile([128, 128], f32)
        nc.vector.tensor_scalar(ident_f[:], iot_t[:], scalar1=0, scalar2=None,
                                op0=ALU.is_equal)
        ident_b = const_p.tile([128, 128], bf16)
        nc.vector.tensor_copy(ident_b[:], ident_f[:])

        emb_t = const_p.tile([128, ND, BC, BC], bf16)
        nc.scalar.dma_start(emb_t[:], emb_d[:])
        fb_t = const_p.tile([1, BC, L], bf16)
        ob_t = const_p.tile([1, BC, L], bf16)
        for s in range(BC):
            nc.scalar.dma_start(fb_t[:, s, :], fb_d[s : s + 1, :])
            nc.scalar.dma_start(ob_t[:, s, :], ob_d[s : s + 1, :])
        b1_t = const_p.tile([1, D], bf16)
        nc.scalar.dma_start(b1_t[:], b1_d[:])
        b2_t = const_p.tile([1, D], bf16)
        nc.scalar.dma_start(b2_t[:], b2_d[:])
        lng_t = const_p.tile([BC, D], f32)
        nc.scalar.dma_start(lng_t[:], lng_d[:])
        lnb_t = const_p.tile([BC, D], f32)
        nc.scalar.dma_start(lnb_t[:], lnb_d[:])

        # ---- big inputs ----
        xT_t = main_p.tile([128, ND, BC, L], bf16)
        for k in range(ND):
            for s in range(BC):
                nc.sync.dma_start(xT_t[:, k, s, :], xT_d[k, :, s, :])
        x_t = main_p.tile([128, BC, NL, D], bf16)
        for s in range(BC):
            nc.sync.dma_start(
                x_t[:, s], x_d[s].rearrange("(t p) d -> p t d", p=128)
            )
        w_ts = []
        for p in range(3):
            wt = w_p.tile([128, ND, D], bf16, name=f"w{p}")
            nc.gpsimd.dma_start(wt[:], m_d[p].rearrange("(k p) c -> p k c", p=128))
            w_ts.append(wt)
        wt_ts = []
        if need_mt:
            for p in range(3):
                wtt = w_p.tile([128, ND, D], bf16, name=f"wt{p}")
                nc.gpsimd.dma_start(
                    wtt[:], mt_d[p].rearrange("(k p) c -> p k c", p=128)
                )
                wt_ts.append(wtt)

        G_all = main_p.tile([128, NL, BC, 12], bf16)   # col = t*4 + s
        nc.vector.memset(G_all[:], 0.0)
        gcol = main_p.tile([128, maxF, BC], f32)

        # ---- gates (all samples): logits in ONE [4, wg] PSUM ----
        ga_ps = ps_med.tile([BC, wg], f32, tag="pm")
        for s in range(BC):
            for k in range(ND):
                nc.tensor.matmul(
                    ga_ps[:], lhsT=emb_t[:, k, s, :], rhs=xT_t[:, k, s, 0:wg],
                    start=(s == 0 and k == 0), stop=False,
                )
        for s in range(BC):
            nc.tensor.matmul(
                ga_ps[:], lhsT=eyerows[:, s, :], rhs=fb_t[:, s, 0:wg],
                start=False, stop=(s == BC - 1),
            )
        grow = sm_p.tile([BC, wg], f32, tag="grow", bufs=1)
        gs_t = sm_p.tile([BC, 1], f32, tag="gs", bufs=1)
        nc.scalar.activation(grow[:], ga_ps[:], AF.Exp, accum_out=gs_t[:])
        nc.vector.tensor_scalar_max(gs_t[:], gs_t[:], 1e-8)
        rg_t = sm_p.tile([BC, 1], f32, tag="rg", bufs=1)
        nc.vector.reciprocal(rg_t[:], gs_t[:])
        nc.vector.tensor_scalar_mul(grow[:], grow[:], rg_t[:])
        for it in range(maxF):
            gt_ps = ps_med.tile([128, BC], f32, tag="pm")
            nc.tensor.transpose(gt_ps[:], grow[:, it * 128 : (it + 1) * 128],
                                ident_f[0:BC, 0:BC])
            nc.vector.tensor_copy(gcol[:, it, :], gt_ps[:])
            for s in range(BC):
                if it < geos[s // 2]["F"]:
                    nc.vector.tensor_copy(G_all[:, it, s, s : s + 1],
                                          gt_ps[:, s : s + 1])

        # ---- per pair: projections -> scores -> E -> coeffs -> r ----
        for pr in range(2):
            g = geos[pr]
            F, OJ, NO, w, lo = g["F"], g["OJ"], g["NO"], g["w"], g["lo"]
            if not g["have"]:
                continue
            s0, s1 = 2 * pr, 2 * pr + 1
            wmats = w_ts if g["side_q"] else wt_ts

            projT = [
                proj_p.tile([128, ND, 2, w], bf16, tag=f"pj{p}", name=f"pj{p}_{pr}")
                for p in range(3)
            ]
            for p in range(3):
                for m in range(ND):
                    if 2 * w <= 512:
                        pj_ps = ps_big.tile([128, 2, w], f32, tag="ps")
                        for k in range(ND):
                            nc.tensor.matmul(
                                pj_ps[:],
                                lhsT=wmats[p][:, k, m * 128 : (m + 1) * 128],
                                rhs=xT_t[:, k, s0 : s0 + 2, lo : lo + w],
                                start=(k == 0), stop=(k == ND - 1),
                            )
                        if m % 2 == 0:
                            nc.vector.tensor_copy(projT[p][:, m], pj_ps[:])
                        else:
                            nc.scalar.activation(projT[p][:, m], pj_ps[:],
                                                 AF.Copy)
                    else:
                        for sp in range(2):
                            pj_ps = ps_big.tile([128, w], f32, tag="ps")
                            for k in range(ND):
                                nc.tensor.matmul(
                                    pj_ps[:],
                                    lhsT=wmats[p][:, k, m * 128 : (m + 1) * 128],
                                    rhs=xT_t[:, k, s0 + sp, lo : lo + w],
                                    start=(k == 0), stop=(k == ND - 1),
                                )
                            if sp == 0:
                                nc.vector.tensor_copy(projT[p][:, m, sp],
                                                      pj_ps[:])
                            else:
                                nc.scalar.activation(projT[p][:, m, sp],
                                                     pj_ps[:], AF.Copy)

            for sp in range(2):
                s4 = s0 + sp
                E_sup = e_p.tile([128, F, NO], bf16, tag=f"Es{sp}",
                                 name=f"Es{sp}_{pr}")
                E_rep = e_p.tile([128, F, NO], bf16, tag=f"Er{sp}",
                                 name=f"Er{sp}_{pr}")
                co_sup = sm_p.tile([128, F], bf16, tag=f"cos{sp}", bufs=1,
                                   name=f"cos{sp}_{pr}")
                co_rep = sm_p.tile([128, F], bf16, tag=f"cor{sp}", bufs=1,
                                   name=f"cor{sp}_{pr}")
                for it in range(F):
                    isl = slice(it * 128, (it + 1) * 128)
                    ps3 = {}
                    for p in (0, 2, 1):   # sup, rep, con
                        ps = ps_big.tile([128, NO], f32, tag="ps",
                                         name=f"sc{p}")
                        ps3[p] = ps
                        for k in range(ND):
                            if g["side_q"]:
                                lhsT = projT[p][:, k, sp, isl]
                                rhs = xT_t[:, k, s4, OJ:L]
                            else:
                                lhsT = xT_t[:, k, s4, isl]
                                rhs = projT[p][:, k, sp, 0:NO]
                            nc.tensor.matmul(ps[:], lhsT=lhsT, rhs=rhs,
                                             start=(k == 0),
                                             stop=(k == ND - 1 and p == 1))
                        if p != 1:
                            # option-mask bias row closes the sup/rep groups
                            nc.tensor.matmul(ps[:], lhsT=ones_row[:],
                                             rhs=ob_t[:, s4, OJ:L],
                                             start=False, stop=True)
                    ps_sup, ps_rep, ps_con = ps3[0], ps3[2], ps3[1]

                    T_t = tmp_p.tile([128, NO], f32, tag="T")
                    nc.scalar.activation(T_t[:], ps_con[:], AF.Tanh, scale=SCALE)
                    A_t = tmp_p.tile([128, NO], f32, tag="A")
                    nc.vector.scalar_tensor_tensor(
                        A_t[:], in0=ps_rep[:], scalar=SCALE, in1=T_t[:],
                        op0=ALU.mult, op1=ALU.add,
                    )
                    rs_sup = sm_p.tile([128, 1], f32, tag="rss")
                    nc.scalar.activation(E_sup[:, it], ps_sup[:], AF.Exp,
                                         scale=SCALE, accum_out=rs_sup[:])
                    rs_rep = sm_p.tile([128, 1], f32, tag="rsr")
                    nc.scalar.activation(E_rep[:, it], A_t[:], AF.Exp,
                                         accum_out=rs_rep[:])
                    rc_sup = sm_p.tile([128, 1], f32, tag="rcs")
                    nc.vector.reciprocal(rc_sup[:], rs_sup[:])
                    nc.vector.tensor_mul(co_sup[:, it : it + 1],
                                         gcol[:, it, s4 : s4 + 1], rc_sup[:])
                    rc_rep = sm_p.tile([128, 1], f32, tag="rcr")
                    nc.vector.reciprocal(rc_rep[:], rs_rep[:])
                    nc.vector.tensor_mul(co_rep[:, it : it + 1],
                                         gcol[:, it, s4 : s4 + 1], rc_rep[:])

                # r vectors: G col 4+s (rep), 8+s (sup)
                for t, (E_t, co_t) in enumerate(((E_rep, co_rep),
                                                 (E_sup, co_sup))):
                    for jt in range(NO // 128):
                        jsl = slice(jt * 128, (jt + 1) * 128)
                        r_ps = ps_sm.tile([128, 1], f32, tag="r")
                        for it in range(F):
                            nc.tensor.matmul(
                                r_ps[:], lhsT=E_t[:, it, jsl],
                                rhs=co_t[:, it : it + 1],
                                start=(it == 0), stop=(it == F - 1),
                            )
                        nc.vector.tensor_copy(
                            G_all[:, g["J0"] + jt, s4,
                                  4 * (t + 1) + s4 : 4 * (t + 1) + s4 + 1],
                            r_ps[:],
                        )

        es1.close()
        es2.close()

        # ---- tail weights (after W matrices freed) ----
        tail_p = ctx.enter_context(tc.tile_pool(name="tail", bufs=1))
        w1_t = tail_p.tile([128, NC3, D], bf16)
        for t in range(3):
            nc.gpsimd.dma_start(
                w1_t[:, t * ND : (t + 1) * ND, :],
                w1_d[t * ND : (t + 1) * ND].rearrange("k p n -> p k n"),
            )
        w2_t = tail_p.tile([128, ND, D], bf16)
        nc.gpsimd.dma_start(w2_t[:], w2_d.rearrange("k p n -> p k n"))

        # ---- pool: pooled[t*4+s, :] = sum_l G[l, t*4+s] * x_s[l, :] ----
        seq = []
        for s4 in range(BC):
            g = geos[s4 // 2]
            rts = sorted(set(range(g["F"]))
                         | (set(range(g["J0"], NL)) if g["have"] else set()))
            seq.extend((s4, rt) for rt in rts)
        pooled_sb = main_p.tile([12, D], bf16)
        for half in range(2):
            hs = slice(half * 512, (half + 1) * 512)
            po_ps = ps_med.tile([12, 512], f32, tag="pm")
            for i, (s4, rt) in enumerate(seq):
                nc.tensor.matmul(
                    po_ps[:], lhsT=G_all[:, rt, s4, :], rhs=x_t[:, s4, rt, hs],
                    start=(i == 0), stop=(i == len(seq) - 1),
                )
            nc.vector.tensor_copy(pooled_sb[:, hs], po_ps[:])

        fused_sb = main_p.tile([128, ND, 3, BC], bf16)
        for m in range(ND):
            tr_ps = ps_sm.tile([128, 12], bf16, tag="r")
            nc.tensor.transpose(tr_ps[:], pooled_sb[:, m * 128 : (m + 1) * 128],
                                ident_b[0:12, 0:12])
            nc.vector.tensor_copy(fused_sb[:, m], tr_ps[:])

        # ---- MLP tail + LayerNorm, all in [4, 1024] row form ----
        h_sb = main_p.tile([BC, D], bf16)
        for half in range(2):
            hs = slice(half * 512, (half + 1) * 512)
            h_ps = ps_big.tile([BC, 512], f32, tag="ps")
            for t in range(3):
                for m in range(ND):
                    nc.tensor.matmul(
                        h_ps[:], lhsT=fused_sb[:, m, t, :],
                        rhs=w1_t[:, t * ND + m, hs],
                        start=(t == 0 and m == 0), stop=False,
                    )
            nc.tensor.matmul(h_ps[:], lhsT=ones4[:], rhs=b1_t[:, hs],
                             start=False, stop=True)
            nc.scalar.activation(h_sb[:, hs], h_ps[:], AF.Relu)

        hT_sb = main_p.tile([128, ND, BC], bf16)
        for m in range(ND):
            ht_ps = ps_sm.tile([128, BC], bf16, tag="r")
            nc.tensor.transpose(ht_ps[:], h_sb[:, m * 128 : (m + 1) * 128],
                                ident_b[0:BC, 0:BC])
            nc.vector.tensor_copy(hT_sb[:, m], ht_ps[:])

        y_sb = main_p.tile([BC, D], f32)
        s1_t = sm_p.tile([BC, 2], f32, tag="s1", bufs=1)
        s2_t = sm_p.tile([BC, 2], f32, tag="s2", bufs=1)
        sq_sb = tmp_p.tile([BC, 512], f32, tag="sq", bufs=2)
        for half in range(2):
            hs = slice(half * 512, (half + 1) * 512)
            y_ps = ps_big.tile([BC, 512], f32, tag="ps")
            for k in range(ND):
                nc.tensor.matmul(y_ps[:], lhsT=hT_sb[:, k, :],
                                 rhs=w2_t[:, k, hs],
                                 start=(k == 0), stop=False)
            nc.tensor.matmul(y_ps[:], lhsT=ones4[:], rhs=b2_t[:, hs],
                             start=False, stop=True)
            nc.scalar.activation(y_sb[:, hs], y_ps[:], AF.Copy,
                                 accum_out=s1_t[:, half : half + 1])
            sq = tmp_p.tile([BC, 512], f32, tag="sq", bufs=2)
            nc.scalar.activation(sq[:], y_ps[:], AF.Square,
                                 accum_out=s2_t[:, half : half + 1])

        mu_t = sm_p.tile([BC, 1], f32, tag="mu", bufs=1)
        nc.vector.tensor_reduce(mu_t[:], s1_t[:], axis=mybir.AxisListType.X,
                                op=ALU.add)
        nc.scalar.mul(mu_t[:], mu_t[:], 1.0 / D)
        msq_t = sm_p.tile([BC, 1], f32, tag="msq", bufs=1)
        nc.vector.tensor_reduce(msq_t[:], s2_t[:], axis=mybir.AxisListType.X,
                                op=ALU.add)
        nc.scalar.mul(msq_t[:], msq_t[:], 1.0 / D)
        m2_t = sm_p.tile([BC, 1], f32, tag="m2", bufs=1)
        nc.vector.tensor_mul(m2_t[:], mu_t[:], mu_t[:])
        var_t = sm_p.tile([BC, 1], f32, tag="var", bufs=1)
        nc.vector.tensor_sub(var_t[:], msq_t[:], m2_t[:])
        nc.vector.tensor_scalar_add(var_t[:], var_t[:], LN_EPS)
        sd_t = sm_p.tile([BC, 1], f32, tag="sd", bufs=1)
        nc.scalar.sqrt(sd_t[:], var_t[:])
        rstd_t = sm_p.tile([BC, 1], f32, tag="rstd", bufs=1)
        nc.vector.reciprocal(rstd_t[:], sd_t[:])

        z_sb = main_p.tile([BC, D], f32)
        nc.vector.tensor_scalar(z_sb[:], y_sb[:], scalar1=mu_t[:],
                                scalar2=rstd_t[:], op0=ALU.subtract,
                                op1=ALU.mult)
        nc.vector.tensor_mul(z_sb[:], z_sb[:], lng_t[:])
        nc.vector.tensor_add(z_sb[:], z_sb[:], lnb_t[:])
        nc.sync.dma_start(out_d[:], z_sb[:])

    nc.compile()
    return nc


def _masks(x_ids, pad_idx, sep_idx):
    valid = x_ids != pad_idx
    sepm = x_ids == sep_idx
    has = sepm.any(axis=1)
    first = sepm.argmax(axis=1)
    vlen = valid.sum(axis=1)
    fb = np.clip(vlen // 2, 1, max(1, L - 2))
    sp = np.where(has, first, fb)
    pos = np.arange(L)
    fmask = (pos[None, :] < sp[:, None]) & valid
    omask = (pos[None, :] > sp[:, None]) & valid
    return sp, fmask, omask


def _host_prep_fast(inputs):
    x = np.asarray(inputs["x"], dtype=np.float32)
    x_ids = np.asarray(inputs["x_ids"])
    pad_idx = int(np.asarray(inputs["pad_idx"]))
    sep_idx = int(np.asarray(inputs["sep_idx"]))
    assert x.shape == (B, L, D), x.shape

    sp, fmask, omask = _masks(x_ids, pad_idx, sep_idx)
    fb = np.where(fmask, 0.0, FBIAS).astype(np_bf16)
    ob = np.where(omask, 0.0, OBIAS_RAW).astype(np_bf16)

    order = np.argsort(-sp, kind="stable")
    F_all = np.ceil(sp / 128).astype(int)
    J0_all = np.minimum((sp + 1) // 128, NL)
    pair_geo = tuple(
        (int(F_all[order[pr * 16 : (pr + 1) * 16]].max()),
         int(J0_all[order[pr * 16 : (pr + 1) * 16]].min()))
        for pr in range(2)
    )
    geos = [_geo(F, J0) for (F, J0) in pair_geo]
    need_mt = any(g["have"] and not g["side_q"] for g in geos)

    def w(name):
        return np.ascontiguousarray(np.asarray(inputs[name], dtype=np.float32))

    shared = {}
    for p, (qn, kn) in enumerate((("w_sq", "w_sk"), ("w_cq", "w_ck"),
                                  ("w_rq", "w_rk"))):
        shared[f"m{p}"] = _m_matrix(inputs[qn], inputs[kn])
        if need_mt:
            shared[f"mt{p}"] = _m_matrix(inputs[qn], inputs[kn], transposed=True)

    wanom_pm = w("w_anom").reshape(ND, 128).T            # [128, ND]
    emb = np.zeros((128, ND, BC, BC), np.float32)
    for s in range(BC):
        emb[:, :, s, s] = wanom_pm
    shared["wanom_emb"] = emb.astype(np_bf16)

    shared["w_f1"] = np.ascontiguousarray(
        w("w_f1").reshape(NC3, 128, D)).astype(np_bf16)
    shared["w_f2"] = np.ascontiguousarray(
        w("w_f2").reshape(ND, 128, D)).astype(np_bf16)
    shared["b_f1"] = w("b_f1").reshape(1, D).astype(np_bf16)
    shared["b_f2"] = w("b_f2").reshape(1, D).astype(np_bf16)
    shared["ln_g"] = np.broadcast_to(w("ln_g").reshape(1, D),
                                     (BC, D)).copy()
    shared["ln_b"] = np.broadcast_to(w("ln_b").reshape(1, D),
                                     (BC, D)).copy()

    xbf = x.astype(np_bf16)
    in_maps = []
    core_idx = []
    for c in range(NCORES):
        idx = order[np.arange(BC) * NCORES + c]
        core_idx.append(idx)
        xs = xbf[idx]
        m = dict(shared)
        m["x"] = np.ascontiguousarray(xs)
        m["xT"] = np.ascontiguousarray(xs.transpose(2, 0, 1)).reshape(
            ND, 128, BC, L)
        m["fbias"] = np.ascontiguousarray(fb[idx])
        m["obias"] = np.ascontiguousarray(ob[idx])
        in_maps.append(m)
    return in_maps, pair_geo, core_idx


def get_program_fast(pair_geo):
    if pair_geo not in _PROGRAM_CACHE:
        _PROGRAM_CACHE[pair_geo] = build_program_fast(pair_geo)
    return _PROGRAM_CACHE[pair_geo]


def run(trace=False, **inputs):
    use_m = all(
        not np.any(np.asarray(inputs[n]))
        for n in ("b_sq", "b_sk", "b_cq", "b_ck", "b_rq", "b_rk")
    )
    if not use_m:
        return _run_legacy(trace=trace, **inputs)
    in_maps, pair_geo, core_idx = _host_prep_fast(inputs)
    import os
    fbnd = os.environ.get("FORCE_BOUNDS")
    if fbnd:
        f0, j0, f1, j1 = (int(v) for v in fbnd.split(","))
        pair_geo = ((f0, j0), (f1, j1))
    nc = get_program_fast(pair_geo)
    res = bass_utils.run_bass_kernel_spmd(
        nc, in_maps, core_ids=list(range(NCORES)), trace=trace
    )
    out = np.empty((B, D), np.float32)
    for c in range(NCORES):
        out[core_idx[c]] = res.results[c]["out"]
    return out, res


def kernel(**inputs):
    out, _ = run(trace=False, **inputs)
    return out


# ---------------------------------------------------------------------------
# Legacy fallback (nonzero projection biases): original per-slot program.
# ---------------------------------------------------------------------------

PROJ_NAMES = ["w_sq", "w_sk", "w_cq", "w_ck", "w_rq", "w_rk"]
PBIAS_NAMES = ["b_sq", "b_sk", "b_cq", "b_ck", "b_rq", "b_rk"]
QS, KS, QC, KC, QR, KR = range(6)
QPROJ = (QS, QC, QR)
_LEGACY_CACHE = {}


def build_program_legacy(bounds=((2, 2),) * BC):
    nc = bacc.Bacc(
        "TRN2",
        target_bir_lowering=False,
        debug=False,
        enable_asserts=False,
        num_devices=NCORES,
    )

    xT_d = nc.dram_tensor("xT", [BC, D, L], bf16, kind="ExternalInput").ap()
    x_d = nc.dram_tensor("x", [BC, L, D], f32, kind="ExternalInput").ap()
    fmask_d = nc.dram_tensor("fmask", [BC, L], f32, kind="ExternalInput").ap()
    obias_d = nc.dram_tensor("obias", [BC, L], bf16, kind="ExternalInput").ap()

    W_d = {p: nc.dram_tensor(PROJ_NAMES[p], [D, D], bf16, kind="ExternalInput").ap()
           for p in range(6)}
    Brow_d = {
        p: nc.dram_tensor(PBIAS_NAMES[p], [1, D], bf16, kind="ExternalInput").ap()
        for p in range(6)}
    wanom_d = nc.dram_tensor("w_anom", [D, 1], bf16, kind="ExternalInput").ap()
    wf1_d = nc.dram_tensor("w_f1", [ND, 128, NC3 * 128], bf16, kind="ExternalInput").ap()
    wf2_d = nc.dram_tensor("w_f2", [ND, 128, ND * 128], bf16, kind="ExternalInput").ap()
    bf1_d = nc.dram_tensor("b_f1", [128, ND], f32, kind="ExternalInput").ap()
    bf2_d = nc.dram_tensor("b_f2", [128, ND], f32, kind="ExternalInput").ap()
    lng_d = nc.dram_tensor("ln_g", [128, ND], f32, kind="ExternalInput").ap()
    lnb_d = nc.dram_tensor("ln_b", [128, ND], f32, kind="ExternalInput").ap()

    out_d = nc.dram_tensor("out", [BC, D], f32, kind="ExternalOutput").ap()

    with tile.TileContext(nc) as tc, ExitStack() as ctx:
        const_p = ctx.enter_context(tc.tile_pool(name="const", bufs=1))
        tmp_p = ctx.enter_context(tc.tile_pool(name="tmp", bufs=2))
        sm_p = ctx.enter_context(tc.tile_pool(name="small", bufs=3))
        tail_p = ctx.enter_context(tc.tile_pool(name="tail", bufs=1))
        ps_big = ctx.enter_context(tc.tile_pool(name="psb", bufs=4, space="PSUM"))
        ps_s = ctx.enter_context(tc.tile_pool(name="pss", bufs=4, space="PSUM"))
        es2 = ExitStack()   # closed after phase C: x, E
        x_p = es2.enter_context(tc.tile_pool(name="x", bufs=3))
        e_p = es2.enter_context(tc.tile_pool(name="emat", bufs=2))
        es1 = ExitStack()   # closed after phase B: xT, W, proj
        xT_p = es1.enter_context(tc.tile_pool(name="xT", bufs=1))
        w_p = es1.enter_context(tc.tile_pool(name="w", bufs=2))
        proj_p = es1.enter_context(tc.tile_pool(name="proj", bufs=1))

        ones_row = const_p.tile([1, L], bf16)
        nc.vector.memset(ones_row[:], 1.0)
        ones_f = const_p.tile([1, 128], f32)
        nc.vector.memset(ones_f[:], 1.0)
        ones_col = const_p.tile([128, 1], f32)
        nc.vector.memset(ones_col[:], 1.0)
        iot_t = const_p.tile([128, 128], mybir.dt.int32)
        nc.gpsimd.iota(iot_t[:], pattern=[[1, 128]], base=0, channel_multiplier=-1)
        ident_t = const_p.tile([128, 128], f32)
        nc.vector.tensor_scalar(ident_t[:], iot_t[:], scalar1=0, scalar2=None,
                                op0=ALU.is_equal)

        wanom_t = const_p.tile([128, ND], bf16)
        nc.scalar.dma_start(wanom_t[:], wanom_d[:, 0].rearrange("(k p) -> p k", p=128))
        brow_t = {}
        for p in Brow_d:
            brow_t[p] = const_p.tile([1, D], bf16, name=f"brow{p}")
            nc.sync.dma_start(brow_t[p][:], Brow_d[p][:])
        bf1_t = const_p.tile([128, ND], f32)
        nc.scalar.dma_start(bf1_t[:], bf1_d[:])
        bf2_t = const_p.tile([128, ND], f32)
        nc.scalar.dma_start(bf2_t[:], bf2_d[:])
        lng_t = const_p.tile([128, ND], f32)
        nc.scalar.dma_start(lng_t[:], lng_d[:])
        lnb_t = const_p.tile([128, ND], f32)
        nc.scalar.dma_start(lnb_t[:], lnb_d[:])

        fusedT = tail_p.tile([128, NC3, BC], bf16)

        geo = []
        for s in range(BC):
            F, J0 = bounds[s]
            geo.append((F, J0, F * 128, J0 * 128, L - J0 * 128,
                        F > 0 and L - J0 * 128 > 0))

        xT_t = xT_p.tile([128, BC * ND, L], bf16)
        fm_ts, ob_ts, x_ts = [], [], []
        for s in range(BC):
            nc.sync.dma_start(
                xT_t[:, s * ND : (s + 1) * ND, :],
                xT_d[s].rearrange("(k p) i -> p k i", p=128),
            )
            fm_t = sm_p.tile([128, NL], f32, tag="fm", bufs=BC, name=f"fm{s}")
            nc.scalar.dma_start(fm_t[:], fmask_d[s].rearrange("(t p) -> p t", p=128))
            fm_ts.append(fm_t)
            ob_t = sm_p.tile([1, L], bf16, tag="ob", bufs=2, name=f"ob{s}")
            nc.scalar.dma_start(ob_t[:], obias_d[s : s + 1, :])
            ob_ts.append(ob_t)

        gate_ts = []
        for s in range(BC):
            F, J0, CQ, OJ, NO, have_attn = geo[s]
            gate_t = sm_p.tile([128, NL], f32, tag="gate", bufs=BC, name=f"gate{s}")
            gate_ts.append(gate_t)
            if F == 0:
                continue
            ghat_t = sm_p.tile([128, NL], f32, tag="ghat")
            for it in range(F):
                al_ps = ps_s.tile([128, 1], f32, tag="pss")
                for k in range(ND):
                    nc.tensor.matmul(
                        al_ps[:],
                        lhsT=xT_t[:, s * ND + k, it * 128 : (it + 1) * 128],
                        rhs=wanom_t[:, k : k + 1],
                        start=(k == 0), stop=(k == ND - 1),
                    )
                eg_t = sm_p.tile([128, 1], f32, tag="eg")
                nc.scalar.activation(eg_t[:], al_ps[:], AF.Exp)
                nc.vector.tensor_mul(
                    ghat_t[:, it : it + 1], eg_t[:], fm_ts[s][:, it : it + 1]
                )
            gsum_t = sm_p.tile([128, 1], f32, tag="gsum")
            nc.vector.tensor_reduce(
                gsum_t[:], ghat_t[:, 0:F], axis=mybir.AxisListType.X, op=ALU.add
            )
            S_ps = ps_s.tile([1, 1], f32, tag="pss")
            nc.tensor.matmul(S_ps[:], lhsT=gsum_t[:], rhs=ones_col[:],
                             start=True, stop=True)
            Smax_t = sm_p.tile([1, 1], f32, tag="Smax")
            nc.vector.tensor_scalar_max(Smax_t[:], S_ps[:], 1e-8)
            Sb_ps = ps_s.tile([128, 1], f32, tag="pss")
            nc.tensor.matmul(Sb_ps[:], lhsT=ones_f[:], rhs=Smax_t[:],
                             start=True, stop=True)
            recipS_t = sm_p.tile([128, 1], f32, tag="recipS")
            nc.vector.reciprocal(recipS_t[:], Sb_ps[:])
            nc.vector.tensor_scalar_mul(gate_t[:, 0:F], ghat_t[:, 0:F],
                                        recipS_t[:])

        projs = [[None] * BC for _ in range(6)]
        for p in range(6):
            qside = p in QPROJ
            widths = [
                ((g[2] if qside else g[4]) if g[5] else 0) for g in geo
            ]
            wmax = max(widths)
            if wmax == 0:
                continue
            wt = w_p.tile([128, ND, D], bf16, tag="w", name=f"w{p}")
            nc.gpsimd.dma_start(wt[:], W_d[p].rearrange("(k p) c -> p k c", p=128))
            pt = proj_p.tile([128, BC, ND, wmax], bf16, tag=f"proj{p}")
            for m in range(ND):
                for s in range(BC):
                    width = widths[s]
                    if width == 0:
                        continue
                    lo = 0 if qside else geo[s][3]
                    ps = ps_big.tile([128, width], f32, tag="ps")
                    for k in range(ND):
                        nc.tensor.matmul(
                            ps[:], lhsT=wt[:, k, m * 128 : (m + 1) * 128],
                            rhs=xT_t[:, s * ND + k, lo : lo + width],
                            start=(k == 0), stop=False,
                        )
                    nc.tensor.matmul(
                        ps[:], lhsT=brow_t[p][:, m * 128 : (m + 1) * 128],
                        rhs=ones_row[:, 0:width], start=False, stop=True,
                    )
                    nc.vector.tensor_copy(pt[:, s, m, :], ps[:])
            for s in range(BC):
                if widths[s]:
                    projs[p][s] = pt

        for s in range(BC):
            x_t = x_p.tile([128, NL, D], f32, tag="x", name=f"x{s}")
            nc.sync.dma_start(x_t[:], x_d[s].rearrange("(t p) d -> p t d", p=128))
            x_ts.append(x_t)

        E_sups, E_reps, co_sups, co_reps = {}, {}, {}, {}
        for s in range(BC):
            F, J0, CQ, OJ, NO, have_attn = geo[s]
            if not have_attn:
                continue
            E_sup = e_p.tile([128, max(F, 1), NO], f32, tag="esup", bufs=BC,
                             name=f"esup{s}")
            E_rep = e_p.tile([128, max(F, 1), NO], f32, tag="erep", bufs=BC,
                             name=f"erep{s}")
            co_sup = sm_p.tile([128, NL], f32, tag="cosup", bufs=BC,
                               name=f"cosup{s}")
            co_rep = sm_p.tile([128, NL], f32, tag="corep", bufs=BC,
                               name=f"corep{s}")
            E_sups[s], E_reps[s] = E_sup, E_rep
            co_sups[s], co_reps[s] = co_sup, co_rep
            gate_t = gate_ts[s]
            ob_t = ob_ts[s]
            for it in range(F):
                isl = slice(it * 128, (it + 1) * 128)
                ps_sup = ps_big.tile([128, NO], f32, tag="ps")
                for k in range(ND):
                    nc.tensor.matmul(
                        ps_sup[:], lhsT=projs[QS][s][:, s, k, isl],
                        rhs=projs[KS][s][:, s, k, 0:NO],
                        start=(k == 0), stop=False,
                    )
                nc.tensor.matmul(ps_sup[:], lhsT=ones_row[:, 0:128],
                                 rhs=ob_t[:, OJ:L], start=False, stop=True)
                ps_con = ps_big.tile([128, NO], f32, tag="ps")
                for k in range(ND):
                    nc.tensor.matmul(
                        ps_con[:], lhsT=projs[QC][s][:, s, k, isl],
                        rhs=projs[KC][s][:, s, k, 0:NO],
                        start=(k == 0), stop=(k == ND - 1),
                    )
                ps_rep = ps_big.tile([128, NO], f32, tag="ps")
                for k in range(ND):
                    nc.tensor.matmul(
                        ps_rep[:], lhsT=projs[QR][s][:, s, k, isl],
                        rhs=projs[KR][s][:, s, k, 0:NO],
                        start=(k == 0), stop=False,
                    )
                nc.tensor.matmul(ps_rep[:], lhsT=ones_row[:, 0:128],
                                 rhs=ob_t[:, OJ:L], start=False, stop=True)

                T_t = tmp_p.tile([128, NO], f32, tag="T")
                nc.scalar.activation(T_t[:], ps_con[:], AF.Tanh, scale=SCALE)
                A_t = tmp_p.tile([128, NO], f32, tag="A")
                nc.vector.scalar_tensor_tensor(
                    A_t[:], in0=ps_rep[:], scalar=SCALE, in1=T_t[:],
                    op0=ALU.mult, op1=ALU.add,
                )
                rs_sup = sm_p.tile([128, 1], f32, tag="rssup")
                nc.scalar.activation(E_sup[:, it, :], ps_sup[:], AF.Exp,
                                     scale=SCALE, accum_out=rs_sup[:])
                rs_rep = sm_p.tile([128, 1], f32, tag="rsrep")
                nc.scalar.activation(E_rep[:, it, :], A_t[:], AF.Exp,
                                     accum_out=rs_rep[:])
                rc_sup = sm_p.tile([128, 1], f32, tag="rcsup")
                nc.vector.reciprocal(rc_sup[:], rs_sup[:])
                nc.vector.tensor_mul(co_sup[:, it : it + 1],
                                     gate_t[:, it : it + 1], rc_sup[:])
                rc_rep = sm_p.tile([128, 1], f32, tag="rcrep")
                nc.vector.reciprocal(rc_rep[:], rs_rep[:])
                nc.vector.tensor_mul(co_rep[:, it : it + 1],
                                     gate_t[:, it : it + 1], rc_rep[:])

        es1.close()

        for s in range(BC):
            F, J0, CQ, OJ, NO, have_attn = geo[s]
            x_t = x_ts[s]

            G_t = sm_p.tile([128, NL, 3], f32, tag="G")
            nc.vector.memset(G_t[:], 0.0)
            if F > 0:
                for it in range(F):
                    nc.vector.tensor_copy(G_t[:, it, 0:1],
                                          gate_ts[s][:, it : it + 1])
            if have_attn:
                E_sup, E_rep = E_sups[s], E_reps[s]
                co_sup, co_rep = co_sups[s], co_reps[s]
                for jt in range(J0, NL):
                    jsl = slice(jt * 128 - OJ, jt * 128 - OJ + 128)
                    r_ps = ps_s.tile([128, 2], f32, tag="pss")
                    for it in range(F):
                        nc.tensor.matmul(
                            r_ps[:, 0:1], lhsT=E_rep[:, it, jsl],
                            rhs=co_rep[:, it : it + 1],
                            start=(it == 0), stop=(it == F - 1),
                        )
                    for it in range(F):
                        nc.tensor.matmul(
                            r_ps[:, 1:2], lhsT=E_sup[:, it, jsl],
                            rhs=co_sup[:, it : it + 1],
                            start=(it == 0), stop=(it == F - 1),
                        )
                    nc.vector.tensor_copy(G_t[:, jt, 1:3], r_ps[:, 0:2])

            rts = sorted(set(range(F)) | (set(range(J0, NL)) if have_attn else set()))
            if not rts:
                rts = [0]
            for m in range(ND):
                pool_ps = ps_s.tile([128, 3], f32, tag="pss")
                for i, rt in enumerate(rts):
                    nc.tensor.matmul(
                        pool_ps[:], lhsT=x_t[:, rt, m * 128 : (m + 1) * 128],
                        rhs=G_t[:, rt, :],
                        start=(i == 0), stop=(i == len(rts) - 1),
                    )
                for t in range(3):
                    nc.vector.tensor_copy(
                        fusedT[:, t * ND + m, s : s + 1], pool_ps[:, t : t + 1]
                    )

        es2.close()

        wf1_p = ctx.enter_context(tc.tile_pool(name="wf1", bufs=8))
        hT_t = tail_p.tile([128, ND, BC], bf16)
        for m in range(ND):
            wt = wf1_p.tile([128, NC3, 128], bf16, tag="wf1")
            nc.gpsimd.dma_start(wt[:], wf1_d[m].rearrange("p (k c) -> p k c", c=128))
            h_ps = ps_s.tile([128, BC], f32, tag="pss")
            for k in range(NC3):
                nc.tensor.matmul(h_ps[:], lhsT=wt[:, k, :], rhs=fusedT[:, k, :],
                                 start=(k == 0), stop=(k == NC3 - 1))
            nc.scalar.activation(hT_t[:, m, :], h_ps[:], AF.Relu,
                                 bias=bf1_t[:, m : m + 1])

        yT_t = tail_p.tile([128, ND, BC], f32)
        sq_t = tail_p.tile([128, ND, BC], f32)
        for m in range(ND):
            wt = wf1_p.tile([128, ND, 128], bf16, tag="wf2")
            nc.gpsimd.dma_start(wt[:], wf2_d[m].rearrange("p (k c) -> p k c", c=128))
            y_ps = ps_s.tile([128, BC], f32, tag="pss")
            for k in range(ND):
                nc.tensor.matmul(y_ps[:], lhsT=wt[:, k, :], rhs=hT_t[:, k, :],
                                 start=(k == 0), stop=(k == ND - 1))
            nc.vector.tensor_scalar_add(yT_t[:, m, :], y_ps[:], bf2_t[:, m : m + 1])
            nc.scalar.square(sq_t[:, m, :], yT_t[:, m, :])

        sum_ps = ps_s.tile([1, BC], f32, tag="pss")
        for m in range(ND):
            nc.tensor.matmul(sum_ps[:], lhsT=ones_col[:], rhs=yT_t[:, m, :],
                             start=(m == 0), stop=(m == ND - 1))
        ssq_ps = ps_s.tile([1, BC], f32, tag="pss")
        for m in range(ND):
            nc.tensor.matmul(ssq_ps[:], lhsT=ones_col[:], rhs=sq_t[:, m, :],
                             start=(m == 0), stop=(m == ND - 1))
        mean_t = sm_p.tile([1, BC], f32, tag="mean")
        nc.scalar.mul(mean_t[:], sum_ps[:], 1.0 / D)
        msq_t = sm_p.tile([1, BC], f32, tag="msq")
        nc.scalar.mul(msq_t[:], ssq_ps[:], 1.0 / D)
        m2_t = sm_p.tile([1, BC], f32, tag="m2")
        nc.vector.tensor_mul(m2_t[:], mean_t[:], mean_t[:])
        var_t = sm_p.tile([1, BC], f32, tag="var")
        nc.vector.tensor_sub(var_t[:], msq_t[:], m2_t[:])
        nc.vector.tensor_scalar_add(var_t[:], var_t[:], LN_EPS)
        sd_t = sm_p.tile([1, BC], f32, tag="sd")
        nc.scalar.sqrt(sd_t[:], var_t[:])
        rstd_t = sm_p.tile([1, BC], f32, tag="rstd")
        nc.vector.reciprocal(rstd_t[:], sd_t[:])

        mb_ps = ps_s.tile([128, BC], f32, tag="pss")
        nc.tensor.matmul(mb_ps[:], lhsT=ones_f[:], rhs=mean_t[:],
                         start=True, stop=True)
        mb_t = sm_p.tile([128, BC], f32, tag="mbt")
        nc.vector.tensor_copy(mb_t[:], mb_ps[:])
        rb_ps = ps_s.tile([128, BC], f32, tag="pss")
        nc.tensor.matmul(rb_ps[:], lhsT=ones_f[:], rhs=rstd_t[:],
                         start=True, stop=True)
        rb_t = sm_p.tile([128, BC], f32, tag="rbt")
        nc.vector.tensor_copy(rb_t[:], rb_ps[:])

        zrow_t = tail_p.tile([BC, D], f32)
        for m in range(ND):
            z_t = tmp_p.tile([128, BC], f32, tag="z")
            nc.vector.tensor_sub(z_t[:], yT_t[:, m, :], mb_t[:])
            nc.vector.tensor_mul(z_t[:], z_t[:], rb_t[:])
            z2_t = tmp_p.tile([128, BC], f32, tag="z2")
            nc.vector.tensor_scalar(
                z2_t[:], z_t[:], scalar1=lng_t[:, m : m + 1],
                scalar2=lnb_t[:, m : m + 1], op0=ALU.mult, op1=ALU.add,
            )
            tr_ps = ps_s.tile([BC, 128], f32, tag="pss")
            nc.tensor.transpose(tr_ps[:], z2_t[:], ident_t[:])
            nc.vector.tensor_copy(zrow_t[:, m * 128 : (m + 1) * 128], tr_ps[:])
        nc.sync.dma_start(out_d[:, :], zrow_t[:, :])

    nc.compile()
    return nc


def _run_legacy(trace=False, **inputs):
    x = np.asarray(inputs["x"], dtype=np.float32)
    x_ids = np.asarray(inputs["x_ids"])
    pad_idx = int(np.asarray(inputs["pad_idx"]))
    sep_idx = int(np.asarray(inputs["sep_idx"]))

    sp, fmask_b, omask = _masks(x_ids, pad_idx, sep_idx)
    fmask = fmask_b.astype(np.float32)
    obias = np.where(omask, 0.0, OBIAS_RAW).astype(np.float32)

    F_all = np.ceil(sp / 128).astype(int)
    J0_all = np.minimum((sp + 1) // 128, NL)
    bounds = tuple(
        (int(F_all.reshape(NCORES, BC)[:, s].max()),
         int(J0_all.reshape(NCORES, BC)[:, s].min()))
        for s in range(BC)
    )

    xT = np.ascontiguousarray(x.transpose(0, 2, 1))

    def w(name):
        return np.ascontiguousarray(np.asarray(inputs[name], dtype=np.float32))

    def ppart(name):
        return np.ascontiguousarray(np.asarray(inputs[name], dtype=np.float32)
                                    .reshape(ND, 128).T)

    shared = {}
    for p in range(6):
        shared[PROJ_NAMES[p]] = w(PROJ_NAMES[p]).astype(np_bf16)
        shared[PBIAS_NAMES[p]] = w(PBIAS_NAMES[p]).reshape(1, D).astype(np_bf16)
    shared["w_anom"] = w("w_anom").reshape(D, 1).astype(np_bf16)

    def mpack(name, nk):
        a = w(name)
        a = a.reshape(nk, 128, ND, 128).transpose(2, 1, 0, 3).reshape(ND, 128, nk * 128)
        return np.ascontiguousarray(a).astype(np_bf16)

    shared["w_f1"] = mpack("w_f1", NC3)
    shared["w_f2"] = mpack("w_f2", ND)
    shared["b_f1"] = ppart("b_f1")
    shared["b_f2"] = ppart("b_f2")
    shared["ln_g"] = ppart("ln_g")
    shared["ln_b"] = ppart("ln_b")

    in_maps = []
    for c in range(NCORES):
        sl = slice(c * BC, (c + 1) * BC)
        m = dict(shared)
        m["x"] = np.ascontiguousarray(x[sl])
        m["xT"] = np.ascontiguousarray(xT[sl]).astype(np_bf16)
        m["fmask"] = np.ascontiguousarray(fmask[sl])
        m["obias"] = np.ascontiguousarray(obias[sl]).astype(np_bf16)
        in_maps.append(m)

    if bounds not in _LEGACY_CACHE:
        _LEGACY_CACHE[bounds] = build_program_legacy(bounds)
    nc = _LEGACY_CACHE[bounds]
    res = bass_utils.run_bass_kernel_spmd(
        nc, in_maps, core_ids=list(range(NCORES)), trace=trace
    )
    out = np.concatenate([res.results[c]["out"] for c in range(NCORES)], axis=0)
    return out.astype(np.float32), res


# revision 15
# speedup vs baseline: 1.9998x; 1.3321x over previous
"""Trainium2 Bass kernel for nn_BertCounterFactTransformer.

Contract: kernel(**inputs) takes FULL unsharded numpy inputs (as produced by
reference.setup_inputs()) and returns the FULL [32, 1024] float32 output.

Strategy (data-parallel over batch, 8 cores x 4 samples):
  - Host: compute sep positions from x_ids, SORT samples by sep position and
    assign sorted rank r -> core (r % 8), slot (r // 8) so the per-slot-pair
    tile bounds are tight and uniform across cores. Precompute
    M_p = W_pq @ W_pk^T (bf16) so scores are x M x^T (no k-side projection).
  - Device, per pair of slots (F tiles of false rows, option cols from OJ):
      gate       all-4-sample anomaly logits in ONE [4, wg] PSUM via
                 block-diagonal embedded w_anom; false-mask folded as a
                 -30 bias row; exp+normalize row-wise; PE-transpose to cols
      proj       qT = (x M_p)^T (or M_p x_opt^T if the option side is
                 smaller), 2 samples batched per matmul (width<=512)
      scores     S = q @ x_opt^T blocks; option mask via -960 bias rows
      E_sup = exp(S_sup/32 + ob), E_rep = exp(S_rep/32 + tanh(S_con/32) + ob)
      coeff_t = gate / rowsum(E_t);  r_t = E_t^T @ coeff_t   (width-1 chains)
  - Pool: ONE [12, 512] PSUM accumulates x_s^T @ [gate|r_rep|r_sup] for all
    4 samples via 12-col G with per-sample zero blocks; PE-transpose the
    [12, 1024] result into fused^T columns.
  - Tail in row form: h = relu(fused @ W1 + b1), y = h @ W2 + b2, LayerNorm
    along the free dim, direct [4, 1024] output DMA.

Key identity: gate @ (attn @ x) == (gate @ attn) @ x, so [L,D] attention
outputs are never materialized.
"""

import sys

if "/opt/trn_rl_repo" not in sys.path:
    sys.path.insert(0, "/opt/trn_rl_repo")

import numpy as np
import ml_dtypes
from contextlib import ExitStack

np_bf16 = ml_dtypes.bfloat16
np_fp8 = ml_dtypes.float8_e4m3

import concourse.bacc as bacc
import concourse.bass as bass
import concourse.mybir as mybir
import concourse.tile as tile
from concourse import bass_utils

f32 = mybir.dt.float32
bf16 = mybir.dt.bfloat16
fp8 = mybir.dt.float8e4
AF = mybir.ActivationFunctionType
ALU = mybir.AluOpType

B, L, D = 32, 512, 1024
NCORES = 8
BC = B // NCORES          # samples per core
NL = L // 128             # 4 L-tiles
ND = D // 128             # 8 D-tiles
NC3 = 3 * D // 128        # 24 tiles of the 3D fused dim
SCALE = 1.0 / 32.0        # 1/sqrt(D)
OBIAS_RAW = -960.0        # -30 after * SCALE
FBIAS = -30.0
LN_EPS = 1e-5
USE_FP8 = True            # fp8 e4m3 + DoubleRow for the projection GEMMs
FP8_SCORES = False        # keep score operands (projT, xo) in bf16
PRE = 64.0 if USE_FP8 else 1.0   # pre-scale on M/w_anom (fp8 normal range)

_PROGRAM_CACHE = {}
_M_CACHE = {}


def _m_matrix(wq, wk, transposed=False):
    import hashlib
    wq = np.asarray(wq, dtype=np.float32)
    wk = np.asarray(wk, dtype=np.float32)
    key = (hashlib.blake2b(wq.tobytes() + wk.tobytes(), digest_size=16).digest(),
           transposed, USE_FP8)
    if key not in _M_CACHE:
        m = wq @ wk.T
        if transposed:
            m = m.T
        m = np.ascontiguousarray(m)
        if USE_FP8:
            _M_CACHE[key] = np.clip(m * PRE, -240, 240).astype(np_fp8)
        else:
            _M_CACHE[key] = m.astype(np_bf16)
    return _M_CACHE[key]


def _geo(F, J0):
    OJ = J0 * 128
    NO = L - OJ
    CQ = F * 128
    have = NO > 0
    side_q = (CQ <= NO) if have else True
    w = (CQ if side_q else NO) if have else 0
    lo = 0 if side_q else OJ
    return dict(F=F, J0=J0, OJ=OJ, NO=NO, CQ=CQ, have=have,
                side_q=side_q, w=w, lo=lo)


def build_program_fast(pair_geo):
    """pair_geo = ((F0, J0_0), (F1, J0_1)); pair p covers slots {2p, 2p+1}.
    Computing a superset is always correct (bias masks zero it)."""
    nc = bacc.Bacc(
        "TRN2",
        target_bir_lowering=False,
        debug=False,
        enable_asserts=False,
        num_devices=NCORES,
    )

    geos = [_geo(F, J0) for (F, J0) in pair_geo]
    need_mt = any(g["have"] and not g["side_q"] for g in geos)
    wg = max(max(g["CQ"] for g in geos), 128)       # gate width (cols 0..wg)
    maxF = max(g["F"] for g in geos)
    DT_X = fp8 if USE_FP8 else bf16
    DT_SC = fp8 if (USE_FP8 and FP8_SCORES) else bf16
    need_xqb = DT_SC != DT_X       # gate chains + k-side score lhsT run bf16
    CQs = [geos[s // 2]["CQ"] for s in range(BC)]
    NOs = [geos[s // 2]["NO"] for s in range(BC)]
    offq = [sum(CQs[:s]) for s in range(BC)]
    offo = [sum(NOs[:s]) for s in range(BC)]
    SQ, SO = sum(CQs), sum(NOs)

    xq_d = nc.dram_tensor("xq", [ND, 128, SQ], DT_X, kind="ExternalInput").ap()
    xqb_d = (nc.dram_tensor("xqb", [ND, 128, SQ], DT_SC,
                            kind="ExternalInput").ap() if need_xqb else None)
    need_xo8 = (DT_SC != DT_X) and any(
        g["have"] and not g["side_q"] for g in geos)
    xo_d = (nc.dram_tensor("xo", [ND, 128, SO], DT_SC, kind="ExternalInput").ap()
            if SO else None)
    xo8_d = (nc.dram_tensor("xo8", [ND, 128, SO], DT_X,
                            kind="ExternalInput").ap()
             if (SO and need_xo8) else None)
    x_d = nc.dram_tensor("x", [BC, L, D], bf16, kind="ExternalInput").ap()
    fb_d = nc.dram_tensor("fbias", [BC, L], bf16, kind="ExternalInput").ap()
    ob_d = nc.dram_tensor("obias", [BC, L], bf16, kind="ExternalInput").ap()
    m_d = [nc.dram_tensor(f"m{p}", [D, D], DT_X, kind="ExternalInput").ap()
           for p in range(3)]
    mt_d = ([nc.dram_tensor(f"mt{p}", [D, D], DT_X, kind="ExternalInput").ap()
             for p in range(3)] if need_mt else None)
    emb_d = nc.dram_tensor("wanom_emb", [128, ND, BC, BC], DT_SC,
                           kind="ExternalInput").ap()
    w1_d = nc.dram_tensor("w_f1", [NC3, 128, D], bf16, kind="ExternalInput").ap()
    w2_d = nc.dram_tensor("w_f2", [ND, 128, D], bf16, kind="ExternalInput").ap()
    b1_d = nc.dram_tensor("b_f1", [1, D], bf16, kind="ExternalInput").ap()
    b2_d = nc.dram_tensor("b_f2", [1, D], bf16, kind="ExternalInput").ap()
    lng_d = nc.dram_tensor("ln_g", [BC, D], f32, kind="ExternalInput").ap()
    lnb_d = nc.dram_tensor("ln_b", [BC, D], f32, kind="ExternalInput").ap()
    out_d = nc.dram_tensor("out", [BC, D], f32, kind="ExternalOutput").ap()

    with tile.TileContext(nc) as tc, ExitStack() as ctx:
        const_p = ctx.enter_context(tc.tile_pool(name="const", bufs=1))
        main_p = ctx.enter_context(tc.tile_pool(name="main", bufs=1))
        sm_p = ctx.enter_context(tc.tile_pool(name="small", bufs=2))
        tmp_p = ctx.enter_context(tc.tile_pool(name="tmp", bufs=2))
        ps_big = ctx.enter_context(tc.tile_pool(name="psb", bufs=4, space="PSUM"))
        ps_med = ctx.enter_context(tc.tile_pool(name="psm", bufs=2, space="PSUM"))
        ps_sm = ctx.enter_context(tc.tile_pool(name="pss", bufs=2, space="PSUM"))
        es2 = ExitStack()   # E matrices; closed after last r
        e_p = es2.enter_context(tc.tile_pool(name="emat", bufs=1))
        es1 = ExitStack()   # W matrices + projT; closed after last scores
        w_p = es1.enter_context(tc.tile_pool(name="w", bufs=1))
        proj_p = es1.enter_context(tc.tile_pool(name="proj", bufs=1))

        # ---- constants ----
        ones_row = const_p.tile([1, 128], bf16)
        nc.vector.memset(ones_row[:], 1.0)
        ones4 = const_p.tile([1, BC], bf16)
        nc.vector.memset(ones4[:], 1.0)
        eyerows = const_p.tile([1, BC, BC], bf16)
        nc.vector.memset(eyerows[:], 0.0)
        for s in range(BC):
            nc.vector.memset(eyerows[:, s, s : s + 1], 1.0)
        iot_t = const_p.tile([128, 128], mybir.dt.int32)
        nc.gpsimd.iota(iot_t[:], pattern=[[1, 128]], base=0, channel_multiplier=-1)
        ident_f = const_p.tile([128, 128], f32)
        nc.vector.tensor_scalar(ident_f[:], iot_t[:], scalar1=0, scalar2=None,
                                op0=ALU.is_equal)
        ident_b = const_p.tile([128, 128], bf16)
        nc.vector.tensor_copy(ident_b[:], ident_f[:])

        emb_t = const_p.tile([128, ND, BC, BC], DT_SC)
        nc.scalar.dma_start(emb_t[:], emb_d[:])
        fb_t = const_p.tile([1, BC, L], bf16)
        ob_t = const_p.tile([1, BC, L], bf16)
        for s in range(BC):
            nc.scalar.dma_start(fb_t[:, s, :], fb_d[s : s + 1, :])
            nc.scalar.dma_start(ob_t[:, s, :], ob_d[s : s + 1, :])
        b1_t = const_p.tile([1, D], bf16)
        nc.scalar.dma_start(b1_t[:], b1_d[:])
        b2_t = const_p.tile([1, D], bf16)
        nc.scalar.dma_start(b2_t[:], b2_d[:])
        lng_t = const_p.tile([BC, D], f32)
        nc.scalar.dma_start(lng_t[:], lng_d[:])
        lnb_t = const_p.tile([BC, D], f32)
        nc.scalar.dma_start(lnb_t[:], lnb_d[:])

        # ---- big inputs ----
        xq_t = main_p.tile([128, ND, SQ], DT_X)
        for k in range(ND):
            nc.sync.dma_start(xq_t[:, k, :], xq_d[k])
        xqb_t = xq_t
        if need_xqb:
            xqb_t = main_p.tile([128, ND, SQ], DT_SC)
            for k in range(ND):
                nc.sync.dma_start(xqb_t[:, k, :], xqb_d[k])
        xo_t = None
        if SO:
            xo_t = main_p.tile([128, ND, SO], DT_SC)
            for k in range(ND):
                nc.sync.dma_start(xo_t[:, k, :], xo_d[k])
        xo8_t = xo_t
        if SO and need_xo8:
            xo8_t = main_p.tile([128, ND, SO], DT_X)
            for k in range(ND):
                nc.sync.dma_start(xo8_t[:, k, :], xo8_d[k])
        w_ts = []
        for p in range(3):
            wt = w_p.tile([128, ND, D], DT_X, name=f"w{p}")
            nc.gpsimd.dma_start(wt[:], m_d[p].rearrange("(k p) c -> p k c", p=128))
            w_ts.append(wt)
        wt_ts = []
        if need_mt:
            for p in range(3):
                wtt = w_p.tile([128, ND, D], DT_X, name=f"wt{p}")
                nc.gpsimd.dma_start(
                    wtt[:], mt_d[p].rearrange("(k p) c -> p k c", p=128)
                )
                wt_ts.append(wtt)
        x_t = main_p.tile([128, BC, NL, D], bf16)
        for s in range(BC):
            nc.sync.dma_start(
                x_t[:, s], x_d[s].rearrange("(t p) d -> p t d", p=128)
            )

        G_all = main_p.tile([128, NL, BC, 12], bf16)   # col = t*4 + s
        nc.vector.memset(G_all[:], 0.0)
        gcol = main_p.tile([128, maxF, BC], f32)

        # ---- gates (all samples): logits in ONE [4, wg] PSUM ----
        ga_ps = ps_med.tile([BC, wg], f32, tag="pm")
        for s in range(BC):
            for k in range(ND):
                nc.tensor.matmul(
                    ga_ps[:, 0 : CQs[s]], lhsT=emb_t[:, k, s, :],
                    rhs=xqb_t[:, k, offq[s] : offq[s] + CQs[s]],
                    start=(s == 0 and k == 0), stop=False,
                )
        for s in range(BC):
            nc.tensor.matmul(
                ga_ps[:], lhsT=eyerows[:, s, :], rhs=fb_t[:, s, 0:wg],
                start=False, stop=(s == BC - 1),
            )
        grow = sm_p.tile([BC, wg], f32, tag="grow", bufs=1)
        gs_t = sm_p.tile([BC, 1], f32, tag="gs", bufs=1)
        nc.scalar.activation(grow[:], ga_ps[:], AF.Exp, scale=1.0 / PRE,
                             accum_out=gs_t[:])
        nc.vector.tensor_scalar_max(gs_t[:], gs_t[:], 1e-8)
        rg_t = sm_p.tile([BC, 1], f32, tag="rg", bufs=1)
        nc.vector.reciprocal(rg_t[:], gs_t[:])
        nc.vector.tensor_scalar_mul(grow[:], grow[:], rg_t[:])
        for it in range(maxF):
            gt_ps = ps_med.tile([128, BC], f32, tag="pm")
            nc.tensor.transpose(gt_ps[:], grow[:, it * 128 : (it + 1) * 128],
                                ident_f[0:BC, 0:BC])
            nc.vector.tensor_copy(gcol[:, it, :], gt_ps[:])
            for s in range(BC):
                if it < geos[s // 2]["F"]:
                    nc.vector.tensor_copy(G_all[:, it, s, s : s + 1],
                                          gt_ps[:, s : s + 1])

        # ---- per pair: projections -> scores -> E -> coeffs -> r ----
        for pr in range(2):
            g = geos[pr]
            F, OJ, NO, w, lo = g["F"], g["OJ"], g["NO"], g["w"], g["lo"]
            if not g["have"]:
                continue
            s0, s1 = 2 * pr, 2 * pr + 1
            wmats = w_ts if g["side_q"] else wt_ts

            xsrc = xq_t if g["side_q"] else xo8_t
            poff = offq if g["side_q"] else offo
            assert poff[s1] == poff[s0] + w

            projT = [
                proj_p.tile([128, ND, 2, w], DT_SC, tag=f"pj{p}",
                            name=f"pj{p}_{pr}")
                for p in range(3)
            ]
            DR = mybir.MatmulPerfMode.DoubleRow if USE_FP8 else None
            NK = ND // 2 if USE_FP8 else ND
            for p in range(3):
                for m in range(ND):
                    msl = slice(m * 128, (m + 1) * 128)
                    if 2 * w <= 512:
                        pj_ps = ps_big.tile([128, 2, w], f32, tag="ps")
                        for k in range(NK):
                            if USE_FP8:
                                nc.tensor.matmul(
                                    pj_ps[:],
                                    lhsT=wmats[p][:, 2 * k : 2 * k + 2, msl],
                                    rhs=xsrc[:, 2 * k : 2 * k + 2,
                                             poff[s0] : poff[s0] + 2 * w],
                                    start=(k == 0), stop=(k == NK - 1),
                                    perf_mode=DR,
                                )
                            else:
                                nc.tensor.matmul(
                                    pj_ps[:],
                                    lhsT=wmats[p][:, k, msl],
                                    rhs=xsrc[:, k, poff[s0] : poff[s0] + 2 * w],
                                    start=(k == 0), stop=(k == NK - 1),
                                )
                        if m % 2 == 0:
                            nc.vector.tensor_copy(projT[p][:, m], pj_ps[:])
                        else:
                            nc.scalar.activation(projT[p][:, m], pj_ps[:],
                                                 AF.Copy)
                    else:
                        for sp in range(2):
                            pj_ps = ps_big.tile([128, w], f32, tag="ps")
                            for k in range(NK):
                                if USE_FP8:
                                    nc.tensor.matmul(
                                        pj_ps[:],
                                        lhsT=wmats[p][:, 2 * k : 2 * k + 2, msl],
                                        rhs=xsrc[:, 2 * k : 2 * k + 2,
                                                 poff[s0 + sp] :
                                                 poff[s0 + sp] + w],
                                        start=(k == 0), stop=(k == NK - 1),
                                        perf_mode=DR,
                                    )
                                else:
                                    nc.tensor.matmul(
                                        pj_ps[:],
                                        lhsT=wmats[p][:, k, msl],
                                        rhs=xsrc[:, k,
                                                 poff[s0 + sp] :
                                                 poff[s0 + sp] + w],
                                        start=(k == 0), stop=(k == NK - 1),
                                    )
                            if sp == 0:
                                nc.vector.tensor_copy(projT[p][:, m, sp],
                                                      pj_ps[:])
                            else:
                                nc.scalar.activation(projT[p][:, m, sp],
                                                     pj_ps[:], AF.Copy)

            for sp in range(2):
                s4 = s0 + sp
                E_sup = e_p.tile([128, F, NO], bf16, tag=f"Es{sp}",
                                 name=f"Es{sp}_{pr}")
                E_rep = e_p.tile([128, F, NO], bf16, tag=f"Er{sp}",
                                 name=f"Er{sp}_{pr}")
                co_sup = sm_p.tile([128, F], bf16, tag=f"cos{sp}", bufs=1,
                                   name=f"cos{sp}_{pr}")
                co_rep = sm_p.tile([128, F], bf16, tag=f"cor{sp}", bufs=1,
                                   name=f"cor{sp}_{pr}")
                for it in range(F):
                    isl = slice(it * 128, (it + 1) * 128)
                    ps3 = {}
                    for p in (0, 2, 1):   # sup, rep, con
                        ps = ps_big.tile([128, NO], f32, tag="ps",
                                         name=f"sc{p}")
                        ps3[p] = ps
                        for k in range(ND):
                            if g["side_q"]:
                                lhsT = projT[p][:, k, sp, isl]
                                rhs = xo_t[:, k, offo[s4] : offo[s4] + NO]
                            else:
                                lhsT = xqb_t[:, k,
                                             offq[s4] + it * 128 :
                                             offq[s4] + (it + 1) * 128]
                                rhs = projT[p][:, k, sp, 0:NO]
                            nc.tensor.matmul(ps[:], lhsT=lhsT, rhs=rhs,
                                             start=(k == 0),
                                             stop=(k == ND - 1 and p == 1))
                        if p != 1:
                            # option-mask bias row closes the sup/rep groups
                            nc.tensor.matmul(ps[:], lhsT=ones_row[:],
                                             rhs=ob_t[:, s4, OJ:L],
                                             start=False, stop=True)
                    ps_sup, ps_rep, ps_con = ps3[0], ps3[2], ps3[1]

                    T_t = tmp_p.tile([128, NO], f32, tag="T")
                    nc.scalar.activation(T_t[:], ps_con[:], AF.Tanh,
                                         scale=SCALE / PRE)
                    A_t = tmp_p.tile([128, NO], f32, tag="A")
                    nc.vector.scalar_tensor_tensor(
                        A_t[:], in0=ps_rep[:], scalar=SCALE / PRE, in1=T_t[:],
                        op0=ALU.mult, op1=ALU.add,
                    )
                    rs_sup = sm_p.tile([128, 1], f32, tag="rss")
                    nc.scalar.activation(E_sup[:, it], ps_sup[:], AF.Exp,
                                         scale=SCALE / PRE, accum_out=rs_sup[:])
                    rs_rep = sm_p.tile([128, 1], f32, tag="rsr")
                    nc.scalar.activation(E_rep[:, it], A_t[:], AF.Exp,
                                         accum_out=rs_rep[:])
                    rc_sup = sm_p.tile([128, 1], f32, tag="rcs")
                    nc.vector.reciprocal(rc_sup[:], rs_sup[:])
                    nc.vector.tensor_mul(co_sup[:, it : it + 1],
                                         gcol[:, it, s4 : s4 + 1], rc_sup[:])
                    rc_rep = sm_p.tile([128, 1], f32, tag="rcr")
                    nc.vector.reciprocal(rc_rep[:], rs_rep[:])
                    nc.vector.tensor_mul(co_rep[:, it : it + 1],
                                         gcol[:, it, s4 : s4 + 1], rc_rep[:])

                # r vectors: G col 4+s (rep), 8+s (sup)
                for t, (E_t, co_t) in enumerate(((E_rep, co_rep),
                                                 (E_sup, co_sup))):
                    for jt in range(NO // 128):
                        jsl = slice(jt * 128, (jt + 1) * 128)
                        r_ps = ps_sm.tile([128, 1], f32, tag="r")
                        for it in range(F):
                            nc.tensor.matmul(
                                r_ps[:], lhsT=E_t[:, it, jsl],
                                rhs=co_t[:, it : it + 1],
                                start=(it == 0), stop=(it == F - 1),
                            )
                        nc.vector.tensor_copy(
                            G_all[:, g["J0"] + jt, s4,
                                  4 * (t + 1) + s4 : 4 * (t + 1) + s4 + 1],
                            r_ps[:],
                        )

        es1.close()
        es2.close()

        # ---- tail weights (after W matrices freed) ----
        tail_p = ctx.enter_context(tc.tile_pool(name="tail", bufs=1))
        w1_t = tail_p.tile([128, NC3, D], bf16)
        for t in range(3):
            nc.gpsimd.dma_start(
                w1_t[:, t * ND : (t + 1) * ND, :],
                w1_d[t * ND : (t + 1) * ND].rearrange("k p n -> p k n"),
            )
        w2_t = tail_p.tile([128, ND, D], bf16)
        nc.gpsimd.dma_start(w2_t[:], w2_d.rearrange("k p n -> p k n"))

        # ---- pool: pooled[t*4+s, :] = sum_l G[l, t*4+s] * x_s[l, :] ----
        seq = []
        for s4 in range(BC):
            g = geos[s4 // 2]
            rts = sorted(set(range(g["F"]))
                         | (set(range(g["J0"], NL)) if g["have"] else set()))
            seq.extend((s4, rt) for rt in rts)
        pooled_sb = main_p.tile([12, D], bf16)
        for half in range(2):
            hs = slice(half * 512, (half + 1) * 512)
            po_ps = ps_med.tile([12, 512], f32, tag="pm")
            for i, (s4, rt) in enumerate(seq):
                nc.tensor.matmul(
                    po_ps[:], lhsT=G_all[:, rt, s4, :], rhs=x_t[:, s4, rt, hs],
                    start=(i == 0), stop=(i == len(seq) - 1),
                )
            nc.vector.tensor_copy(pooled_sb[:, hs], po_ps[:])

        fused_sb = main_p.tile([128, ND, 3, BC], bf16)
        for m in range(ND):
            tr_ps = ps_sm.tile([128, 12], bf16, tag="r")
            nc.tensor.transpose(tr_ps[:], pooled_sb[:, m * 128 : (m + 1) * 128],
                                ident_b[0:12, 0:12])
            nc.vector.tensor_copy(fused_sb[:, m], tr_ps[:])

        # ---- MLP tail + LayerNorm, all in [4, 1024] row form ----
        h_sb = main_p.tile([BC, D], bf16)
        for half in range(2):
            hs = slice(half * 512, (half + 1) * 512)
            h_ps = ps_big.tile([BC, 512], f32, tag="ps")
            for t in range(3):
                for m in range(ND):
                    nc.tensor.matmul(
                        h_ps[:], lhsT=fused_sb[:, m, t, :],
                        rhs=w1_t[:, t * ND + m, hs],
                        start=(t == 0 and m == 0), stop=False,
                    )
            nc.tensor.matmul(h_ps[:], lhsT=ones4[:], rhs=b1_t[:, hs],
                             start=False, stop=True)
            nc.scalar.activation(h_sb[:, hs], h_ps[:], AF.Relu)

        hT_sb = main_p.tile([128, ND, BC], bf16)
        for m in range(ND):
            ht_ps = ps_sm.tile([128, BC], bf16, tag="r")
            nc.tensor.transpose(ht_ps[:], h_sb[:, m * 128 : (m + 1) * 128],
                                ident_b[0:BC, 0:BC])
            nc.vector.tensor_copy(hT_sb[:, m], ht_ps[:])

        y_sb = main_p.tile([BC, D], f32)
        s1_t = sm_p.tile([BC, 2], f32, tag="s1", bufs=1)
        s2_t = sm_p.tile([BC, 2], f32, tag="s2", bufs=1)
        sq_sb = tmp_p.tile([BC, 512], f32, tag="sq", bufs=2)
        for half in range(2):
            hs = slice(half * 512, (half + 1) * 512)
            y_ps = ps_big.tile([BC, 512], f32, tag="ps")
            for k in range(ND):
                nc.tensor.matmul(y_ps[:], lhsT=hT_sb[:, k, :],
                                 rhs=w2_t[:, k, hs],
                                 start=(k == 0), stop=False)
            nc.tensor.matmul(y_ps[:], lhsT=ones4[:], rhs=b2_t[:, hs],
                             start=False, stop=True)
            nc.scalar.activation(y_sb[:, hs], y_ps[:], AF.Copy,
                                 accum_out=s1_t[:, half : half + 1])
            sq = tmp_p.tile([BC, 512], f32, tag="sq", bufs=2)
            nc.scalar.activation(sq[:], y_ps[:], AF.Square,
                                 accum_out=s2_t[:, half : half + 1])

        mu_t = sm_p.tile([BC, 1], f32, tag="mu", bufs=1)
        nc.vector.tensor_reduce(mu_t[:], s1_t[:], axis=mybir.AxisListType.X,
                                op=ALU.add)
        nc.scalar.mul(mu_t[:], mu_t[:], 1.0 / D)
        msq_t = sm_p.tile([BC, 1], f32, tag="msq", bufs=1)
        nc.vector.tensor_reduce(msq_t[:], s2_t[:], axis=mybir.AxisListType.X,
                                op=ALU.add)
        nc.scalar.mul(msq_t[:], msq_t[:], 1.0 / D)
        m2_t = sm_p.tile([BC, 1], f32, tag="m2", bufs=1)
        nc.vector.tensor_mul(m2_t[:], mu_t[:], mu_t[:])
        var_t = sm_p.tile([BC, 1], f32, tag="var", bufs=1)
        nc.vector.tensor_sub(var_t[:], msq_t[:], m2_t[:])
        nc.vector.tensor_scalar_add(var_t[:], var_t[:], LN_EPS)
        sd_t = sm_p.tile([BC, 1], f32, tag="sd", bufs=1)
        nc.scalar.sqrt(sd_t[:], var_t[:])
        rstd_t = sm_p.tile([BC, 1], f32, tag="rstd", bufs=1)
        nc.vector.reciprocal(rstd_t[:], sd_t[:])

        z_sb = main_p.tile([BC, D], f32)
        nc.vector.tensor_scalar(z_sb[:], y_sb[:], scalar1=mu_t[:],
                                scalar2=rstd_t[:], op0=ALU.subtract,
                                op1=ALU.mult)
        nc.vector.tensor_mul(z_sb[:], z_sb[:], lng_t[:])
        nc.vector.tensor_add(z_sb[:], z_sb[:], lnb_t[:])
        nc.sync.dma_start(out_d[:], z_sb[:])

    nc.compile()
    return nc


def _masks(x_ids, pad_idx, sep_idx):
    valid = x_ids != pad_idx
    sepm = x_ids == sep_idx
    has = sepm.any(axis=1)
    first = sepm.argmax(axis=1)
    vlen = valid.sum(axis=1)
    fb = np.clip(vlen // 2, 1, max(1, L - 2))
    sp = np.where(has, first, fb)
    pos = np.arange(L)
    fmask = (pos[None, :] < sp[:, None]) & valid
    omask = (pos[None, :] > sp[:, None]) & valid
    return sp, fmask, omask


def _host_prep_fast(inputs):
    import os

    x = np.asarray(inputs["x"], dtype=np.float32)
    x_ids = np.asarray(inputs["x_ids"])
    pad_idx = int(np.asarray(inputs["pad_idx"]))
    sep_idx = int(np.asarray(inputs["sep_idx"]))
    assert x.shape == (B, L, D), x.shape
    np_x = np_fp8 if USE_FP8 else np_bf16

    sp, fmask, omask = _masks(x_ids, pad_idx, sep_idx)
    fb = np.where(fmask, 0.0, FBIAS * PRE).astype(np_bf16)
    ob = np.where(omask, 0.0, OBIAS_RAW * PRE).astype(np_bf16)

    order = np.argsort(-sp, kind="stable")
    F_all = np.ceil(sp / 128).astype(int)
    J0_all = np.minimum((sp + 1) // 128, NL)
    pair_geo = tuple(
        (int(F_all[order[pr * 16 : (pr + 1) * 16]].max()),
         int(J0_all[order[pr * 16 : (pr + 1) * 16]].min()))
        for pr in range(2)
    )
    fbnd = os.environ.get("FORCE_BOUNDS")
    if fbnd:
        f0, j0, f1, j1 = (int(v) for v in fbnd.split(","))
        pair_geo = ((f0, j0), (f1, j1))
    geos = [_geo(F, J0) for (F, J0) in pair_geo]
    need_mt = any(g["have"] and not g["side_q"] for g in geos)
    np_sc = np_fp8 if (USE_FP8 and FP8_SCORES) else np_bf16
    has_kside = any(g["have"] and not g["side_q"] for g in geos)
    need_xqb = np_sc != np_x
    need_xo8 = (np_sc != np_x) and has_kside
    CQs = [geos[s // 2]["CQ"] for s in range(BC)]
    NOs = [geos[s // 2]["NO"] for s in range(BC)]
    OJs = [geos[s // 2]["OJ"] for s in range(BC)]
    SQ, SO = sum(CQs), sum(NOs)

    def w(name):
        return np.ascontiguousarray(np.asarray(inputs[name], dtype=np.float32))

    shared = {}
    for p, (qn, kn) in enumerate((("w_sq", "w_sk"), ("w_cq", "w_ck"),
                                  ("w_rq", "w_rk"))):
        shared[f"m{p}"] = _m_matrix(inputs[qn], inputs[kn])
        if need_mt:
            shared[f"mt{p}"] = _m_matrix(inputs[qn], inputs[kn], transposed=True)

    wanom_pm = w("w_anom").reshape(ND, 128).T            # [128, ND]
    emb = np.zeros((128, ND, BC, BC), np.float32)
    for s in range(BC):
        emb[:, :, s, s] = wanom_pm * PRE
    shared["wanom_emb"] = emb.astype(np_sc)

    shared["w_f1"] = np.ascontiguousarray(
        w("w_f1").reshape(NC3, 128, D)).astype(np_bf16)
    shared["w_f2"] = np.ascontiguousarray(
        w("w_f2").reshape(ND, 128, D)).astype(np_bf16)
    shared["b_f1"] = w("b_f1").reshape(1, D).astype(np_bf16)
    shared["b_f2"] = w("b_f2").reshape(1, D).astype(np_bf16)
    shared["ln_g"] = np.broadcast_to(w("ln_g").reshape(1, D),
                                     (BC, D)).copy()
    shared["ln_b"] = np.broadcast_to(w("ln_b").reshape(1, D),
                                     (BC, D)).copy()

    in_maps = []
    core_idx = []
    for c in range(NCORES):
        idx = order[np.arange(BC) * NCORES + c]
        core_idx.append(idx)
        xs = x[idx]                                      # [BC, L, D] f32
        m = dict(shared)
        m["x"] = xs.astype(np_bf16)
        xsT = np.ascontiguousarray(xs.transpose(2, 0, 1))   # [D, BC, L] f32
        xq_f = np.ascontiguousarray(np.concatenate(
            [xsT[:, s, 0 : CQs[s]] for s in range(BC)], axis=1,
        ))
        m["xq"] = xq_f.reshape(ND, 128, SQ).astype(np_x)
        if need_xqb:
            m["xqb"] = xq_f.reshape(ND, 128, SQ).astype(np_sc)
        if SO:
            xo_f = np.ascontiguousarray(np.concatenate(
                [xsT[:, s, OJs[s] : L] for s in range(BC)], axis=1,
            ))
            m["xo"] = xo_f.reshape(ND, 128, SO).astype(np_sc)
            if need_xo8:
                m["xo8"] = xo_f.reshape(ND, 128, SO).astype(np_x)
        m["fbias"] = np.ascontiguousarray(fb[idx])
        m["obias"] = np.ascontiguousarray(ob[idx])
        in_maps.append(m)
    return in_maps, pair_geo, core_idx


def get_program_fast(pair_geo):
    if pair_geo not in _PROGRAM_CACHE:
        _PROGRAM_CACHE[pair_geo] = build_program_fast(pair_geo)
    return _PROGRAM_CACHE[pair_geo]


def run(trace=False, **inputs):
    use_m = all(
        not np.any(np.asarray(inputs[n]))
        for n in ("b_sq", "b_sk", "b_cq", "b_ck", "b_rq", "b_rk")
    )
    if not use_m:
        return _run_legacy(trace=trace, **inputs)
    in_maps, pair_geo, core_idx = _host_prep_fast(inputs)
    nc = get_program_fast(pair_geo)
    res = bass_utils.run_bass_kernel_spmd(
        nc, in_maps, core_ids=list(range(NCORES)), trace=trace
    )
    out = np.empty((B, D), np.float32)
    for c in range(NCORES):
        out[core_idx[c]] = res.results[c]["out"]
    return out, res


def kernel(**inputs):
    out, _ = run(trace=False, **inputs)
    return out


# ---------------------------------------------------------------------------
# Legacy fallback (nonzero projection biases): original per-slot program.
# ---------------------------------------------------------------------------

PROJ_NAMES = ["w_sq", "w_sk", "w_cq", "w_ck", "w_rq", "w_rk"]
PBIAS_NAMES = ["b_sq", "b_sk", "b_cq", "b_ck", "b_rq", "b_rk"]
QS, KS, QC, KC, QR, KR = range(6)
QPROJ = (QS, QC, QR)
_LEGACY_CACHE = {}


def build_program_legacy(bounds=((2, 2),) * BC):
    nc = bacc.Bacc(
        "TRN2",
        target_bir_lowering=False,
        debug=False,
        enable_asserts=False,
        num_devices=NCORES,
    )

    xT_d = nc.dram_tensor("xT", [BC, D, L], bf16, kind="ExternalInput").ap()
    x_d = nc.dram_tensor("x", [BC, L, D], f32, kind="ExternalInput").ap()
    fmask_d = nc.dram_tensor("fmask", [BC, L], f32, kind="ExternalInput").ap()
    obias_d = nc.dram_tensor("obias", [BC, L], bf16, kind="ExternalInput").ap()

    W_d = {p: nc.dram_tensor(PROJ_NAMES[p], [D, D], bf16, kind="ExternalInput").ap()
           for p in range(6)}
    Brow_d = {
        p: nc.dram_tensor(PBIAS_NAMES[p], [1, D], bf16, kind="ExternalInput").ap()
        for p in range(6)}
    wanom_d = nc.dram_tensor("w_anom", [D, 1], bf16, kind="ExternalInput").ap()
    wf1_d = nc.dram_tensor("w_f1", [ND, 128, NC3 * 128], bf16, kind="ExternalInput").ap()
    wf2_d = nc.dram_tensor("w_f2", [ND, 128, ND * 128], bf16, kind="ExternalInput").ap()
    bf1_d = nc.dram_tensor("b_f1", [128, ND], f32, kind="ExternalInput").ap()
    bf2_d = nc.dram_tensor("b_f2", [128, ND], f32, kind="ExternalInput").ap()
    lng_d = nc.dram_tensor("ln_g", [128, ND], f32, kind="ExternalInput").ap()
    lnb_d = nc.dram_tensor("ln_b", [128, ND], f32, kind="ExternalInput").ap()

    out_d = nc.dram_tensor("out", [BC, D], f32, kind="ExternalOutput").ap()

    with tile.TileContext(nc) as tc, ExitStack() as ctx:
        const_p = ctx.enter_context(tc.tile_pool(name="const", bufs=1))
        tmp_p = ctx.enter_context(tc.tile_pool(name="tmp", bufs=2))
        sm_p = ctx.enter_context(tc.tile_pool(name="small", bufs=3))
        tail_p = ctx.enter_context(tc.tile_pool(name="tail", bufs=1))
        ps_big = ctx.enter_context(tc.tile_pool(name="psb", bufs=4, space="PSUM"))
        ps_s = ctx.enter_context(tc.tile_pool(name="pss", bufs=4, space="PSUM"))
        es2 = ExitStack()   # closed after phase C: x, E
        x_p = es2.enter_context(tc.tile_pool(name="x", bufs=3))
        e_p = es2.enter_context(tc.tile_pool(name="emat", bufs=2))
        es1 = ExitStack()   # closed after phase B: xT, W, proj
        xT_p = es1.enter_context(tc.tile_pool(name="xT", bufs=1))
        w_p = es1.enter_context(tc.tile_pool(name="w", bufs=2))
        proj_p = es1.enter_context(tc.tile_pool(name="proj", bufs=1))

        ones_row = const_p.tile([1, L], bf16)
        nc.vector.memset(ones_row[:], 1.0)
        ones_f = const_p.tile([1, 128], f32)
        nc.vector.memset(ones_f[:], 1.0)
        ones_col = const_p.tile([128, 1], f32)
        nc.vector.memset(ones_col[:], 1.0)
        iot_t = const_p.tile([128, 128], mybir.dt.int32)
        nc.gpsimd.iota(iot_t[:], pattern=[[1, 128]], base=0, channel_multiplier=-1)
        ident_t = const_p.tile([128, 128], f32)
        nc.vector.tensor_scalar(ident_t[:], iot_t[:], scalar1=0, scalar2=None,
                                op0=ALU.is_equal)

        wanom_t = const_p.tile([128, ND], bf16)
        nc.scalar.dma_start(wanom_t[:], wanom_d[:, 0].rearrange("(k p) -> p k", p=128))
        brow_t = {}
        for p in Brow_d:
            brow_t[p] = const_p.tile([1, D], bf16, name=f"brow{p}")
            nc.sync.dma_start(brow_t[p][:], Brow_d[p][:])
        bf1_t = const_p.tile([128, ND], f32)
        nc.scalar.dma_start(bf1_t[:], bf1_d[:])
        bf2_t = const_p.tile([128, ND], f32)
        nc.scalar.dma_start(bf2_t[:], bf2_d[:])
        lng_t = const_p.tile([128, ND], f32)
        nc.scalar.dma_start(lng_t[:], lng_d[:])
        lnb_t = const_p.tile([128, ND], f32)
        nc.scalar.dma_start(lnb_t[:], lnb_d[:])

        fusedT = tail_p.tile([128, NC3, BC], bf16)

        geo = []
        for s in range(BC):
            F, J0 = bounds[s]
            geo.append((F, J0, F * 128, J0 * 128, L - J0 * 128,
                        F > 0 and L - J0 * 128 > 0))

        xT_t = xT_p.tile([128, BC * ND, L], bf16)
        fm_ts, ob_ts, x_ts = [], [], []
        for s in range(BC):
            nc.sync.dma_start(
                xT_t[:, s * ND : (s + 1) * ND, :],
                xT_d[s].rearrange("(k p) i -> p k i", p=128),
            )
            fm_t = sm_p.tile([128, NL], f32, tag="fm", bufs=BC, name=f"fm{s}")
            nc.scalar.dma_start(fm_t[:], fmask_d[s].rearrange("(t p) -> p t", p=128))
            fm_ts.append(fm_t)
            ob_t = sm_p.tile([1, L], bf16, tag="ob", bufs=2, name=f"ob{s}")
            nc.scalar.dma_start(ob_t[:], obias_d[s : s + 1, :])
            ob_ts.append(ob_t)

        gate_ts = []
        for s in range(BC):
            F, J0, CQ, OJ, NO, have_attn = geo[s]
            gate_t = sm_p.tile([128, NL], f32, tag="gate", bufs=BC, name=f"gate{s}")
            gate_ts.append(gate_t)
            if F == 0:
                continue
            ghat_t = sm_p.tile([128, NL], f32, tag="ghat")
            for it in range(F):
                al_ps = ps_s.tile([128, 1], f32, tag="pss")
                for k in range(ND):
                    nc.tensor.matmul(
                        al_ps[:],
                        lhsT=xT_t[:, s * ND + k, it * 128 : (it + 1) * 128],
                        rhs=wanom_t[:, k : k + 1],
                        start=(k == 0), stop=(k == ND - 1),
                    )
                eg_t = sm_p.tile([128, 1], f32, tag="eg")
                nc.scalar.activation(eg_t[:], al_ps[:], AF.Exp)
                nc.vector.tensor_mul(
                    ghat_t[:, it : it + 1], eg_t[:], fm_ts[s][:, it : it + 1]
                )
            gsum_t = sm_p.tile([128, 1], f32, tag="gsum")
            nc.vector.tensor_reduce(
                gsum_t[:], ghat_t[:, 0:F], axis=mybir.AxisListType.X, op=ALU.add
            )
            S_ps = ps_s.tile([1, 1], f32, tag="pss")
            nc.tensor.matmul(S_ps[:], lhsT=gsum_t[:], rhs=ones_col[:],
                             start=True, stop=True)
            Smax_t = sm_p.tile([1, 1], f32, tag="Smax")
            nc.vector.tensor_scalar_max(Smax_t[:], S_ps[:], 1e-8)
            Sb_ps = ps_s.tile([128, 1], f32, tag="pss")
            nc.tensor.matmul(Sb_ps[:], lhsT=ones_f[:], rhs=Smax_t[:],
                             start=True, stop=True)
            recipS_t = sm_p.tile([128, 1], f32, tag="recipS")
            nc.vector.reciprocal(recipS_t[:], Sb_ps[:])
            nc.vector.tensor_scalar_mul(gate_t[:, 0:F], ghat_t[:, 0:F],
                                        recipS_t[:])

        projs = [[None] * BC for _ in range(6)]
        for p in range(6):
            qside = p in QPROJ
            widths = [
                ((g[2] if qside else g[4]) if g[5] else 0) for g in geo
            ]
            wmax = max(widths)
            if wmax == 0:
                continue
            wt = w_p.tile([128, ND, D], bf16, tag="w", name=f"w{p}")
            nc.gpsimd.dma_start(wt[:], W_d[p].rearrange("(k p) c -> p k c", p=128))
            pt = proj_p.tile([128, BC, ND, wmax], bf16, tag=f"proj{p}")
            for m in range(ND):
                for s in range(BC):
                    width = widths[s]
                    if width == 0:
                        continue
                    lo = 0 if qside else geo[s][3]
                    ps = ps_big.tile([128, width], f32, tag="ps")
                    for k in range(ND):
                        nc.tensor.matmul(
                            ps[:], lhsT=wt[:, k, m * 128 : (m + 1) * 128],
                            rhs=xT_t[:, s * ND + k, lo : lo + width],
                            start=(k == 0), stop=False,
                        )
                    nc.tensor.matmul(
                        ps[:], lhsT=brow_t[p][:, m * 128 : (m + 1) * 128],
                        rhs=ones_row[:, 0:width], start=False, stop=True,
                    )
                    nc.vector.tensor_copy(pt[:, s, m, :], ps[:])
            for s in range(BC):
                if widths[s]:
                    projs[p][s] = pt

        for s in range(BC):
            x_t = x_p.tile([128, NL, D], f32, tag="x", name=f"x{s}")
            nc.sync.dma_start(x_t[:], x_d[s].rearrange("(t p) d -> p t d", p=128))
            x_ts.append(x_t)

        E_sups, E_reps, co_sups, co_reps = {}, {}, {}, {}
        for s in range(BC):
            F, J0, CQ, OJ, NO, have_attn = geo[s]
            if not have_attn:
                continue
            E_sup = e_p.tile([128, max(F, 1), NO], f32, tag="esup", bufs=BC,
                             name=f"esup{s}")
            E_rep = e_p.tile([128, max(F, 1), NO], f32, tag="erep", bufs=BC,
                             name=f"erep{s}")
            co_sup = sm_p.tile([128, NL], f32, tag="cosup", bufs=BC,
                               name=f"cosup{s}")
            co_rep = sm_p.tile([128, NL], f32, tag="corep", bufs=BC,
                               name=f"corep{s}")
            E_sups[s], E_reps[s] = E_sup, E_rep
            co_sups[s], co_reps[s] = co_sup, co_rep
            gate_t = gate_ts[s]
            ob_t = ob_ts[s]
            for it in range(F):
                isl = slice(it * 128, (it + 1) * 128)
                ps_sup = ps_big.tile([128, NO], f32, tag="ps")
                for k in range(ND):
                    nc.tensor.matmul(
                        ps_sup[:], lhsT=projs[QS][s][:, s, k, isl],
                        rhs=projs[KS][s][:, s, k, 0:NO],
                        start=(k == 0), stop=False,
                    )
                nc.tensor.matmul(ps_sup[:], lhsT=ones_row[:, 0:128],
                                 rhs=ob_t[:, OJ:L], start=False, stop=True)
                ps_con = ps_big.tile([128, NO], f32, tag="ps")
                for k in range(ND):
                    nc.tensor.matmul(
                        ps_con[:], lhsT=projs[QC][s][:, s, k, isl],
                        rhs=projs[KC][s][:, s, k, 0:NO],
                        start=(k == 0), stop=(k == ND - 1),
                    )
                ps_rep = ps_big.tile([128, NO], f32, tag="ps")
                for k in range(ND):
                    nc.tensor.matmul(
                        ps_rep[:], lhsT=projs[QR][s][:, s, k, isl],
                        rhs=projs[KR][s][:, s, k, 0:NO],
                        start=(k == 0), stop=False,
                    )
                nc.tensor.matmul(ps_rep[:], lhsT=ones_row[:, 0:128],
                                 rhs=ob_t[:, OJ:L], start=False, stop=True)

                T_t = tmp_p.tile([128, NO], f32, tag="T")
                nc.scalar.activation(T_t[:], ps_con[:], AF.Tanh, scale=SCALE)
                A_t = tmp_p.tile([128, NO], f32, tag="A")
                nc.vector.scalar_tensor_tensor(
                    A_t[:], in0=ps_rep[:], scalar=SCALE, in1=T_t[:],
                    op0=ALU.mult, op1=ALU.add,
                )
                rs_sup = sm_p.tile([128, 1], f32, tag="rssup")
                nc.scalar.activation(E_sup[:, it, :], ps_sup[:], AF.Exp,
                                     scale=SCALE, accum_out=rs_sup[:])
                rs_rep = sm_p.tile([128, 1], f32, tag="rsrep")
                nc.scalar.activation(E_rep[:, it, :], A_t[:], AF.Exp,
                                     accum_out=rs_rep[:])
                rc_sup = sm_p.tile([128, 1], f32, tag="rcsup")
                nc.vector.reciprocal(rc_sup[:], rs_sup[:])
                nc.vector.tensor_mul(co_sup[:, it : it + 1],
                                     gate_t[:, it : it + 1], rc_sup[:])
                rc_rep = sm_p.tile([128, 1], f32, tag="rcrep")
                nc.vector.reciprocal(rc_rep[:], rs_rep[:])
                nc.vector.tensor_mul(co_rep[:, it : it + 1],
                                     gate_t[:, it : it + 1], rc_rep[:])

        es1.close()

        for s in range(BC):
            F, J0, CQ, OJ, NO, have_attn = geo[s]
            x_t = x_ts[s]

            G_t = sm_p.tile([128, NL, 3], f32, tag="G")
            nc.vector.memset(G_t[:], 0.0)
            if F > 0:
                for it in range(F):
                    nc.vector.tensor_copy(G_t[:, it, 0:1],
                                          gate_ts[s][:, it : it + 1])
            if have_attn:
                E_sup, E_rep = E_sups[s], E_reps[s]
                co_sup, co_rep = co_sups[s], co_reps[s]
                for jt in range(J0, NL):
                    jsl = slice(jt * 128 - OJ, jt * 128 - OJ + 128)
                    r_ps = ps_s.tile([128, 2], f32, tag="pss")
                    for it in range(F):
                        nc.tensor.matmul(
                            r_ps[:, 0:1], lhsT=E_rep[:, it, jsl],
                            rhs=co_rep[:, it : it + 1],
                            start=(it == 0), stop=(it == F - 1),
                        )
                    for it in range(F):
                        nc.tensor.matmul(
                            r_ps[:, 1:2], lhsT=E_sup[:, it, jsl],
                            rhs=co_sup[:, it : it + 1],
                            start=(it == 0), stop=(it == F - 1),
                        )
                    nc.vector.tensor_copy(G_t[:, jt, 1:3], r_ps[:, 0:2])

            rts = sorted(set(range(F)) | (set(range(J0, NL)) if have_attn else set()))
            if not rts:
                rts = [0]
            for m in range(ND):
                pool_ps = ps_s.tile([128, 3], f32, tag="pss")
                for i, rt in enumerate(rts):
                    nc.tensor.matmul(
                        pool_ps[:], lhsT=x_t[:, rt, m * 128 : (m + 1) * 128],
                        rhs=G_t[:, rt, :],
                        start=(i == 0), stop=(i == len(rts) - 1),
                    )
                for t in range(3):
                    nc.vector.tensor_copy(
                        fusedT[:, t * ND + m, s : s + 1], pool_ps[:, t : t + 1]
                    )

        es2.close()

        wf1_p = ctx.enter_context(tc.tile_pool(name="wf1", bufs=8))
        hT_t = tail_p.tile([128, ND, BC], bf16)
        for m in range(ND):
            wt = wf1_p.tile([128, NC3, 128], bf16, tag="wf1")
            nc.gpsimd.dma_start(wt[:], wf1_d[m].rearrange("p (k c) -> p k c", c=128))
            h_ps = ps_s.tile([128, BC], f32, tag="pss")
            for k in range(NC3):
                nc.tensor.matmul(h_ps[:], lhsT=wt[:, k, :], rhs=fusedT[:, k, :],
                                 start=(k == 0), stop=(k == NC3 - 1))
            nc.scalar.activation(hT_t[:, m, :], h_ps[:], AF.Relu,
                                 bias=bf1_t[:, m : m + 1])

        yT_t = tail_p.tile([128, ND, BC], f32)
        sq_t = tail_p.tile([128, ND, BC], f32)
        for m in range(ND):
            wt = wf1_p.tile([128, ND, 128], bf16, tag="wf2")
            nc.gpsimd.dma_start(wt[:], wf2_d[m].rearrange("p (k c) -> p k c", c=128))
            y_ps = ps_s.tile([128, BC], f32, tag="pss")
            for k in range(ND):
                nc.tensor.matmul(y_ps[:], lhsT=wt[:, k, :], rhs=hT_t[:, k, :],
                                 start=(k == 0), stop=(k == ND - 1))
            nc.vector.tensor_scalar_add(yT_t[:, m, :], y_ps[:], bf2_t[:, m : m + 1])
            nc.scalar.square(sq_t[:, m, :], yT_t[:, m, :])

        sum_ps = ps_s.tile([1, BC], f32, tag="pss")
        for m in range(ND):
            nc.tensor.matmul(sum_ps[:], lhsT=ones_col[:], rhs=yT_t[:, m, :],
                             start=(m == 0), stop=(m == ND - 1))
        ssq_ps = ps_s.tile([1, BC], f32, tag="pss")
        for m in range(ND):
            nc.tensor.matmul(ssq_ps[:], lhsT=ones_col[:], rhs=sq_t[:, m, :],
                             start=(m == 0), stop=(m == ND - 1))
        mean_t = sm_p.tile([1, BC], f32, tag="mean")
        nc.scalar.mul(mean_t[:], sum_ps[:], 1.0 / D)
        msq_t = sm_p.tile([1, BC], f32, tag="msq")
        nc.scalar.mul(msq_t[:], ssq_ps[:], 1.0 / D)
        m2_t = sm_p.tile([1, BC], f32, tag="m2")
        nc.vector.tensor_mul(m2_t[:], mean_t[:], mean_t[:])
        var_t = sm_p.tile([1, BC], f32, tag="var")
        nc.vector.tensor_sub(var_t[:], msq_t[:], m2_t[:])
        nc.vector.tensor_scalar_add(var_t[:], var_t[:], LN_EPS)
        sd_t = sm_p.tile([1, BC], f32, tag="sd")
        nc.scalar.sqrt(sd_t[:], var_t[:])
        rstd_t = sm_p.tile([1, BC], f32, tag="rstd")
        nc.vector.reciprocal(rstd_t[:], sd_t[:])

        mb_ps = ps_s.tile([128, BC], f32, tag="pss")
        nc.tensor.matmul(mb_ps[:], lhsT=ones_f[:], rhs=mean_t[:],
                         start=True, stop=True)
        mb_t = sm_p.tile([128, BC], f32, tag="mbt")
        nc.vector.tensor_copy(mb_t[:], mb_ps[:])
        rb_ps = ps_s.tile([128, BC], f32, tag="pss")
        nc.tensor.matmul(rb_ps[:], lhsT=ones_f[:], rhs=rstd_t[:],
                         start=True, stop=True)
        rb_t = sm_p.tile([128, BC], f32, tag="rbt")
        nc.vector.tensor_copy(rb_t[:], rb_ps[:])

        zrow_t = tail_p.tile([BC, D], f32)
        for m in range(ND):
            z_t = tmp_p.tile([128, BC], f32, tag="z")
            nc.vector.tensor_sub(z_t[:], yT_t[:, m, :], mb_t[:])
            nc.vector.tensor_mul(z_t[:], z_t[:], rb_t[:])
            z2_t = tmp_p.tile([128, BC], f32, tag="z2")
            nc.vector.tensor_scalar(
                z2_t[:], z_t[:], scalar1=lng_t[:, m : m + 1],
                scalar2=lnb_t[:, m : m + 1], op0=ALU.mult, op1=ALU.add,
            )
            tr_ps = ps_s.tile([BC, 128], f32, tag="pss")
            nc.tensor.transpose(tr_ps[:], z2_t[:], ident_t[:])
            nc.vector.tensor_copy(zrow_t[:, m * 128 : (m + 1) * 128], tr_ps[:])
        nc.sync.dma_start(out_d[:, :], zrow_t[:, :])

    nc.compile()
    return nc


def _run_legacy(trace=False, **inputs):
    x = np.asarray(inputs["x"], dtype=np.float32)
    x_ids = np.asarray(inputs["x_ids"])
    pad_idx = int(np.asarray(inputs["pad_idx"]))
    sep_idx = int(np.asarray(inputs["sep_idx"]))

    sp, fmask_b, omask = _masks(x_ids, pad_idx, sep_idx)
    fmask = fmask_b.astype(np.float32)
    obias = np.where(omask, 0.0, OBIAS_RAW).astype(np.float32)

    F_all = np.ceil(sp / 128).astype(int)
    J0_all = np.minimum((sp + 1) // 128, NL)
    bounds = tuple(
        (int(F_all.reshape(NCORES, BC)[:, s].max()),
         int(J0_all.reshape(NCORES, BC)[:, s].min()))
        for s in range(BC)
    )

    xT = np.ascontiguousarray(x.transpose(0, 2, 1))

    def w(name):
        return np.ascontiguousarray(np.asarray(inputs[name], dtype=np.float32))

    def ppart(name):
        return np.ascontiguousarray(np.asarray(inputs[name], dtype=np.float32)
                                    .reshape(ND, 128).T)

    shared = {}
    for p in range(6):
        shared[PROJ_NAMES[p]] = w(PROJ_NAMES[p]).astype(np_bf16)
        shared[PBIAS_NAMES[p]] = w(PBIAS_NAMES[p]).reshape(1, D).astype(np_bf16)
    shared["w_anom"] = w("w_anom").reshape(D, 1).astype(np_bf16)

    def mpack(name, nk):
        a = w(name)
        a = a.reshape(nk, 128, ND, 128).transpose(2, 1, 0, 3).reshape(ND, 128, nk * 128)
        return np.ascontiguousarray(a).astype(np_bf16)

    shared["w_f1"] = mpack("w_f1", NC3)
    shared["w_f2"] = mpack("w_f2", ND)
    shared["b_f1"] = ppart("b_f1")
    shared["b_f2"] = ppart("b_f2")
    shared["ln_g"] = ppart("ln_g")
    shared["ln_b"] = ppart("ln_b")

    in_maps = []
    for c in range(NCORES):
        sl = slice(c * BC, (c + 1) * BC)
        m = dict(shared)
        m["x"] = np.ascontiguousarray(x[sl])
        m["xT"] = np.ascontiguousarray(xT[sl]).astype(np_bf16)
        m["fmask"] = np.ascontiguousarray(fmask[sl])
        m["obias"] = np.ascontiguousarray(obias[sl]).astype(np_bf16)
        in_maps.append(m)

    if bounds not in _LEGACY_CACHE:
        _LEGACY_CACHE[bounds] = build_program_legacy(bounds)
    nc = _LEGACY_CACHE[bounds]
    res = bass_utils.run_bass_kernel_spmd(
        nc, in_maps, core_ids=list(range(NCORES)), trace=trace
    )
    out = np.concatenate([res.results[c]["out"] for c in range(NCORES)], axis=0)
    return out.astype(np.float32), res


# revision 19
# speedup vs baseline: 2.1076x; 1.0539x over previous
"""Trainium2 Bass kernel for nn_BertCounterFactTransformer.

Contract: kernel(**inputs) takes FULL unsharded numpy inputs (as produced by
reference.setup_inputs()) and returns the FULL [32, 1024] float32 output.

Strategy (data-parallel over batch, 8 cores x 4 samples):
  - Host: compute sep positions from x_ids, SORT samples by sep position and
    assign sorted rank r -> core (r % 8), slot (r // 8) so the per-slot-pair
    tile bounds are tight and uniform across cores. Precompute
    M_p = W_pq @ W_pk^T (bf16) so scores are x M x^T (no k-side projection).
  - Device, per pair of slots (F tiles of false rows, option cols from OJ):
      gate       all-4-sample anomaly logits in ONE [4, wg] PSUM via
                 block-diagonal embedded w_anom; false-mask folded as a
                 -30 bias row; exp+normalize row-wise; PE-transpose to cols
      proj       qT = (x M_p)^T (or M_p x_opt^T if the option side is
                 smaller), 2 samples batched per matmul (width<=512)
      scores     S = q @ x_opt^T blocks; option mask via -960 bias rows
      E_sup = exp(S_sup/32 + ob), E_rep = exp(S_rep/32 + tanh(S_con/32) + ob)
      coeff_t = gate / rowsum(E_t);  r_t = E_t^T @ coeff_t   (width-1 chains)
  - Pool: ONE [12, 512] PSUM accumulates x_s^T @ [gate|r_rep|r_sup] for all
    4 samples via 12-col G with per-sample zero blocks; PE-transpose the
    [12, 1024] result into fused^T columns.
  - Tail in row form: h = relu(fused @ W1 + b1), y = h @ W2 + b2, LayerNorm
    along the free dim, direct [4, 1024] output DMA.

Key identity: gate @ (attn @ x) == (gate @ attn) @ x, so [L,D] attention
outputs are never materialized.
"""

import sys

if "/opt/trn_rl_repo" not in sys.path:
    sys.path.insert(0, "/opt/trn_rl_repo")

import numpy as np
import ml_dtypes
from contextlib import ExitStack

np_bf16 = ml_dtypes.bfloat16
np_fp8 = ml_dtypes.float8_e4m3

import concourse.bacc as bacc
import concourse.bass as bass
import concourse.mybir as mybir
import concourse.tile as tile
from concourse import bass_utils

f32 = mybir.dt.float32
bf16 = mybir.dt.bfloat16
fp8 = mybir.dt.float8e4
AF = mybir.ActivationFunctionType
ALU = mybir.AluOpType

B, L, D = 32, 512, 1024
NCORES = 8
BC = B // NCORES          # samples per core
NL = L // 128             # 4 L-tiles
ND = D // 128             # 8 D-tiles
NC3 = 3 * D // 128        # 24 tiles of the 3D fused dim
SCALE = 1.0 / 32.0        # 1/sqrt(D)
OBIAS_RAW = -960.0        # -30 after * SCALE
FBIAS = -30.0
LN_EPS = 1e-5
USE_FP8 = True            # fp8 e4m3 + DoubleRow for the projection GEMMs
FP8_SCORES = True         # score operands (projT, xo) also fp8 + DoubleRow
PRE = 64.0 if USE_FP8 else 1.0   # pre-scale on M/w_anom (fp8 normal range)

_PROGRAM_CACHE = {}
_M_CACHE = {}


def _m_matrix(wq, wk, transposed=False):
    import hashlib
    wq = np.asarray(wq, dtype=np.float32)
    wk = np.asarray(wk, dtype=np.float32)
    key = (hashlib.blake2b(wq.tobytes() + wk.tobytes(), digest_size=16).digest(),
           transposed, USE_FP8)
    if key not in _M_CACHE:
        m = wq @ wk.T
        if transposed:
            m = m.T
        m = np.ascontiguousarray(m)
        if USE_FP8:
            _M_CACHE[key] = np.clip(m * PRE, -240, 240).astype(np_fp8)
        else:
            _M_CACHE[key] = m.astype(np_bf16)
    return _M_CACHE[key]


def _geo(F, J0):
    OJ = J0 * 128
    NO = L - OJ
    CQ = F * 128
    have = NO > 0
    side_q = (CQ <= NO) if have else True
    w = (CQ if side_q else NO) if have else 0
    lo = 0 if side_q else OJ
    return dict(F=F, J0=J0, OJ=OJ, NO=NO, CQ=CQ, have=have,
                side_q=side_q, w=w, lo=lo)


def build_program_fast(pair_geo):
    """pair_geo = ((F0, J0_0), (F1, J0_1)); pair p covers slots {2p, 2p+1}.
    Computing a superset is always correct (bias masks zero it)."""
    nc = bacc.Bacc(
        "TRN2",
        target_bir_lowering=False,
        debug=False,
        enable_asserts=False,
        num_devices=NCORES,
    )

    geos = [_geo(F, J0) for (F, J0) in pair_geo]
    need_mt = any(g["have"] and not g["side_q"] for g in geos)
    wg = max(max(g["CQ"] for g in geos), 128)       # gate width (cols 0..wg)
    maxF = max(g["F"] for g in geos)
    DT_X = fp8 if USE_FP8 else bf16
    DT_SC = fp8 if (USE_FP8 and FP8_SCORES) else bf16
    need_xqb = USE_FP8             # gate chains always run bf16
    CQs = [geos[s // 2]["CQ"] for s in range(BC)]
    NOs = [geos[s // 2]["NO"] for s in range(BC)]
    offq = [sum(CQs[:s]) for s in range(BC)]
    offo = [sum(NOs[:s]) for s in range(BC)]
    SQ, SO = sum(CQs), sum(NOs)

    # per-partition SBUF estimate (bytes) with w1-half0 resident early
    _xb = 1 if USE_FP8 else 2
    _scb = 1 if (USE_FP8 and FP8_SCORES) else 2
    _est = (ND * SQ * _xb + ND * SO * _scb + BC * NL * D * 2     # xq, xo, x
            + (ND * SQ * 2 if USE_FP8 else 0)                    # xqb
            + 3 * ND * D * _xb * (2 if need_mt else 1)           # M (+MT)
            + max(ND * 2 * g["w"] * _scb for g in geos) * 3      # projT
            + max(4 * g["F"] * g["NO"] * 2 for g in geos)        # E (bf16)
            + NC3 * 512 * 2                                      # w1 half 0
            + 60 * 1024)                                         # misc + slack
    early_tail = _est <= 200 * 1024

    xq_d = nc.dram_tensor("xq", [ND, 128, SQ], DT_X, kind="ExternalInput").ap()
    xqb_d = (nc.dram_tensor("xqb", [ND, 128, SQ], bf16,
                            kind="ExternalInput").ap() if need_xqb else None)
    need_xo8 = (DT_SC != DT_X) and any(
        g["have"] and not g["side_q"] for g in geos)
    xo_d = (nc.dram_tensor("xo", [ND, 128, SO], DT_SC, kind="ExternalInput").ap()
            if SO else None)
    xo8_d = (nc.dram_tensor("xo8", [ND, 128, SO], DT_X,
                            kind="ExternalInput").ap()
             if (SO and need_xo8) else None)
    x_d = nc.dram_tensor("x", [BC, L, D], bf16, kind="ExternalInput").ap()
    fb_d = nc.dram_tensor("fbias", [BC, L], bf16, kind="ExternalInput").ap()
    ob_d = nc.dram_tensor("obias", [BC, L], bf16, kind="ExternalInput").ap()
    m_d = [nc.dram_tensor(f"m{p}", [D, D], DT_X, kind="ExternalInput").ap()
           for p in range(3)]
    mt_d = ([nc.dram_tensor(f"mt{p}", [D, D], DT_X, kind="ExternalInput").ap()
             for p in range(3)] if need_mt else None)
    emb_d = nc.dram_tensor("wanom_emb", [128, ND, BC, BC], bf16,
                           kind="ExternalInput").ap()
    w1_d = nc.dram_tensor("w_f1", [NC3, 128, D], bf16, kind="ExternalInput").ap()
    w2_d = nc.dram_tensor("w_f2", [ND, 128, D], bf16, kind="ExternalInput").ap()
    b1_d = nc.dram_tensor("b_f1", [1, D], bf16, kind="ExternalInput").ap()
    b2_d = nc.dram_tensor("b_f2", [1, D], bf16, kind="ExternalInput").ap()
    lng_d = nc.dram_tensor("ln_g", [BC, D], f32, kind="ExternalInput").ap()
    lnb_d = nc.dram_tensor("ln_b", [BC, D], f32, kind="ExternalInput").ap()
    out_d = nc.dram_tensor("out", [BC, D], f32, kind="ExternalOutput").ap()

    with tile.TileContext(nc) as tc, ExitStack() as ctx:
        const_p = ctx.enter_context(tc.tile_pool(name="const", bufs=1))
        main_p = ctx.enter_context(tc.tile_pool(name="main", bufs=1))
        sm_p = ctx.enter_context(tc.tile_pool(name="small", bufs=2))
        tmp_p = ctx.enter_context(tc.tile_pool(name="tmp", bufs=2))
        ps_big = ctx.enter_context(tc.tile_pool(name="psb", bufs=4, space="PSUM"))
        ps_med = ctx.enter_context(tc.tile_pool(name="psm", bufs=2, space="PSUM"))
        ps_sm = ctx.enter_context(tc.tile_pool(name="pss", bufs=2, space="PSUM"))
        tailA_p = (ctx.enter_context(tc.tile_pool(name="tailA", bufs=1))
                   if early_tail else None)
        es2 = ExitStack()   # E matrices; closed after last r
        e_p = es2.enter_context(tc.tile_pool(name="emat", bufs=1))
        es1 = ExitStack()   # W matrices + projT; closed after last scores
        w_p = es1.enter_context(tc.tile_pool(name="w", bufs=1))
        proj_p = es1.enter_context(tc.tile_pool(name="proj", bufs=1))

        # ---- constants ----
        ones_row = const_p.tile([1, 128], bf16)
        nc.vector.memset(ones_row[:], 1.0)
        ones4 = const_p.tile([1, BC], bf16)
        nc.vector.memset(ones4[:], 1.0)
        eyerows = const_p.tile([1, BC, BC], bf16)
        nc.vector.memset(eyerows[:], 0.0)
        for s in range(BC):
            nc.vector.memset(eyerows[:, s, s : s + 1], 1.0)
        iot_t = const_p.tile([128, 128], mybir.dt.int32)
        nc.gpsimd.iota(iot_t[:], pattern=[[1, 128]], base=0, channel_multiplier=-1)
        ident_f = const_p.tile([128, 128], f32)
        nc.vector.tensor_scalar(ident_f[:], iot_t[:], scalar1=0, scalar2=None,
                                op0=ALU.is_equal)
        ident_b = const_p.tile([128, 128], bf16)
        nc.vector.tensor_copy(ident_b[:], ident_f[:])
        warm_t = const_p.tile([1, 1], f32)
        nc.scalar.sqrt(warm_t[:], ones4[0:1, 0:1])

        emb_t = const_p.tile([128, ND, BC, BC], bf16)
        nc.scalar.dma_start(emb_t[:], emb_d[:])
        fb_t = const_p.tile([1, BC, L], bf16)
        ob_t = const_p.tile([1, BC, L], bf16)
        for s in range(BC):
            nc.scalar.dma_start(fb_t[:, s, :], fb_d[s : s + 1, :])
            nc.scalar.dma_start(ob_t[:, s, :], ob_d[s : s + 1, :])
        b1_t = const_p.tile([1, D], bf16)
        nc.scalar.dma_start(b1_t[:], b1_d[:])
        b2_t = const_p.tile([1, D], bf16)
        nc.scalar.dma_start(b2_t[:], b2_d[:])
        lng_t = const_p.tile([BC, D], f32)
        nc.scalar.dma_start(lng_t[:], lng_d[:])
        lnb_t = const_p.tile([BC, D], f32)
        nc.scalar.dma_start(lnb_t[:], lnb_d[:])

        # ---- big inputs ----
        xq_t = main_p.tile([128, ND, SQ], DT_X)
        for k in range(ND):
            nc.sync.dma_start(xq_t[:, k, :], xq_d[k])
        xqb_t = xq_t
        if need_xqb:
            xqb_t = w_p.tile([128, ND, SQ], bf16, name="xqb")
            for k in range(ND):
                nc.sync.dma_start(xqb_t[:, k, :], xqb_d[k])
        xo_t = None
        if SO:
            xo_t = main_p.tile([128, ND, SO], DT_SC)
            for k in range(ND):
                nc.sync.dma_start(xo_t[:, k, :], xo_d[k])
        xo8_t = xo_t
        if SO and need_xo8:
            xo8_t = main_p.tile([128, ND, SO], DT_X)
            for k in range(ND):
                nc.sync.dma_start(xo8_t[:, k, :], xo8_d[k])
        w_ts = []
        for p in range(3):
            wt = w_p.tile([128, ND, D], DT_X, name=f"w{p}")
            nc.gpsimd.dma_start(wt[:], m_d[p].rearrange("(k p) c -> p k c", p=128))
            w_ts.append(wt)
        wt_ts = []
        if need_mt:
            for p in range(3):
                wtt = w_p.tile([128, ND, D], DT_X, name=f"wt{p}")
                nc.gpsimd.dma_start(
                    wtt[:], mt_d[p].rearrange("(k p) c -> p k c", p=128)
                )
                wt_ts.append(wtt)
        x_t = main_p.tile([128, BC, NL, D], bf16)
        for s in range(BC):
            nc.sync.dma_start(
                x_t[:, s], x_d[s].rearrange("(t p) d -> p t d", p=128)
            )

        def _load_w1_half(pool, half, name):
            w1h = pool.tile([128, NC3, 512], bf16, name=name)
            hs = slice(half * 512, (half + 1) * 512)
            for t in range(3):
                nc.gpsimd.dma_start(
                    w1h[:, t * ND : (t + 1) * ND, :],
                    w1_d[t * ND : (t + 1) * ND, :, hs].rearrange(
                        "k p n -> p k n"),
                )
            return w1h

        w1h_t = [None, None]
        if early_tail:
            w1h_t[0] = _load_w1_half(tailA_p, 0, "w1a")

        G_all = main_p.tile([128, NL, BC, 12], bf16)   # col = t*4 + s
        nc.vector.memset(G_all[:], 0.0)
        gcol = main_p.tile([128, maxF, BC], f32)

        # ---- gates (all samples): logits in ONE [4, wg] PSUM ----
        ga_ps = ps_med.tile([BC, wg], f32, tag="pm")
        for s in range(BC):
            for k in range(ND):
                nc.tensor.matmul(
                    ga_ps[:, 0 : CQs[s]], lhsT=emb_t[:, k, s, :],
                    rhs=xqb_t[:, k, offq[s] : offq[s] + CQs[s]],
                    start=(s == 0 and k == 0), stop=False,
                )
        for s in range(BC):
            nc.tensor.matmul(
                ga_ps[:], lhsT=eyerows[:, s, :], rhs=fb_t[:, s, 0:wg],
                start=False, stop=(s == BC - 1),
            )
        grow = sm_p.tile([BC, wg], f32, tag="grow", bufs=1)
        gs_t = sm_p.tile([BC, 1], f32, tag="gs", bufs=1)
        nc.scalar.activation(grow[:], ga_ps[:], AF.Exp, scale=1.0 / PRE,
                             accum_out=gs_t[:])
        nc.vector.tensor_scalar_max(gs_t[:], gs_t[:], 1e-8)
        rg_t = sm_p.tile([BC, 1], f32, tag="rg", bufs=1)
        nc.vector.reciprocal(rg_t[:], gs_t[:])
        nc.vector.tensor_scalar_mul(grow[:], grow[:], rg_t[:])
        for it in range(maxF):
            gt_ps = ps_med.tile([128, BC], f32, tag="pm")
            nc.tensor.transpose(gt_ps[:], grow[:, it * 128 : (it + 1) * 128],
                                ident_f[0:BC, 0:BC])
            nc.vector.tensor_copy(gcol[:, it, :], gt_ps[:])
            for s in range(BC):
                if it < geos[s // 2]["F"]:
                    nc.vector.tensor_copy(G_all[:, it, s, s : s + 1],
                                          gt_ps[:, s : s + 1])

        # ---- per pair: projections -> scores -> E -> coeffs -> r ----
        for pr in range(2):
            g = geos[pr]
            F, OJ, NO, w, lo = g["F"], g["OJ"], g["NO"], g["w"], g["lo"]
            if not g["have"]:
                continue
            s0, s1 = 2 * pr, 2 * pr + 1
            wmats = w_ts if g["side_q"] else wt_ts

            xsrc = xq_t if g["side_q"] else xo8_t
            poff = offq if g["side_q"] else offo
            assert poff[s1] == poff[s0] + w

            projT = [
                proj_p.tile([128, ND, 2, w], DT_SC, tag=f"pj{p}",
                            name=f"pj{p}_{pr}")
                for p in range(3)
            ]
            DR = mybir.MatmulPerfMode.DoubleRow if USE_FP8 else None
            NK = ND // 2 if USE_FP8 else ND
            for p in range(3):
                for m in range(ND):
                    msl = slice(m * 128, (m + 1) * 128)
                    if 2 * w <= 512:
                        pj_ps = ps_big.tile([128, 2, w], f32, tag="ps")
                        for k in range(NK):
                            if USE_FP8:
                                nc.tensor.matmul(
                                    pj_ps[:],
                                    lhsT=wmats[p][:, 2 * k : 2 * k + 2, msl],
                                    rhs=xsrc[:, 2 * k : 2 * k + 2,
                                             poff[s0] : poff[s0] + 2 * w],
                                    start=(k == 0), stop=(k == NK - 1),
                                    perf_mode=DR,
                                )
                            else:
                                nc.tensor.matmul(
                                    pj_ps[:],
                                    lhsT=wmats[p][:, k, msl],
                                    rhs=xsrc[:, k, poff[s0] : poff[s0] + 2 * w],
                                    start=(k == 0), stop=(k == NK - 1),
                                )
                        if m % 2 == 0:
                            nc.vector.tensor_copy(projT[p][:, m], pj_ps[:])
                        else:
                            nc.scalar.activation(projT[p][:, m], pj_ps[:],
                                                 AF.Copy)
                    else:
                        for sp in range(2):
                            pj_ps = ps_big.tile([128, w], f32, tag="ps")
                            for k in range(NK):
                                if USE_FP8:
                                    nc.tensor.matmul(
                                        pj_ps[:],
                                        lhsT=wmats[p][:, 2 * k : 2 * k + 2, msl],
                                        rhs=xsrc[:, 2 * k : 2 * k + 2,
                                                 poff[s0 + sp] :
                                                 poff[s0 + sp] + w],
                                        start=(k == 0), stop=(k == NK - 1),
                                        perf_mode=DR,
                                    )
                                else:
                                    nc.tensor.matmul(
                                        pj_ps[:],
                                        lhsT=wmats[p][:, k, msl],
                                        rhs=xsrc[:, k,
                                                 poff[s0 + sp] :
                                                 poff[s0 + sp] + w],
                                        start=(k == 0), stop=(k == NK - 1),
                                    )
                            if sp == 0:
                                nc.vector.tensor_copy(projT[p][:, m, sp],
                                                      pj_ps[:])
                            else:
                                nc.scalar.activation(projT[p][:, m, sp],
                                                     pj_ps[:], AF.Copy)

            for sp in range(2):
                s4 = s0 + sp
                E_sup = e_p.tile([128, F, NO], bf16, tag=f"Es{sp}",
                                 name=f"Es{sp}_{pr}")
                E_rep = e_p.tile([128, F, NO], bf16, tag=f"Er{sp}",
                                 name=f"Er{sp}_{pr}")
                co_sup = sm_p.tile([128, F], bf16, tag=f"cos{sp}", bufs=1,
                                   name=f"cos{sp}_{pr}")
                co_rep = sm_p.tile([128, F], bf16, tag=f"cor{sp}", bufs=1,
                                   name=f"cor{sp}_{pr}")
                sc_dr = USE_FP8 and FP8_SCORES
                NKS = ND // 2 if sc_dr else ND
                for it in range(F):
                    isl = slice(it * 128, (it + 1) * 128)
                    ps3 = {}
                    for p in (0, 2, 1):   # sup, rep, con
                        ps = ps_big.tile([128, NO], f32, tag="ps",
                                         name=f"sc{p}")
                        ps3[p] = ps
                        for k in range(NKS):
                            if sc_dr:
                                ksl = slice(2 * k, 2 * k + 2)
                                if g["side_q"]:
                                    lhsT = projT[p][:, ksl, sp, isl]
                                    rhs = xo_t[:, ksl,
                                               offo[s4] : offo[s4] + NO]
                                else:
                                    lhsT = xq_t[:, ksl,
                                                offq[s4] + it * 128 :
                                                offq[s4] + (it + 1) * 128]
                                    rhs = projT[p][:, ksl, sp, 0:NO]
                                nc.tensor.matmul(
                                    ps[:], lhsT=lhsT, rhs=rhs,
                                    start=(k == 0),
                                    stop=(k == NKS - 1 and p == 1),
                                    perf_mode=mybir.MatmulPerfMode.DoubleRow,
                                )
                            else:
                                if g["side_q"]:
                                    lhsT = projT[p][:, k, sp, isl]
                                    rhs = xo_t[:, k, offo[s4] : offo[s4] + NO]
                                else:
                                    lhsT = xqb_t[:, k,
                                                 offq[s4] + it * 128 :
                                                 offq[s4] + (it + 1) * 128]
                                    rhs = projT[p][:, k, sp, 0:NO]
                                nc.tensor.matmul(ps[:], lhsT=lhsT, rhs=rhs,
                                                 start=(k == 0),
                                                 stop=(k == NKS - 1 and p == 1))
                        if p != 1:
                            # option-mask bias row closes the sup/rep groups
                            nc.tensor.matmul(ps[:], lhsT=ones_row[:],
                                             rhs=ob_t[:, s4, OJ:L],
                                             start=False, stop=True)
                    ps_sup, ps_rep, ps_con = ps3[0], ps3[2], ps3[1]

                    T_t = tmp_p.tile([128, NO], f32, tag="T")
                    nc.scalar.activation(T_t[:], ps_con[:], AF.Tanh,
                                         scale=SCALE / PRE)
                    A_t = tmp_p.tile([128, NO], f32, tag="A")
                    nc.vector.scalar_tensor_tensor(
                        A_t[:], in0=ps_rep[:], scalar=SCALE / PRE, in1=T_t[:],
                        op0=ALU.mult, op1=ALU.add,
                    )
                    rs_sup = sm_p.tile([128, 1], f32, tag="rss")
                    nc.scalar.activation(E_sup[:, it], ps_sup[:], AF.Exp,
                                         scale=SCALE / PRE, accum_out=rs_sup[:])
                    rs_rep = sm_p.tile([128, 1], f32, tag="rsr")
                    nc.scalar.activation(E_rep[:, it], A_t[:], AF.Exp,
                                         accum_out=rs_rep[:])
                    rc_sup = sm_p.tile([128, 1], f32, tag="rcs")
                    nc.vector.reciprocal(rc_sup[:], rs_sup[:])
                    nc.vector.tensor_mul(co_sup[:, it : it + 1],
                                         gcol[:, it, s4 : s4 + 1], rc_sup[:])
                    rc_rep = sm_p.tile([128, 1], f32, tag="rcr")
                    nc.vector.reciprocal(rc_rep[:], rs_rep[:])
                    nc.vector.tensor_mul(co_rep[:, it : it + 1],
                                         gcol[:, it, s4 : s4 + 1], rc_rep[:])

                # r vectors: G col 4+s (rep), 8+s (sup)
                for t, (E_t, co_t) in enumerate(((E_rep, co_rep),
                                                 (E_sup, co_sup))):
                    for jt in range(NO // 128):
                        jsl = slice(jt * 128, (jt + 1) * 128)
                        r_ps = ps_sm.tile([128, 1], f32, tag="r")
                        for it in range(F):
                            nc.tensor.matmul(
                                r_ps[:], lhsT=E_t[:, it, jsl],
                                rhs=co_t[:, it : it + 1],
                                start=(it == 0), stop=(it == F - 1),
                            )
                        nc.vector.tensor_copy(
                            G_all[:, g["J0"] + jt, s4,
                                  4 * (t + 1) + s4 : 4 * (t + 1) + s4 + 1],
                            r_ps[:],
                        )

        es1.close()
        es2.close()

        # ---- remaining tail weights (after proj/W pools freed) ----
        tail_p = ctx.enter_context(tc.tile_pool(name="tail", bufs=1))
        if not early_tail:
            w1h_t[0] = _load_w1_half(tail_p, 0, "w1a2")
        w1h_t[1] = _load_w1_half(tail_p, 1, "w1b")
        w2_t = tail_p.tile([128, ND, D], bf16, name="w2sb")
        nc.gpsimd.dma_start(w2_t[:], w2_d.rearrange("k p n -> p k n"))

        # ---- pool: pooled[t*4+s, :] = sum_l G[l, t*4+s] * x_s[l, :] ----
        seq = []
        for s4 in range(BC):
            g = geos[s4 // 2]
            rts = sorted(set(range(g["F"]))
                         | (set(range(g["J0"], NL)) if g["have"] else set()))
            seq.extend((s4, rt) for rt in rts)
        pooled_sb = main_p.tile([12, D], bf16)
        for half in range(2):
            hs = slice(half * 512, (half + 1) * 512)
            po_ps = ps_med.tile([12, 512], f32, tag="pm")
            for i, (s4, rt) in enumerate(seq):
                nc.tensor.matmul(
                    po_ps[:], lhsT=G_all[:, rt, s4, :], rhs=x_t[:, s4, rt, hs],
                    start=(i == 0), stop=(i == len(seq) - 1),
                )
            nc.vector.tensor_copy(pooled_sb[:, hs], po_ps[:])

        fused_sb = main_p.tile([128, ND, 3, BC], bf16)
        for m in range(ND):
            tr_ps = ps_sm.tile([128, 12], bf16, tag="r")
            nc.tensor.transpose(tr_ps[:], pooled_sb[:, m * 128 : (m + 1) * 128],
                                ident_b[0:12, 0:12])
            nc.vector.tensor_copy(fused_sb[:, m], tr_ps[:])

        # ---- MLP tail + LayerNorm, all in [4, 1024] row form ----
        h_sb = main_p.tile([BC, D], bf16)
        for half in range(2):
            hs = slice(half * 512, (half + 1) * 512)
            h_ps = ps_big.tile([BC, 512], f32, tag="ps")
            for t in range(3):
                for m in range(ND):
                    nc.tensor.matmul(
                        h_ps[:], lhsT=fused_sb[:, m, t, :],
                        rhs=w1h_t[half][:, t * ND + m, :],
                        start=(t == 0 and m == 0), stop=False,
                    )
            nc.tensor.matmul(h_ps[:], lhsT=ones4[:], rhs=b1_t[:, hs],
                             start=False, stop=True)
            nc.scalar.activation(h_sb[:, hs], h_ps[:], AF.Relu)

        hT_sb = main_p.tile([128, ND, BC], bf16)
        for m in range(ND):
            ht_ps = ps_sm.tile([128, BC], bf16, tag="r")
            nc.tensor.transpose(ht_ps[:], h_sb[:, m * 128 : (m + 1) * 128],
                                ident_b[0:BC, 0:BC])
            nc.vector.tensor_copy(hT_sb[:, m], ht_ps[:])

        y_sb = main_p.tile([BC, D], f32)
        s1_t = sm_p.tile([BC, 2], f32, tag="s1", bufs=1)
        s2_t = sm_p.tile([BC, 2], f32, tag="s2", bufs=1)
        sq_sb = tmp_p.tile([BC, 512], f32, tag="sq", bufs=2)
        for half in range(2):
            hs = slice(half * 512, (half + 1) * 512)
            y_ps = ps_big.tile([BC, 512], f32, tag="ps")
            for k in range(ND):
                nc.tensor.matmul(y_ps[:], lhsT=hT_sb[:, k, :],
                                 rhs=w2_t[:, k, hs],
                                 start=(k == 0), stop=False)
            nc.tensor.matmul(y_ps[:], lhsT=ones4[:], rhs=b2_t[:, hs],
                             start=False, stop=True)
            nc.scalar.activation(y_sb[:, hs], y_ps[:], AF.Copy,
                                 accum_out=s1_t[:, half : half + 1])
            sq = tmp_p.tile([BC, 512], f32, tag="sq", bufs=2)
            nc.scalar.activation(sq[:], y_ps[:], AF.Square,
                                 accum_out=s2_t[:, half : half + 1])

        mu_t = sm_p.tile([BC, 1], f32, tag="mu", bufs=1)
        nc.vector.tensor_reduce(mu_t[:], s1_t[:], axis=mybir.AxisListType.X,
                                op=ALU.add)
        nc.scalar.mul(mu_t[:], mu_t[:], 1.0 / D)
        msq_t = sm_p.tile([BC, 1], f32, tag="msq", bufs=1)
        nc.vector.tensor_reduce(msq_t[:], s2_t[:], axis=mybir.AxisListType.X,
                                op=ALU.add)
        nc.scalar.mul(msq_t[:], msq_t[:], 1.0 / D)
        m2_t = sm_p.tile([BC, 1], f32, tag="m2", bufs=1)
        nc.vector.tensor_mul(m2_t[:], mu_t[:], mu_t[:])
        var_t = sm_p.tile([BC, 1], f32, tag="var", bufs=1)
        nc.vector.tensor_sub(var_t[:], msq_t[:], m2_t[:])
        nc.vector.tensor_scalar_add(var_t[:], var_t[:], LN_EPS)
        sd_t = sm_p.tile([BC, 1], f32, tag="sd", bufs=1)
        nc.scalar.sqrt(sd_t[:], var_t[:])
        rstd_t = sm_p.tile([BC, 1], f32, tag="rstd", bufs=1)
        nc.vector.reciprocal(rstd_t[:], sd_t[:])

        z_sb = main_p.tile([BC, D], f32)
        nc.vector.tensor_scalar(z_sb[:], y_sb[:], scalar1=mu_t[:],
                                scalar2=rstd_t[:], op0=ALU.subtract,
                                op1=ALU.mult)
        nc.vector.tensor_mul(z_sb[:], z_sb[:], lng_t[:])
        nc.vector.tensor_add(z_sb[:], z_sb[:], lnb_t[:])
        nc.sync.dma_start(out_d[:], z_sb[:])

    nc.compile()
    return nc


def _masks(x_ids, pad_idx, sep_idx):
    valid = x_ids != pad_idx
    sepm = x_ids == sep_idx
    has = sepm.any(axis=1)
    first = sepm.argmax(axis=1)
    vlen = valid.sum(axis=1)
    fb = np.clip(vlen // 2, 1, max(1, L - 2))
    sp = np.where(has, first, fb)
    pos = np.arange(L)
    fmask = (pos[None, :] < sp[:, None]) & valid
    omask = (pos[None, :] > sp[:, None]) & valid
    return sp, fmask, omask


def _host_prep_fast(inputs):
    import os

    x = np.asarray(inputs["x"], dtype=np.float32)
    x_ids = np.asarray(inputs["x_ids"])
    pad_idx = int(np.asarray(inputs["pad_idx"]))
    sep_idx = int(np.asarray(inputs["sep_idx"]))
    assert x.shape == (B, L, D), x.shape
    np_x = np_fp8 if USE_FP8 else np_bf16

    sp, fmask, omask = _masks(x_ids, pad_idx, sep_idx)
    fb = np.where(fmask, 0.0, FBIAS * PRE).astype(np_bf16)
    ob = np.where(omask, 0.0, OBIAS_RAW * PRE).astype(np_bf16)

    order = np.argsort(-sp, kind="stable")
    F_all = np.ceil(sp / 128).astype(int)
    J0_all = np.minimum((sp + 1) // 128, NL)
    pair_geo = tuple(
        (int(F_all[order[pr * 16 : (pr + 1) * 16]].max()),
         int(J0_all[order[pr * 16 : (pr + 1) * 16]].min()))
        for pr in range(2)
    )
    fbnd = os.environ.get("FORCE_BOUNDS")
    if fbnd:
        f0, j0, f1, j1 = (int(v) for v in fbnd.split(","))
        pair_geo = ((f0, j0), (f1, j1))
    geos = [_geo(F, J0) for (F, J0) in pair_geo]
    need_mt = any(g["have"] and not g["side_q"] for g in geos)
    np_sc = np_fp8 if (USE_FP8 and FP8_SCORES) else np_bf16
    has_kside = any(g["have"] and not g["side_q"] for g in geos)
    need_xqb = USE_FP8
    need_xo8 = (np_sc != np_x) and has_kside
    CQs = [geos[s // 2]["CQ"] for s in range(BC)]
    NOs = [geos[s // 2]["NO"] for s in range(BC)]
    OJs = [geos[s // 2]["OJ"] for s in range(BC)]
    SQ, SO = sum(CQs), sum(NOs)

    def w(name):
        return np.ascontiguousarray(np.asarray(inputs[name], dtype=np.float32))

    shared = {}
    for p, (qn, kn) in enumerate((("w_sq", "w_sk"), ("w_cq", "w_ck"),
                                  ("w_rq", "w_rk"))):
        shared[f"m{p}"] = _m_matrix(inputs[qn], inputs[kn])
        if need_mt:
            shared[f"mt{p}"] = _m_matrix(inputs[qn], inputs[kn], transposed=True)

    wanom_pm = w("w_anom").reshape(ND, 128).T            # [128, ND]
    emb = np.zeros((128, ND, BC, BC), np.float32)
    for s in range(BC):
        emb[:, :, s, s] = wanom_pm * PRE
    shared["wanom_emb"] = emb.astype(np_bf16)

    shared["w_f1"] = np.ascontiguousarray(
        w("w_f1").reshape(NC3, 128, D)).astype(np_bf16)
    shared["w_f2"] = np.ascontiguousarray(
        w("w_f2").reshape(ND, 128, D)).astype(np_bf16)
    shared["b_f1"] = w("b_f1").reshape(1, D).astype(np_bf16)
    shared["b_f2"] = w("b_f2").reshape(1, D).astype(np_bf16)
    shared["ln_g"] = np.broadcast_to(w("ln_g").reshape(1, D),
                                     (BC, D)).copy()
    shared["ln_b"] = np.broadcast_to(w("ln_b").reshape(1, D),
                                     (BC, D)).copy()

    in_maps = []
    core_idx = []
    for c in range(NCORES):
        idx = order[np.arange(BC) * NCORES + c]
        core_idx.append(idx)
        xs = x[idx]                                      # [BC, L, D] f32
        m = dict(shared)
        m["x"] = xs.astype(np_bf16)
        xsT = np.ascontiguousarray(xs.transpose(2, 0, 1))   # [D, BC, L] f32
        xq_f = np.ascontiguousarray(np.concatenate(
            [xsT[:, s, 0 : CQs[s]] for s in range(BC)], axis=1,
        ))
        m["xq"] = xq_f.reshape(ND, 128, SQ).astype(np_x)
        if need_xqb:
            m["xqb"] = xq_f.reshape(ND, 128, SQ).astype(np_bf16)
        if SO:
            xo_f = np.ascontiguousarray(np.concatenate(
                [xsT[:, s, OJs[s] : L] for s in range(BC)], axis=1,
            ))
            m["xo"] = xo_f.reshape(ND, 128, SO).astype(np_sc)
            if need_xo8:
                m["xo8"] = xo_f.reshape(ND, 128, SO).astype(np_x)
        m["fbias"] = np.ascontiguousarray(fb[idx])
        m["obias"] = np.ascontiguousarray(ob[idx])
        in_maps.append(m)
    return in_maps, pair_geo, core_idx


def get_program_fast(pair_geo):
    if pair_geo not in _PROGRAM_CACHE:
        _PROGRAM_CACHE[pair_geo] = build_program_fast(pair_geo)
    return _PROGRAM_CACHE[pair_geo]


def run(trace=False, **inputs):
    use_m = all(
        not np.any(np.asarray(inputs[n]))
        for n in ("b_sq", "b_sk", "b_cq", "b_ck", "b_rq", "b_rk")
    )
    if not use_m:
        return _run_legacy(trace=trace, **inputs)
    in_maps, pair_geo, core_idx = _host_prep_fast(inputs)
    nc = get_program_fast(pair_geo)
    res = bass_utils.run_bass_kernel_spmd(
        nc, in_maps, core_ids=list(range(NCORES)), trace=trace
    )
    out = np.empty((B, D), np.float32)
    for c in range(NCORES):
        out[core_idx[c]] = res.results[c]["out"]
    return out, res


def kernel(**inputs):
    out, _ = run(trace=False, **inputs)
    return out


# ---------------------------------------------------------------------------
# Legacy fallback (nonzero projection biases): original per-slot program.
# ---------------------------------------------------------------------------

PROJ_NAMES = ["w_sq", "w_sk", "w_cq", "w_ck", "w_rq", "w_rk"]
PBIAS_NAMES = ["b_sq", "b_sk", "b_cq", "b_ck", "b_rq", "b_rk"]
QS, KS, QC, KC, QR, KR = range(6)
QPROJ = (QS, QC, QR)
_LEGACY_CACHE = {}


def build_program_legacy(bounds=((2, 2),) * BC):
    nc = bacc.Bacc(
        "TRN2",
        target_bir_lowering=False,
        debug=False,
        enable_asserts=False,
        num_devices=NCORES,
    )

    xT_d = nc.dram_tensor("xT", [BC, D, L], bf16, kind="ExternalInput").ap()
    x_d = nc.dram_tensor("x", [BC, L, D], f32, kind="ExternalInput").ap()
    fmask_d = nc.dram_tensor("fmask", [BC, L], f32, kind="ExternalInput").ap()
    obias_d = nc.dram_tensor("obias", [BC, L], bf16, kind="ExternalInput").ap()

    W_d = {p: nc.dram_tensor(PROJ_NAMES[p], [D, D], bf16, kind="ExternalInput").ap()
           for p in range(6)}
    Brow_d = {
        p: nc.dram_tensor(PBIAS_NAMES[p], [1, D], bf16, kind="ExternalInput").ap()
        for p in range(6)}
    wanom_d = nc.dram_tensor("w_anom", [D, 1], bf16, kind="ExternalInput").ap()
    wf1_d = nc.dram_tensor("w_f1", [ND, 128, NC3 * 128], bf16, kind="ExternalInput").ap()
    wf2_d = nc.dram_tensor("w_f2", [ND, 128, ND * 128], bf16, kind="ExternalInput").ap()
    bf1_d = nc.dram_tensor("b_f1", [128, ND], f32, kind="ExternalInput").ap()
    bf2_d = nc.dram_tensor("b_f2", [128, ND], f32, kind="ExternalInput").ap()
    lng_d = nc.dram_tensor("ln_g", [128, ND], f32, kind="ExternalInput").ap()
    lnb_d = nc.dram_tensor("ln_b", [128, ND], f32, kind="ExternalInput").ap()

    out_d = nc.dram_tensor("out", [BC, D], f32, kind="ExternalOutput").ap()

    with tile.TileContext(nc) as tc, ExitStack() as ctx:
        const_p = ctx.enter_context(tc.tile_pool(name="const", bufs=1))
        tmp_p = ctx.enter_context(tc.tile_pool(name="tmp", bufs=2))
        sm_p = ctx.enter_context(tc.tile_pool(name="small", bufs=3))
        tail_p = ctx.enter_context(tc.tile_pool(name="tail", bufs=1))
        ps_big = ctx.enter_context(tc.tile_pool(name="psb", bufs=4, space="PSUM"))
        ps_s = ctx.enter_context(tc.tile_pool(name="pss", bufs=4, space="PSUM"))
        es2 = ExitStack()   # closed after phase C: x, E
        x_p = es2.enter_context(tc.tile_pool(name="x", bufs=3))
        e_p = es2.enter_context(tc.tile_pool(name="emat", bufs=2))
        es1 = ExitStack()   # closed after phase B: xT, W, proj
        xT_p = es1.enter_context(tc.tile_pool(name="xT", bufs=1))
        w_p = es1.enter_context(tc.tile_pool(name="w", bufs=2))
        proj_p = es1.enter_context(tc.tile_pool(name="proj", bufs=1))

        ones_row = const_p.tile([1, L], bf16)
        nc.vector.memset(ones_row[:], 1.0)
        ones_f = const_p.tile([1, 128], f32)
        nc.vector.memset(ones_f[:], 1.0)
        ones_col = const_p.tile([128, 1], f32)
        nc.vector.memset(ones_col[:], 1.0)
        iot_t = const_p.tile([128, 128], mybir.dt.int32)
        nc.gpsimd.iota(iot_t[:], pattern=[[1, 128]], base=0, channel_multiplier=-1)
        ident_t = const_p.tile([128, 128], f32)
        nc.vector.tensor_scalar(ident_t[:], iot_t[:], scalar1=0, scalar2=None,
                                op0=ALU.is_equal)

        wanom_t = const_p.tile([128, ND], bf16)
        nc.scalar.dma_start(wanom_t[:], wanom_d[:, 0].rearrange("(k p) -> p k", p=128))
        brow_t = {}
        for p in Brow_d:
            brow_t[p] = const_p.tile([1, D], bf16, name=f"brow{p}")
            nc.sync.dma_start(brow_t[p][:], Brow_d[p][:])
        bf1_t = const_p.tile([128, ND], f32)
        nc.scalar.dma_start(bf1_t[:], bf1_d[:])
        bf2_t = const_p.tile([128, ND], f32)
        nc.scalar.dma_start(bf2_t[:], bf2_d[:])
        lng_t = const_p.tile([128, ND], f32)
        nc.scalar.dma_start(lng_t[:], lng_d[:])
        lnb_t = const_p.tile([128, ND], f32)
        nc.scalar.dma_start(lnb_t[:], lnb_d[:])

        fusedT = tail_p.tile([128, NC3, BC], bf16)

        geo = []
        for s in range(BC):
            F, J0 = bounds[s]
            geo.append((F, J0, F * 128, J0 * 128, L - J0 * 128,
                        F > 0 and L - J0 * 128 > 0))

        xT_t = xT_p.tile([128, BC * ND, L], bf16)
        fm_ts, ob_ts, x_ts = [], [], []
        for s in range(BC):
            nc.sync.dma_start(
                xT_t[:, s * ND : (s + 1) * ND, :],
                xT_d[s].rearrange("(k p) i -> p k i", p=128),
            )
            fm_t = sm_p.tile([128, NL], f32, tag="fm", bufs=BC, name=f"fm{s}")
            nc.scalar.dma_start(fm_t[:], fmask_d[s].rearrange("(t p) -> p t", p=128))
            fm_ts.append(fm_t)
            ob_t = sm_p.tile([1, L], bf16, tag="ob", bufs=2, name=f"ob{s}")
            nc.scalar.dma_start(ob_t[:], obias_d[s : s + 1, :])
            ob_ts.append(ob_t)

        gate_ts = []
        for s in range(BC):
            F, J0, CQ, OJ, NO, have_attn = geo[s]
            gate_t = sm_p.tile([128, NL], f32, tag="gate", bufs=BC, name=f"gate{s}")
            gate_ts.append(gate_t)
            if F == 0:
                continue
            ghat_t = sm_p.tile([128, NL], f32, tag="ghat")
            for it in range(F):
                al_ps = ps_s.tile([128, 1], f32, tag="pss")
                for k in range(ND):
                    nc.tensor.matmul(
                        al_ps[:],
                        lhsT=xT_t[:, s * ND + k, it * 128 : (it + 1) * 128],
                        rhs=wanom_t[:, k : k + 1],
                        start=(k == 0), stop=(k == ND - 1),
                    )
                eg_t = sm_p.tile([128, 1], f32, tag="eg")
                nc.scalar.activation(eg_t[:], al_ps[:], AF.Exp)
                nc.vector.tensor_mul(
                    ghat_t[:, it : it + 1], eg_t[:], fm_ts[s][:, it : it + 1]
                )
            gsum_t = sm_p.tile([128, 1], f32, tag="gsum")
            nc.vector.tensor_reduce(
                gsum_t[:], ghat_t[:, 0:F], axis=mybir.AxisListType.X, op=ALU.add
            )
            S_ps = ps_s.tile([1, 1], f32, tag="pss")
            nc.tensor.matmul(S_ps[:], lhsT=gsum_t[:], rhs=ones_col[:],
                             start=True, stop=True)
            Smax_t = sm_p.tile([1, 1], f32, tag="Smax")
            nc.vector.tensor_scalar_max(Smax_t[:], S_ps[:], 1e-8)
            Sb_ps = ps_s.tile([128, 1], f32, tag="pss")
            nc.tensor.matmul(Sb_ps[:], lhsT=ones_f[:], rhs=Smax_t[:],
                             start=True, stop=True)
            recipS_t = sm_p.tile([128, 1], f32, tag="recipS")
            nc.vector.reciprocal(recipS_t[:], Sb_ps[:])
            nc.vector.tensor_scalar_mul(gate_t[:, 0:F], ghat_t[:, 0:F],
                                        recipS_t[:])

        projs = [[None] * BC for _ in range(6)]
        for p in range(6):
            qside = p in QPROJ
            widths = [
                ((g[2] if qside else g[4]) if g[5] else 0) for g in geo
            ]
            wmax = max(widths)
            if wmax == 0:
                continue
            wt = w_p.tile([128, ND, D], bf16, tag="w", name=f"w{p}")
            nc.gpsimd.dma_start(wt[:], W_d[p].rearrange("(k p) c -> p k c", p=128))
            pt = proj_p.tile([128, BC, ND, wmax], bf16, tag=f"proj{p}")
            for m in range(ND):
                for s in range(BC):
                    width = widths[s]
                    if width == 0:
                        continue
                    lo = 0 if qside else geo[s][3]
                    ps = ps_big.tile([128, width], f32, tag="ps")
                    for k in range(ND):
                        nc.tensor.matmul(
                            ps[:], lhsT=wt[:, k, m * 128 : (m + 1) * 128],
                            rhs=xT_t[:, s * ND + k, lo : lo + width],
                            start=(k == 0), stop=False,
                        )
                    nc.tensor.matmul(
                        ps[:], lhsT=brow_t[p][:, m * 128 : (m + 1) * 128],
                        rhs=ones_row[:, 0:width], start=False, stop=True,
                    )
                    nc.vector.tensor_copy(pt[:, s, m, :], ps[:])
            for s in range(BC):
                if widths[s]:
                    projs[p][s] = pt

        for s in range(BC):
            x_t = x_p.tile([128, NL, D], f32, tag="x", name=f"x{s}")
            nc.sync.dma_start(x_t[:], x_d[s].rearrange("(t p) d -> p t d", p=128))
            x_ts.append(x_t)

        E_sups, E_reps, co_sups, co_reps = {}, {}, {}, {}
        for s in range(BC):
            F, J0, CQ, OJ, NO, have_attn = geo[s]
            if not have_attn:
                continue
            E_sup = e_p.tile([128, max(F, 1), NO], f32, tag="esup", bufs=BC,
                             name=f"esup{s}")
            E_rep = e_p.tile([128, max(F, 1), NO], f32, tag="erep", bufs=BC,
                             name=f"erep{s}")
            co_sup = sm_p.tile([128, NL], f32, tag="cosup", bufs=BC,
                               name=f"cosup{s}")
            co_rep = sm_p.tile([128, NL], f32, tag="corep", bufs=BC,
                               name=f"corep{s}")
            E_sups[s], E_reps[s] = E_sup, E_rep
            co_sups[s], co_reps[s] = co_sup, co_rep
            gate_t = gate_ts[s]
            ob_t = ob_ts[s]
            for it in range(F):
                isl = slice(it * 128, (it + 1) * 128)
                ps_sup = ps_big.tile([128, NO], f32, tag="ps")
                for k in range(ND):
                    nc.tensor.matmul(
                        ps_sup[:], lhsT=projs[QS][s][:, s, k, isl],
                        rhs=projs[KS][s][:, s, k, 0:NO],
                        start=(k == 0), stop=False,
                    )
                nc.tensor.matmul(ps_sup[:], lhsT=ones_row[:, 0:128],
                                 rhs=ob_t[:, OJ:L], start=False, stop=True)
                ps_con = ps_big.tile([128, NO], f32, tag="ps")
                for k in range(ND):
                    nc.tensor.matmul(
                        ps_con[:], lhsT=projs[QC][s][:, s, k, isl],
                        rhs=projs[KC][s][:, s, k, 0:NO],
                        start=(k == 0), stop=(k == ND - 1),
                    )
                ps_rep = ps_big.tile([128, NO], f32, tag="ps")
                for k in range(ND):
                    nc.tensor.matmul(
                        ps_rep[:], lhsT=projs[QR][s][:, s, k, isl],
                        rhs=projs[KR][s][:, s, k, 0:NO],
                        start=(k == 0), stop=False,
                    )
                nc.tensor.matmul(ps_rep[:], lhsT=ones_row[:, 0:128],
                                 rhs=ob_t[:, OJ:L], start=False, stop=True)

                T_t = tmp_p.tile([128, NO], f32, tag="T")
                nc.scalar.activation(T_t[:], ps_con[:], AF.Tanh, scale=SCALE)
                A_t = tmp_p.tile([128, NO], f32, tag="A")
                nc.vector.scalar_tensor_tensor(
                    A_t[:], in0=ps_rep[:], scalar=SCALE, in1=T_t[:],
                    op0=ALU.mult, op1=ALU.add,
                )
                rs_sup = sm_p.tile([128, 1], f32, tag="rssup")
                nc.scalar.activation(E_sup[:, it, :], ps_sup[:], AF.Exp,
                                     scale=SCALE, accum_out=rs_sup[:])
                rs_rep = sm_p.tile([128, 1], f32, tag="rsrep")
                nc.scalar.activation(E_rep[:, it, :], A_t[:], AF.Exp,
                                     accum_out=rs_rep[:])
                rc_sup = sm_p.tile([128, 1], f32, tag="rcsup")
                nc.vector.reciprocal(rc_sup[:], rs_sup[:])
                nc.vector.tensor_mul(co_sup[:, it : it + 1],
                                     gate_t[:, it : it + 1], rc_sup[:])
                rc_rep = sm_p.tile([128, 1], f32, tag="rcrep")
                nc.vector.reciprocal(rc_rep[:], rs_rep[:])
                nc.vector.tensor_mul(co_rep[:, it : it + 1],
                                     gate_t[:, it : it + 1], rc_rep[:])

        es1.close()

        for s in range(BC):
            F, J0, CQ, OJ, NO, have_attn = geo[s]
            x_t = x_ts[s]

            G_t = sm_p.tile([128, NL, 3], f32, tag="G")
            nc.vector.memset(G_t[:], 0.0)
            if F > 0:
                for it in range(F):
                    nc.vector.tensor_copy(G_t[:, it, 0:1],
                                          gate_ts[s][:, it : it + 1])
            if have_attn:
                E_sup, E_rep = E_sups[s], E_reps[s]
                co_sup, co_rep = co_sups[s], co_reps[s]
                for jt in range(J0, NL):
                    jsl = slice(jt * 128 - OJ, jt * 128 - OJ + 128)
                    r_ps = ps_s.tile([128, 2], f32, tag="pss")
                    for it in range(F):
                        nc.tensor.matmul(
                            r_ps[:, 0:1], lhsT=E_rep[:, it, jsl],
                            rhs=co_rep[:, it : it + 1],
                            start=(it == 0), stop=(it == F - 1),
                        )
                    for it in range(F):
                        nc.tensor.matmul(
                            r_ps[:, 1:2], lhsT=E_sup[:, it, jsl],
                            rhs=co_sup[:, it : it + 1],
                            start=(it == 0), stop=(it == F - 1),
                        )
                    nc.vector.tensor_copy(G_t[:, jt, 1:3], r_ps[:, 0:2])

            rts = sorted(set(range(F)) | (set(range(J0, NL)) if have_attn else set()))
            if not rts:
                rts = [0]
            for m in range(ND):
                pool_ps = ps_s.tile([128, 3], f32, tag="pss")
                for i, rt in enumerate(rts):
                    nc.tensor.matmul(
                        pool_ps[:], lhsT=x_t[:, rt, m * 128 : (m + 1) * 128],
                        rhs=G_t[:, rt, :],
                        start=(i == 0), stop=(i == len(rts) - 1),
                    )
                for t in range(3):
                    nc.vector.tensor_copy(
                        fusedT[:, t * ND + m, s : s + 1], pool_ps[:, t : t + 1]
                    )

        es2.close()

        wf1_p = ctx.enter_context(tc.tile_pool(name="wf1", bufs=8))
        hT_t = tail_p.tile([128, ND, BC], bf16)
        for m in range(ND):
            wt = wf1_p.tile([128, NC3, 128], bf16, tag="wf1")
            nc.gpsimd.dma_start(wt[:], wf1_d[m].rearrange("p (k c) -> p k c", c=128))
            h_ps = ps_s.tile([128, BC], f32, tag="pss")
            for k in range(NC3):
                nc.tensor.matmul(h_ps[:], lhsT=wt[:, k, :], rhs=fusedT[:, k, :],
                                 start=(k == 0), stop=(k == NC3 - 1))
            nc.scalar.activation(hT_t[:, m, :], h_ps[:], AF.Relu,
                                 bias=bf1_t[:, m : m + 1])

        yT_t = tail_p.tile([128, ND, BC], f32)
        sq_t = tail_p.tile([128, ND, BC], f32)
        for m in range(ND):
            wt = wf1_p.tile([128, ND, 128], bf16, tag="wf2")
            nc.gpsimd.dma_start(wt[:], wf2_d[m].rearrange("p (k c) -> p k c", c=128))
            y_ps = ps_s.tile([128, BC], f32, tag="pss")
            for k in range(ND):
                nc.tensor.matmul(y_ps[:], lhsT=wt[:, k, :], rhs=hT_t[:, k, :],
                                 start=(k == 0), stop=(k == ND - 1))
            nc.vector.tensor_scalar_add(yT_t[:, m, :], y_ps[:], bf2_t[:, m : m + 1])
            nc.scalar.square(sq_t[:, m, :], yT_t[:, m, :])

        sum_ps = ps_s.tile([1, BC], f32, tag="pss")
        for m in range(ND):
            nc.tensor.matmul(sum_ps[:], lhsT=ones_col[:], rhs=yT_t[:, m, :],
                             start=(m == 0), stop=(m == ND - 1))
        ssq_ps = ps_s.tile([1, BC], f32, tag="pss")
        for m in range(ND):
            nc.tensor.matmul(ssq_ps[:], lhsT=ones_col[:], rhs=sq_t[:, m, :],
                             start=(m == 0), stop=(m == ND - 1))
        mean_t = sm_p.tile([1, BC], f32, tag="mean")
        nc.scalar.mul(mean_t[:], sum_ps[:], 1.0 / D)
        msq_t = sm_p.tile([1, BC], f32, tag="msq")
        nc.scalar.mul(msq_t[:], ssq_ps[:], 1.0 / D)
        m2_t = sm_p.tile([1, BC], f32, tag="m2")
        nc.vector.tensor_mul(m2_t[:], mean_t[:], mean_t[:])
        var_t = sm_p.tile([1, BC], f32, tag="var")
        nc.vector.tensor_sub(var_t[:], msq_t[:], m2_t[:])
        nc.vector.tensor_scalar_add(var_t[:], var_t[:], LN_EPS)
        sd_t = sm_p.tile([1, BC], f32, tag="sd")
        nc.scalar.sqrt(sd_t[:], var_t[:])
        rstd_t = sm_p.tile([1, BC], f32, tag="rstd")
        nc.vector.reciprocal(rstd_t[:], sd_t[:])

        mb_ps = ps_s.tile([128, BC], f32, tag="pss")
        nc.tensor.matmul(mb_ps[:], lhsT=ones_f[:], rhs=mean_t[:],
                         start=True, stop=True)
        mb_t = sm_p.tile([128, BC], f32, tag="mbt")
        nc.vector.tensor_copy(mb_t[:], mb_ps[:])
        rb_ps = ps_s.tile([128, BC], f32, tag="pss")
        nc.tensor.matmul(rb_ps[:], lhsT=ones_f[:], rhs=rstd_t[:],
                         start=True, stop=True)
        rb_t = sm_p.tile([128, BC], f32, tag="rbt")
        nc.vector.tensor_copy(rb_t[:], rb_ps[:])

        zrow_t = tail_p.tile([BC, D], f32)
        for m in range(ND):
            z_t = tmp_p.tile([128, BC], f32, tag="z")
            nc.vector.tensor_sub(z_t[:], yT_t[:, m, :], mb_t[:])
            nc.vector.tensor_mul(z_t[:], z_t[:], rb_t[:])
            z2_t = tmp_p.tile([128, BC], f32, tag="z2")
            nc.vector.tensor_scalar(
                z2_t[:], z_t[:], scalar1=lng_t[:, m : m + 1],
                scalar2=lnb_t[:, m : m + 1], op0=ALU.mult, op1=ALU.add,
            )
            tr_ps = ps_s.tile([BC, 128], f32, tag="pss")
            nc.tensor.transpose(tr_ps[:], z2_t[:], ident_t[:])
            nc.vector.tensor_copy(zrow_t[:, m * 128 : (m + 1) * 128], tr_ps[:])
        nc.sync.dma_start(out_d[:, :], zrow_t[:, :])

    nc.compile()
    return nc


def _run_legacy(trace=False, **inputs):
    x = np.asarray(inputs["x"], dtype=np.float32)
    x_ids = np.asarray(inputs["x_ids"])
    pad_idx = int(np.asarray(inputs["pad_idx"]))
    sep_idx = int(np.asarray(inputs["sep_idx"]))

    sp, fmask_b, omask = _masks(x_ids, pad_idx, sep_idx)
    fmask = fmask_b.astype(np.float32)
    obias = np.where(omask, 0.0, OBIAS_RAW).astype(np.float32)

    F_all = np.ceil(sp / 128).astype(int)
    J0_all = np.minimum((sp + 1) // 128, NL)
    bounds = tuple(
        (int(F_all.reshape(NCORES, BC)[:, s].max()),
         int(J0_all.reshape(NCORES, BC)[:, s].min()))
        for s in range(BC)
    )

    xT = np.ascontiguousarray(x.transpose(0, 2, 1))

    def w(name):
        return np.ascontiguousarray(np.asarray(inputs[name], dtype=np.float32))

    def ppart(name):
        return np.ascontiguousarray(np.asarray(inputs[name], dtype=np.float32)
                                    .reshape(ND, 128).T)

    shared = {}
    for p in range(6):
        shared[PROJ_NAMES[p]] = w(PROJ_NAMES[p]).astype(np_bf16)
        shared[PBIAS_NAMES[p]] = w(PBIAS_NAMES[p]).reshape(1, D).astype(np_bf16)
    shared["w_anom"] = w("w_anom").reshape(D, 1).astype(np_bf16)

    def mpack(name, nk):
        a = w(name)
        a = a.reshape(nk, 128, ND, 128).transpose(2, 1, 0, 3).reshape(ND, 128, nk * 128)
        return np.ascontiguousarray(a).astype(np_bf16)

    shared["w_f1"] = mpack("w_f1", NC3)
    shared["w_f2"] = mpack("w_f2", ND)
    shared["b_f1"] = ppart("b_f1")
    shared["b_f2"] = ppart("b_f2")
    shared["ln_g"] = ppart("ln_g")
    shared["ln_b"] = ppart("ln_b")

    in_maps = []
    for c in range(NCORES):
        sl = slice(c * BC, (c + 1) * BC)
        m = dict(shared)
        m["x"] = np.ascontiguousarray(x[sl])
        m["xT"] = np.ascontiguousarray(xT[sl]).astype(np_bf16)
        m["fmask"] = np.ascontiguousarray(fmask[sl])
        m["obias"] = np.ascontiguousarray(obias[sl]).astype(np_bf16)
        in_maps.append(m)

    if bounds not in _LEGACY_CACHE:
        _LEGACY_CACHE[bounds] = build_program_legacy(bounds)
    nc = _LEGACY_CACHE[bounds]
    res = bass_utils.run_bass_kernel_spmd(
        nc, in_maps, core_ids=list(range(NCORES)), trace=trace
    )
    out = np.concatenate([res.results[c]["out"] for c in range(NCORES)], axis=0)
    return out.astype(np.float32), res


# revision 22
# speedup vs baseline: 2.1295x; 1.0104x over previous
"""Trainium2 Bass kernel for nn_BertCounterFactTransformer.

Contract: kernel(**inputs) takes FULL unsharded numpy inputs (as produced by
reference.setup_inputs()) and returns the FULL [32, 1024] float32 output.

Strategy (data-parallel over batch, 8 cores x 4 samples):
  - Host: compute sep positions from x_ids, SORT samples by sep position and
    assign sorted rank r -> core (r % 8), slot (r // 8) so the per-slot-pair
    tile bounds are tight and uniform across cores. Precompute
    M_p = W_pq @ W_pk^T (bf16) so scores are x M x^T (no k-side projection).
  - Device, per pair of slots (F tiles of false rows, option cols from OJ):
      gate       all-4-sample anomaly logits in ONE [4, wg] PSUM via
                 block-diagonal embedded w_anom; false-mask folded as a
                 -30 bias row; exp+normalize row-wise; PE-transpose to cols
      proj       qT = (x M_p)^T (or M_p x_opt^T if the option side is
                 smaller), 2 samples batched per matmul (width<=512)
      scores     S = q @ x_opt^T blocks; option mask via -960 bias rows
      E_sup = exp(S_sup/32 + ob), E_rep = exp(S_rep/32 + tanh(S_con/32) + ob)
      coeff_t = gate / rowsum(E_t);  r_t = E_t^T @ coeff_t   (width-1 chains)
  - Pool: ONE [12, 512] PSUM accumulates x_s^T @ [gate|r_rep|r_sup] for all
    4 samples via 12-col G with per-sample zero blocks; PE-transpose the
    [12, 1024] result into fused^T columns.
  - Tail in row form: h = relu(fused @ W1 + b1), y = h @ W2 + b2, LayerNorm
    along the free dim, direct [4, 1024] output DMA.

Key identity: gate @ (attn @ x) == (gate @ attn) @ x, so [L,D] attention
outputs are never materialized.
"""

import sys

if "/opt/trn_rl_repo" not in sys.path:
    sys.path.insert(0, "/opt/trn_rl_repo")

import numpy as np
import ml_dtypes
from contextlib import ExitStack

np_bf16 = ml_dtypes.bfloat16
np_fp8 = ml_dtypes.float8_e4m3

import concourse.bacc as bacc
import concourse.bass as bass
import concourse.mybir as mybir
import concourse.tile as tile
from concourse import bass_utils

f32 = mybir.dt.float32
bf16 = mybir.dt.bfloat16
fp8 = mybir.dt.float8e4
AF = mybir.ActivationFunctionType
ALU = mybir.AluOpType

B, L, D = 32, 512, 1024
NCORES = 8
BC = B // NCORES          # samples per core
NL = L // 128             # 4 L-tiles
ND = D // 128             # 8 D-tiles
NC3 = 3 * D // 128        # 24 tiles of the 3D fused dim
SCALE = 1.0 / 32.0        # 1/sqrt(D)
OBIAS_RAW = -960.0        # -30 after * SCALE
FBIAS = -30.0
LN_EPS = 1e-5
USE_FP8 = True            # fp8 e4m3 + DoubleRow for the projection GEMMs
FP8_SCORES = True         # score operands (projT, xo) also fp8 + DoubleRow
PRE = 64.0 if USE_FP8 else 1.0   # pre-scale on M/w_anom (fp8 normal range)

_PROGRAM_CACHE = {}
_M_CACHE = {}


def _m_matrix(wq, wk, transposed=False):
    import hashlib
    wq = np.asarray(wq, dtype=np.float32)
    wk = np.asarray(wk, dtype=np.float32)
    key = (hashlib.blake2b(wq.tobytes() + wk.tobytes(), digest_size=16).digest(),
           transposed, USE_FP8)
    if key not in _M_CACHE:
        m = wq @ wk.T
        if transposed:
            m = m.T
        m = np.ascontiguousarray(m)
        if USE_FP8:
            _M_CACHE[key] = np.clip(m * PRE, -240, 240).astype(np_fp8)
        else:
            _M_CACHE[key] = m.astype(np_bf16)
    return _M_CACHE[key]


def _geo(F, J0):
    OJ = J0 * 128
    NO = L - OJ
    CQ = F * 128
    have = NO > 0
    side_q = (CQ <= NO) if have else True
    w = (CQ if side_q else NO) if have else 0
    lo = 0 if side_q else OJ
    return dict(F=F, J0=J0, OJ=OJ, NO=NO, CQ=CQ, have=have,
                side_q=side_q, w=w, lo=lo)


def build_program_fast(pair_geo, ln_trivial=False):
    """pair_geo = ((F0, J0_0), (F1, J0_1)); pair p covers slots {2p, 2p+1}.
    Computing a superset is always correct (bias masks zero it)."""
    nc = bacc.Bacc(
        "TRN2",
        target_bir_lowering=False,
        debug=False,
        enable_asserts=False,
        num_devices=NCORES,
    )

    geos = [_geo(F, J0) for (F, J0) in pair_geo]
    need_mt = any(g["have"] and not g["side_q"] for g in geos)
    wg = max(max(g["CQ"] for g in geos), 128)       # gate width (cols 0..wg)
    maxF = max(g["F"] for g in geos)
    DT_X = fp8 if USE_FP8 else bf16
    DT_SC = fp8 if (USE_FP8 and FP8_SCORES) else bf16
    need_xqb = USE_FP8             # gate chains always run bf16
    CQs = [geos[s // 2]["CQ"] for s in range(BC)]
    NOs = [geos[s // 2]["NO"] for s in range(BC)]
    offq = [sum(CQs[:s]) for s in range(BC)]
    offo = [sum(NOs[:s]) for s in range(BC)]
    SQ, SO = sum(CQs), sum(NOs)

    # per-partition SBUF estimate (bytes) with w1-half0 resident early
    _xb = 1 if USE_FP8 else 2
    _scb = 1 if (USE_FP8 and FP8_SCORES) else 2
    _est = (ND * SQ * _xb + ND * SO * _scb + BC * NL * D * 2     # xq, xo, x
            + (ND * SQ * 2 if USE_FP8 else 0)                    # xqb
            + 3 * ND * D * _xb * (2 if need_mt else 1)           # M (+MT)
            + max(ND * 2 * g["w"] * _scb for g in geos) * 3      # projT
            + max(4 * g["F"] * g["NO"] * 2 for g in geos)        # E (bf16)
            + NC3 * 512 * 2                                      # w1 half 0
            + 60 * 1024)                                         # misc + slack
    early_tail = _est <= 200 * 1024

    xq_d = nc.dram_tensor("xq", [ND, 128, SQ], DT_X, kind="ExternalInput").ap()
    xqb_d = (nc.dram_tensor("xqb", [ND, 128, SQ], bf16,
                            kind="ExternalInput").ap() if need_xqb else None)
    need_xo8 = (DT_SC != DT_X) and any(
        g["have"] and not g["side_q"] for g in geos)
    xo_d = (nc.dram_tensor("xo", [ND, 128, SO], DT_SC, kind="ExternalInput").ap()
            if SO else None)
    xo8_d = (nc.dram_tensor("xo8", [ND, 128, SO], DT_X,
                            kind="ExternalInput").ap()
             if (SO and need_xo8) else None)
    x_d = nc.dram_tensor("x", [BC, L, D], bf16, kind="ExternalInput").ap()
    fb_d = nc.dram_tensor("fbias", [BC, L], bf16, kind="ExternalInput").ap()
    ob_d = nc.dram_tensor("obias", [BC, L], bf16, kind="ExternalInput").ap()
    m_d = [nc.dram_tensor(f"m{p}", [D, D], DT_X, kind="ExternalInput").ap()
           for p in range(3)]
    mt_d = ([nc.dram_tensor(f"mt{p}", [D, D], DT_X, kind="ExternalInput").ap()
             for p in range(3)] if need_mt else None)
    emb_d = nc.dram_tensor("wanom_emb", [128, ND, BC, BC], bf16,
                           kind="ExternalInput").ap()
    w1_d = nc.dram_tensor("w_f1", [NC3, 128, D], bf16, kind="ExternalInput").ap()
    w2_d = nc.dram_tensor("w_f2", [ND, 128, D], bf16, kind="ExternalInput").ap()
    b1_d = nc.dram_tensor("b_f1", [1, D], bf16, kind="ExternalInput").ap()
    b2_d = nc.dram_tensor("b_f2", [1, D], bf16, kind="ExternalInput").ap()
    lng_d = lnb_d = None
    if not ln_trivial:
        lng_d = nc.dram_tensor("ln_g", [BC, D], f32, kind="ExternalInput").ap()
        lnb_d = nc.dram_tensor("ln_b", [BC, D], f32, kind="ExternalInput").ap()
    out_d = nc.dram_tensor("out", [BC, D], f32, kind="ExternalOutput").ap()

    with tile.TileContext(nc) as tc, ExitStack() as ctx:
        const_p = ctx.enter_context(tc.tile_pool(name="const", bufs=1))
        main_p = ctx.enter_context(tc.tile_pool(name="main", bufs=1))
        sm_p = ctx.enter_context(tc.tile_pool(name="small", bufs=2))
        tmp_p = ctx.enter_context(tc.tile_pool(name="tmp", bufs=2))
        ps_big = ctx.enter_context(tc.tile_pool(name="psb", bufs=4, space="PSUM"))
        ps_med = ctx.enter_context(tc.tile_pool(name="psm", bufs=2, space="PSUM"))
        ps_sm = ctx.enter_context(tc.tile_pool(name="pss", bufs=2, space="PSUM"))
        tailA_p = (ctx.enter_context(tc.tile_pool(name="tailA", bufs=1))
                   if early_tail else None)
        es2 = ExitStack()   # E matrices; closed after last r
        e_p = es2.enter_context(tc.tile_pool(name="emat", bufs=1))
        es1 = ExitStack()   # W matrices + projT; closed after last scores
        w_p = es1.enter_context(tc.tile_pool(name="w", bufs=1))
        proj_p = es1.enter_context(tc.tile_pool(name="proj", bufs=1))

        # ---- constants ----
        ones_row = const_p.tile([1, 128], bf16)
        nc.vector.memset(ones_row[:], 1.0)
        ones4 = const_p.tile([1, BC], bf16)
        nc.vector.memset(ones4[:], 1.0)
        eyerows = const_p.tile([1, BC, BC], bf16)
        nc.vector.memset(eyerows[:], 0.0)
        for s in range(BC):
            nc.vector.memset(eyerows[:, s, s : s + 1], 1.0)
        iot_t = const_p.tile([128, 128], mybir.dt.int32)
        nc.gpsimd.iota(iot_t[:], pattern=[[1, 128]], base=0, channel_multiplier=-1)
        ident_f = const_p.tile([128, 128], f32)
        nc.vector.tensor_scalar(ident_f[:], iot_t[:], scalar1=0, scalar2=None,
                                op0=ALU.is_equal)
        ident_b = const_p.tile([128, 128], bf16)
        nc.vector.tensor_copy(ident_b[:], ident_f[:])
        warm_t = const_p.tile([1, 1], f32)
        nc.scalar.sqrt(warm_t[:], ones4[0:1, 0:1])

        emb_t = const_p.tile([128, ND, BC, BC], bf16)
        nc.scalar.dma_start(emb_t[:], emb_d[:])
        fb_t = const_p.tile([1, BC, L], bf16)
        ob_t = const_p.tile([1, BC, L], bf16)
        for s in range(BC):
            nc.scalar.dma_start(fb_t[:, s, :], fb_d[s : s + 1, :])
            nc.scalar.dma_start(ob_t[:, s, :], ob_d[s : s + 1, :])
        b1_t = const_p.tile([1, D], bf16)
        nc.scalar.dma_start(b1_t[:], b1_d[:])
        b2_t = const_p.tile([1, D], bf16)
        nc.scalar.dma_start(b2_t[:], b2_d[:])
        lng_t = lnb_t = None
        if not ln_trivial:
            lng_t = const_p.tile([BC, D], f32)
            nc.scalar.dma_start(lng_t[:], lng_d[:])
            lnb_t = const_p.tile([BC, D], f32)
            nc.scalar.dma_start(lnb_t[:], lnb_d[:])

        # ---- big inputs ----
        xq_t = main_p.tile([128, ND, SQ], DT_X)
        for k in range(ND):
            nc.sync.dma_start(xq_t[:, k, :], xq_d[k])
        xqb_t = xq_t
        if need_xqb:
            xqb_t = w_p.tile([128, ND, SQ], bf16, name="xqb")
            for k in range(ND):
                nc.sync.dma_start(xqb_t[:, k, :], xqb_d[k])
        xo_t = None
        if SO:
            xo_t = main_p.tile([128, ND, SO], DT_SC)
            for k in range(ND):
                nc.sync.dma_start(xo_t[:, k, :], xo_d[k])
        xo8_t = xo_t
        if SO and need_xo8:
            xo8_t = main_p.tile([128, ND, SO], DT_X)
            for k in range(ND):
                nc.sync.dma_start(xo8_t[:, k, :], xo8_d[k])
        w_ts = []
        for p in range(3):
            wt = w_p.tile([128, ND, D], DT_X, name=f"w{p}")
            nc.gpsimd.dma_start(wt[:], m_d[p].rearrange("(k p) c -> p k c", p=128))
            w_ts.append(wt)
        wt_ts = []
        if need_mt:
            for p in range(3):
                wtt = w_p.tile([128, ND, D], DT_X, name=f"wt{p}")
                nc.gpsimd.dma_start(
                    wtt[:], mt_d[p].rearrange("(k p) c -> p k c", p=128)
                )
                wt_ts.append(wtt)
        x_t = main_p.tile([128, BC, NL, D], bf16)
        for s in range(BC):
            nc.sync.dma_start(
                x_t[:, s], x_d[s].rearrange("(t p) d -> p t d", p=128)
            )

        def _load_w1_half(pool, half, name):
            w1h = pool.tile([128, NC3, 512], bf16, name=name)
            hs = slice(half * 512, (half + 1) * 512)
            for t in range(3):
                nc.gpsimd.dma_start(
                    w1h[:, t * ND : (t + 1) * ND, :],
                    w1_d[t * ND : (t + 1) * ND, :, hs].rearrange(
                        "k p n -> p k n"),
                )
            return w1h

        w1h_t = [None, None]
        if early_tail:
            w1h_t[0] = _load_w1_half(tailA_p, 0, "w1a")

        G_all = main_p.tile([128, NL, BC, 12], bf16)   # col = t*4 + s
        nc.vector.memset(G_all[:], 0.0)
        gcol = main_p.tile([128, maxF, BC], f32)

        # ---- gates (all samples): logits in ONE [4, wg] PSUM ----
        ga_ps = ps_med.tile([BC, wg], f32, tag="pm")
        for s in range(BC):
            for k in range(ND):
                nc.tensor.matmul(
                    ga_ps[:, 0 : CQs[s]], lhsT=emb_t[:, k, s, :],
                    rhs=xqb_t[:, k, offq[s] : offq[s] + CQs[s]],
                    start=(s == 0 and k == 0), stop=False,
                )
        for s in range(BC):
            nc.tensor.matmul(
                ga_ps[:], lhsT=eyerows[:, s, :], rhs=fb_t[:, s, 0:wg],
                start=False, stop=(s == BC - 1),
            )
        grow = sm_p.tile([BC, wg], f32, tag="grow", bufs=1)
        gs_t = sm_p.tile([BC, 1], f32, tag="gs", bufs=1)
        nc.scalar.activation(grow[:], ga_ps[:], AF.Exp, scale=1.0 / PRE,
                             accum_out=gs_t[:])
        nc.vector.tensor_scalar_max(gs_t[:], gs_t[:], 1e-8)
        rg_t = sm_p.tile([BC, 1], f32, tag="rg", bufs=1)
        nc.vector.reciprocal(rg_t[:], gs_t[:])
        nc.vector.tensor_scalar_mul(grow[:], grow[:], rg_t[:])
        for it in range(maxF):
            gt_ps = ps_med.tile([128, BC], f32, tag="pm")
            nc.tensor.transpose(gt_ps[:], grow[:, it * 128 : (it + 1) * 128],
                                ident_f[0:BC, 0:BC])
            nc.vector.tensor_copy(gcol[:, it, :], gt_ps[:])
            for s in range(BC):
                if it < geos[s // 2]["F"]:
                    nc.vector.tensor_copy(G_all[:, it, s, s : s + 1],
                                          gt_ps[:, s : s + 1])

        # ---- per pair: projections -> scores -> E -> coeffs -> r ----
        for pr in range(2):
            g = geos[pr]
            F, OJ, NO, w, lo = g["F"], g["OJ"], g["NO"], g["w"], g["lo"]
            if not g["have"]:
                continue
            s0, s1 = 2 * pr, 2 * pr + 1
            wmats = w_ts if g["side_q"] else wt_ts

            xsrc = xq_t if g["side_q"] else xo8_t
            poff = offq if g["side_q"] else offo
            assert poff[s1] == poff[s0] + w

            projT = [
                proj_p.tile([128, ND, 2, w], DT_SC, tag=f"pj{p}",
                            name=f"pj{p}_{pr}")
                for p in range(3)
            ]
            DR = mybir.MatmulPerfMode.DoubleRow if USE_FP8 else None
            NK = ND // 2 if USE_FP8 else ND
            for p in range(3):
                for m in range(ND):
                    msl = slice(m * 128, (m + 1) * 128)
                    if 2 * w <= 512:
                        pj_ps = ps_big.tile([128, 2, w], f32, tag="ps")
                        for k in range(NK):
                            if USE_FP8:
                                nc.tensor.matmul(
                                    pj_ps[:],
                                    lhsT=wmats[p][:, 2 * k : 2 * k + 2, msl],
                                    rhs=xsrc[:, 2 * k : 2 * k + 2,
                                             poff[s0] : poff[s0] + 2 * w],
                                    start=(k == 0), stop=(k == NK - 1),
                                    perf_mode=DR,
                                )
                            else:
                                nc.tensor.matmul(
                                    pj_ps[:],
                                    lhsT=wmats[p][:, k, msl],
                                    rhs=xsrc[:, k, poff[s0] : poff[s0] + 2 * w],
                                    start=(k == 0), stop=(k == NK - 1),
                                )
                        if m % 2 == 0:
                            nc.vector.tensor_copy(projT[p][:, m], pj_ps[:])
                        else:
                            nc.scalar.activation(projT[p][:, m], pj_ps[:],
                                                 AF.Copy)
                    else:
                        for sp in range(2):
                            pj_ps = ps_big.tile([128, w], f32, tag="ps")
                            for k in range(NK):
                                if USE_FP8:
                                    nc.tensor.matmul(
                                        pj_ps[:],
                                        lhsT=wmats[p][:, 2 * k : 2 * k + 2, msl],
                                        rhs=xsrc[:, 2 * k : 2 * k + 2,
                                                 poff[s0 + sp] :
                                                 poff[s0 + sp] + w],
                                        start=(k == 0), stop=(k == NK - 1),
                                        perf_mode=DR,
                                    )
                                else:
                                    nc.tensor.matmul(
                                        pj_ps[:],
                                        lhsT=wmats[p][:, k, msl],
                                        rhs=xsrc[:, k,
                                                 poff[s0 + sp] :
                                                 poff[s0 + sp] + w],
                                        start=(k == 0), stop=(k == NK - 1),
                                    )
                            if sp == 0:
                                nc.vector.tensor_copy(projT[p][:, m, sp],
                                                      pj_ps[:])
                            else:
                                nc.scalar.activation(projT[p][:, m, sp],
                                                     pj_ps[:], AF.Copy)

            for sp in range(2):
                s4 = s0 + sp
                E_sup = e_p.tile([128, F, NO], bf16, tag=f"Es{sp}",
                                 name=f"Es{sp}_{pr}")
                E_rep = e_p.tile([128, F, NO], bf16, tag=f"Er{sp}",
                                 name=f"Er{sp}_{pr}")
                co_sup = sm_p.tile([128, F], bf16, tag=f"cos{sp}", bufs=1,
                                   name=f"cos{sp}_{pr}")
                co_rep = sm_p.tile([128, F], bf16, tag=f"cor{sp}", bufs=1,
                                   name=f"cor{sp}_{pr}")
                sc_dr = USE_FP8 and FP8_SCORES
                NKS = ND // 2 if sc_dr else ND
                for it in range(F):
                    isl = slice(it * 128, (it + 1) * 128)
                    ps3 = {}
                    for p in (0, 2, 1):   # sup, rep, con
                        ps = ps_big.tile([128, NO], f32, tag="ps",
                                         name=f"sc{p}")
                        ps3[p] = ps
                        for k in range(NKS):
                            if sc_dr:
                                ksl = slice(2 * k, 2 * k + 2)
                                if g["side_q"]:
                                    lhsT = projT[p][:, ksl, sp, isl]
                                    rhs = xo_t[:, ksl,
                                               offo[s4] : offo[s4] + NO]
                                else:
                                    lhsT = xq_t[:, ksl,
                                                offq[s4] + it * 128 :
                                                offq[s4] + (it + 1) * 128]
                                    rhs = projT[p][:, ksl, sp, 0:NO]
                                nc.tensor.matmul(
                                    ps[:], lhsT=lhsT, rhs=rhs,
                                    start=(k == 0),
                                    stop=(k == NKS - 1 and p == 1),
                                    perf_mode=mybir.MatmulPerfMode.DoubleRow,
                                )
                            else:
                                if g["side_q"]:
                                    lhsT = projT[p][:, k, sp, isl]
                                    rhs = xo_t[:, k, offo[s4] : offo[s4] + NO]
                                else:
                                    lhsT = xqb_t[:, k,
                                                 offq[s4] + it * 128 :
                                                 offq[s4] + (it + 1) * 128]
                                    rhs = projT[p][:, k, sp, 0:NO]
                                nc.tensor.matmul(ps[:], lhsT=lhsT, rhs=rhs,
                                                 start=(k == 0),
                                                 stop=(k == NKS - 1 and p == 1))
                        if p != 1:
                            # option-mask bias row closes the sup/rep groups
                            nc.tensor.matmul(ps[:], lhsT=ones_row[:],
                                             rhs=ob_t[:, s4, OJ:L],
                                             start=False, stop=True)
                    ps_sup, ps_rep, ps_con = ps3[0], ps3[2], ps3[1]

                    T_t = tmp_p.tile([128, NO], f32, tag="T")
                    nc.scalar.activation(T_t[:], ps_con[:], AF.Tanh,
                                         scale=SCALE / PRE)
                    A_t = tmp_p.tile([128, NO], f32, tag="A")
                    nc.vector.scalar_tensor_tensor(
                        A_t[:], in0=ps_rep[:], scalar=SCALE / PRE, in1=T_t[:],
                        op0=ALU.mult, op1=ALU.add,
                    )
                    rs_sup = sm_p.tile([128, 1], f32, tag="rss")
                    nc.scalar.activation(E_sup[:, it], ps_sup[:], AF.Exp,
                                         scale=SCALE / PRE, accum_out=rs_sup[:])
                    rs_rep = sm_p.tile([128, 1], f32, tag="rsr")
                    nc.scalar.activation(E_rep[:, it], A_t[:], AF.Exp,
                                         accum_out=rs_rep[:])
                    rc_sup = sm_p.tile([128, 1], f32, tag="rcs")
                    nc.vector.reciprocal(rc_sup[:], rs_sup[:])
                    nc.vector.tensor_mul(co_sup[:, it : it + 1],
                                         gcol[:, it, s4 : s4 + 1], rc_sup[:])
                    rc_rep = sm_p.tile([128, 1], f32, tag="rcr")
                    nc.vector.reciprocal(rc_rep[:], rs_rep[:])
                    nc.vector.tensor_mul(co_rep[:, it : it + 1],
                                         gcol[:, it, s4 : s4 + 1], rc_rep[:])

                # r vectors: G col 4+s (rep), 8+s (sup)
                for t, (E_t, co_t) in enumerate(((E_rep, co_rep),
                                                 (E_sup, co_sup))):
                    for jt in range(NO // 128):
                        jsl = slice(jt * 128, (jt + 1) * 128)
                        r_ps = ps_sm.tile([128, 1], f32, tag="r")
                        for it in range(F):
                            nc.tensor.matmul(
                                r_ps[:], lhsT=E_t[:, it, jsl],
                                rhs=co_t[:, it : it + 1],
                                start=(it == 0), stop=(it == F - 1),
                            )
                        nc.vector.tensor_copy(
                            G_all[:, g["J0"] + jt, s4,
                                  4 * (t + 1) + s4 : 4 * (t + 1) + s4 + 1],
                            r_ps[:],
                        )

        es1.close()
        es2.close()

        # ---- remaining tail weights (after proj/W pools freed) ----
        tail_p = ctx.enter_context(tc.tile_pool(name="tail", bufs=1))
        if not early_tail:
            w1h_t[0] = _load_w1_half(tail_p, 0, "w1a2")
        w1h_t[1] = _load_w1_half(tail_p, 1, "w1b")
        w2_t = tail_p.tile([128, ND, D], bf16, name="w2sb")
        nc.gpsimd.dma_start(w2_t[:], w2_d.rearrange("k p n -> p k n"))

        # ---- pool: pooled[t*4+s, :] = sum_l G[l, t*4+s] * x_s[l, :] ----
        seq = []
        for s4 in range(BC):
            g = geos[s4 // 2]
            rts = sorted(set(range(g["F"]))
                         | (set(range(g["J0"], NL)) if g["have"] else set()))
            seq.extend((s4, rt) for rt in rts)
        pooled_sb = main_p.tile([12, D], bf16)
        fused_sb = main_p.tile([128, ND, 3, BC], bf16)
        for half in range(2):
            hs = slice(half * 512, (half + 1) * 512)
            po_ps = ps_med.tile([12, 512], f32, tag="pm")
            for i, (s4, rt) in enumerate(seq):
                nc.tensor.matmul(
                    po_ps[:], lhsT=G_all[:, rt, s4, :], rhs=x_t[:, s4, rt, hs],
                    start=(i == 0), stop=(i == len(seq) - 1),
                )
            nc.vector.tensor_copy(pooled_sb[:, hs], po_ps[:])
            for m in range(half * 4, half * 4 + 4):
                tr_ps = ps_sm.tile([128, 12], bf16, tag="r")
                nc.tensor.transpose(tr_ps[:],
                                    pooled_sb[:, m * 128 : (m + 1) * 128],
                                    ident_b[0:12, 0:12])
                nc.vector.tensor_copy(fused_sb[:, m], tr_ps[:])

        # ---- MLP tail + LayerNorm, all in [4, 1024] row form ----
        h_sb = main_p.tile([BC, D], bf16)
        hT_sb = main_p.tile([128, ND, BC], bf16)
        for half in range(2):
            hs = slice(half * 512, (half + 1) * 512)
            h_ps = ps_big.tile([BC, 512], f32, tag="ps")
            for t in range(3):
                for m in range(ND):
                    nc.tensor.matmul(
                        h_ps[:], lhsT=fused_sb[:, m, t, :],
                        rhs=w1h_t[half][:, t * ND + m, :],
                        start=(t == 0 and m == 0), stop=False,
                    )
            nc.tensor.matmul(h_ps[:], lhsT=ones4[:], rhs=b1_t[:, hs],
                             start=False, stop=True)
            nc.scalar.activation(h_sb[:, hs], h_ps[:], AF.Relu)
            for m in range(half * 4, half * 4 + 4):
                ht_ps = ps_sm.tile([128, BC], bf16, tag="r")
                nc.tensor.transpose(ht_ps[:], h_sb[:, m * 128 : (m + 1) * 128],
                                    ident_b[0:BC, 0:BC])
                nc.vector.tensor_copy(hT_sb[:, m], ht_ps[:])

        y_sb = main_p.tile([BC, D], f32)
        s1_t = sm_p.tile([BC, 2], f32, tag="s1", bufs=1)
        s2_t = sm_p.tile([BC, 2], f32, tag="s2", bufs=1)
        for half in range(2):
            hs = slice(half * 512, (half + 1) * 512)
            y_ps = ps_big.tile([BC, 512], f32, tag="ps")
            for k in range(ND):
                nc.tensor.matmul(y_ps[:], lhsT=hT_sb[:, k, :],
                                 rhs=w2_t[:, k, hs],
                                 start=(k == 0), stop=False)
            nc.tensor.matmul(y_ps[:], lhsT=ones4[:], rhs=b2_t[:, hs],
                             start=False, stop=True)
            if half == 0:
                nc.scalar.activation(y_sb[:, hs], y_ps[:], AF.Copy,
                                     accum_out=s1_t[:, half : half + 1])
            else:
                # split across engines: DVE copies+sums while ACT squares
                nc.vector.tensor_copy(y_sb[:, hs], y_ps[:])
                nc.vector.tensor_reduce(s1_t[:, half : half + 1],
                                        y_ps[:], axis=mybir.AxisListType.X,
                                        op=ALU.add)
            sq = tmp_p.tile([BC, 512], f32, tag="sq", bufs=2)
            nc.scalar.activation(sq[:], y_ps[:], AF.Square,
                                 accum_out=s2_t[:, half : half + 1])

        mu_t = sm_p.tile([BC, 1], f32, tag="mu", bufs=1)
        nc.vector.tensor_reduce(mu_t[:], s1_t[:], axis=mybir.AxisListType.X,
                                op=ALU.add)
        nc.scalar.mul(mu_t[:], mu_t[:], 1.0 / D)
        msq_t = sm_p.tile([BC, 1], f32, tag="msq", bufs=1)
        nc.vector.tensor_reduce(msq_t[:], s2_t[:], axis=mybir.AxisListType.X,
                                op=ALU.add)
        nc.scalar.mul(msq_t[:], msq_t[:], 1.0 / D)
        m2_t = sm_p.tile([BC, 1], f32, tag="m2", bufs=1)
        nc.vector.tensor_mul(m2_t[:], mu_t[:], mu_t[:])
        var_t = sm_p.tile([BC, 1], f32, tag="var", bufs=1)
        nc.vector.tensor_scalar(var_t[:], msq_t[:], scalar1=m2_t[:],
                                scalar2=LN_EPS, op0=ALU.subtract, op1=ALU.add)
        sd_t = sm_p.tile([BC, 1], f32, tag="sd", bufs=1)
        nc.scalar.sqrt(sd_t[:], var_t[:])
        rstd_t = sm_p.tile([BC, 1], f32, tag="rstd", bufs=1)
        nc.vector.reciprocal(rstd_t[:], sd_t[:])
        nmr_t = sm_p.tile([BC, 1], f32, tag="nmr", bufs=1)
        nc.vector.tensor_scalar(nmr_t[:], mu_t[:], scalar1=rstd_t[:],
                                scalar2=-1.0, op0=ALU.mult, op1=ALU.mult)

        z_sb = main_p.tile([BC, D], f32)
        for half in range(2):
            hs = slice(half * 512, (half + 1) * 512)
            nc.scalar.activation(z_sb[:, hs], y_sb[:, hs], AF.Identity,
                                 scale=rstd_t[:], bias=nmr_t[:])
            if not ln_trivial:
                nc.vector.tensor_mul(z_sb[:, hs], z_sb[:, hs], lng_t[:, hs])
                nc.vector.tensor_add(z_sb[:, hs], z_sb[:, hs], lnb_t[:, hs])
            nc.sync.dma_start(out_d[:, hs], z_sb[:, hs])

    nc.compile()
    return nc


def _masks(x_ids, pad_idx, sep_idx):
    valid = x_ids != pad_idx
    sepm = x_ids == sep_idx
    has = sepm.any(axis=1)
    first = sepm.argmax(axis=1)
    vlen = valid.sum(axis=1)
    fb = np.clip(vlen // 2, 1, max(1, L - 2))
    sp = np.where(has, first, fb)
    pos = np.arange(L)
    fmask = (pos[None, :] < sp[:, None]) & valid
    omask = (pos[None, :] > sp[:, None]) & valid
    return sp, fmask, omask


def _host_prep_fast(inputs):
    import os

    x = np.asarray(inputs["x"], dtype=np.float32)
    x_ids = np.asarray(inputs["x_ids"])
    pad_idx = int(np.asarray(inputs["pad_idx"]))
    sep_idx = int(np.asarray(inputs["sep_idx"]))
    assert x.shape == (B, L, D), x.shape
    np_x = np_fp8 if USE_FP8 else np_bf16

    sp, fmask, omask = _masks(x_ids, pad_idx, sep_idx)
    fb = np.where(fmask, 0.0, FBIAS * PRE).astype(np_bf16)
    ob = np.where(omask, 0.0, OBIAS_RAW * PRE).astype(np_bf16)

    order = np.argsort(-sp, kind="stable")
    F_all = np.ceil(sp / 128).astype(int)
    J0_all = np.minimum((sp + 1) // 128, NL)
    pair_geo = tuple(
        (int(F_all[order[pr * 16 : (pr + 1) * 16]].max()),
         int(J0_all[order[pr * 16 : (pr + 1) * 16]].min()))
        for pr in range(2)
    )
    fbnd = os.environ.get("FORCE_BOUNDS")
    if fbnd:
        f0, j0, f1, j1 = (int(v) for v in fbnd.split(","))
        pair_geo = ((f0, j0), (f1, j1))
    geos = [_geo(F, J0) for (F, J0) in pair_geo]
    need_mt = any(g["have"] and not g["side_q"] for g in geos)
    np_sc = np_fp8 if (USE_FP8 and FP8_SCORES) else np_bf16
    has_kside = any(g["have"] and not g["side_q"] for g in geos)
    need_xqb = USE_FP8
    need_xo8 = (np_sc != np_x) and has_kside
    CQs = [geos[s // 2]["CQ"] for s in range(BC)]
    NOs = [geos[s // 2]["NO"] for s in range(BC)]
    OJs = [geos[s // 2]["OJ"] for s in range(BC)]
    SQ, SO = sum(CQs), sum(NOs)

    def w(name):
        return np.ascontiguousarray(np.asarray(inputs[name], dtype=np.float32))

    shared = {}
    for p, (qn, kn) in enumerate((("w_sq", "w_sk"), ("w_cq", "w_ck"),
                                  ("w_rq", "w_rk"))):
        shared[f"m{p}"] = _m_matrix(inputs[qn], inputs[kn])
        if need_mt:
            shared[f"mt{p}"] = _m_matrix(inputs[qn], inputs[kn], transposed=True)

    wanom_pm = w("w_anom").reshape(ND, 128).T            # [128, ND]
    emb = np.zeros((128, ND, BC, BC), np.float32)
    for s in range(BC):
        emb[:, :, s, s] = wanom_pm * PRE
    shared["wanom_emb"] = emb.astype(np_bf16)

    shared["w_f1"] = np.ascontiguousarray(
        w("w_f1").reshape(NC3, 128, D)).astype(np_bf16)
    shared["w_f2"] = np.ascontiguousarray(
        w("w_f2").reshape(ND, 128, D)).astype(np_bf16)
    shared["b_f1"] = w("b_f1").reshape(1, D).astype(np_bf16)
    shared["b_f2"] = w("b_f2").reshape(1, D).astype(np_bf16)
    ln_g, ln_b = w("ln_g"), w("ln_b")
    ln_trivial = bool(np.all(ln_g == 1.0) and np.all(ln_b == 0.0))
    if not ln_trivial:
        shared["ln_g"] = np.broadcast_to(ln_g.reshape(1, D), (BC, D)).copy()
        shared["ln_b"] = np.broadcast_to(ln_b.reshape(1, D), (BC, D)).copy()

    in_maps = []
    core_idx = []
    for c in range(NCORES):
        idx = order[np.arange(BC) * NCORES + c]
        core_idx.append(idx)
        xs = x[idx]                                      # [BC, L, D] f32
        m = dict(shared)
        m["x"] = xs.astype(np_bf16)
        xsT = np.ascontiguousarray(xs.transpose(2, 0, 1))   # [D, BC, L] f32
        xq_f = np.ascontiguousarray(np.concatenate(
            [xsT[:, s, 0 : CQs[s]] for s in range(BC)], axis=1,
        ))
        m["xq"] = xq_f.reshape(ND, 128, SQ).astype(np_x)
        if need_xqb:
            m["xqb"] = xq_f.reshape(ND, 128, SQ).astype(np_bf16)
        if SO:
            xo_f = np.ascontiguousarray(np.concatenate(
                [xsT[:, s, OJs[s] : L] for s in range(BC)], axis=1,
            ))
            m["xo"] = xo_f.reshape(ND, 128, SO).astype(np_sc)
            if need_xo8:
                m["xo8"] = xo_f.reshape(ND, 128, SO).astype(np_x)
        m["fbias"] = np.ascontiguousarray(fb[idx])
        m["obias"] = np.ascontiguousarray(ob[idx])
        in_maps.append(m)
    return in_maps, (pair_geo, ln_trivial), core_idx


def get_program_fast(key):
    if key not in _PROGRAM_CACHE:
        pair_geo, ln_trivial = key
        _PROGRAM_CACHE[key] = build_program_fast(pair_geo, ln_trivial)
    return _PROGRAM_CACHE[key]


def run(trace=False, **inputs):
    use_m = all(
        not np.any(np.asarray(inputs[n]))
        for n in ("b_sq", "b_sk", "b_cq", "b_ck", "b_rq", "b_rk")
    )
    if not use_m:
        return _run_legacy(trace=trace, **inputs)
    in_maps, key, core_idx = _host_prep_fast(inputs)
    nc = get_program_fast(key)
    res = bass_utils.run_bass_kernel_spmd(
        nc, in_maps, core_ids=list(range(NCORES)), trace=trace
    )
    out = np.empty((B, D), np.float32)
    for c in range(NCORES):
        out[core_idx[c]] = res.results[c]["out"]
    return out, res


def kernel(**inputs):
    out, _ = run(trace=False, **inputs)
    return out


# ---------------------------------------------------------------------------
# Legacy fallback (nonzero projection biases): original per-slot program.
# ---------------------------------------------------------------------------

PROJ_NAMES = ["w_sq", "w_sk", "w_cq", "w_ck", "w_rq", "w_rk"]
PBIAS_NAMES = ["b_sq", "b_sk", "b_cq", "b_ck", "b_rq", "b_rk"]
QS, KS, QC, KC, QR, KR = range(6)
QPROJ = (QS, QC, QR)
_LEGACY_CACHE = {}


def build_program_legacy(bounds=((2, 2),) * BC):
    nc = bacc.Bacc(
        "TRN2",
        target_bir_lowering=False,
        debug=False,
        enable_asserts=False,
        num_devices=NCORES,
    )

    xT_d = nc.dram_tensor("xT", [BC, D, L], bf16, kind="ExternalInput").ap()
    x_d = nc.dram_tensor("x", [BC, L, D], f32, kind="ExternalInput").ap()
    fmask_d = nc.dram_tensor("fmask", [BC, L], f32, kind="ExternalInput").ap()
    obias_d = nc.dram_tensor("obias", [BC, L], bf16, kind="ExternalInput").ap()

    W_d = {p: nc.dram_tensor(PROJ_NAMES[p], [D, D], bf16, kind="ExternalInput").ap()
           for p in range(6)}
    Brow_d = {
        p: nc.dram_tensor(PBIAS_NAMES[p], [1, D], bf16, kind="ExternalInput").ap()
        for p in range(6)}
    wanom_d = nc.dram_tensor("w_anom", [D, 1], bf16, kind="ExternalInput").ap()
    wf1_d = nc.dram_tensor("w_f1", [ND, 128, NC3 * 128], bf16, kind="ExternalInput").ap()
    wf2_d = nc.dram_tensor("w_f2", [ND, 128, ND * 128], bf16, kind="ExternalInput").ap()
    bf1_d = nc.dram_tensor("b_f1", [128, ND], f32, kind="ExternalInput").ap()
    bf2_d = nc.dram_tensor("b_f2", [128, ND], f32, kind="ExternalInput").ap()
    lng_d = nc.dram_tensor("ln_g", [128, ND], f32, kind="ExternalInput").ap()
    lnb_d = nc.dram_tensor("ln_b", [128, ND], f32, kind="ExternalInput").ap()

    out_d = nc.dram_tensor("out", [BC, D], f32, kind="ExternalOutput").ap()

    with tile.TileContext(nc) as tc, ExitStack() as ctx:
        const_p = ctx.enter_context(tc.tile_pool(name="const", bufs=1))
        tmp_p = ctx.enter_context(tc.tile_pool(name="tmp", bufs=2))
        sm_p = ctx.enter_context(tc.tile_pool(name="small", bufs=3))
        tail_p = ctx.enter_context(tc.tile_pool(name="tail", bufs=1))
        ps_big = ctx.enter_context(tc.tile_pool(name="psb", bufs=4, space="PSUM"))
        ps_s = ctx.enter_context(tc.tile_pool(name="pss", bufs=4, space="PSUM"))
        es2 = ExitStack()   # closed after phase C: x, E
        x_p = es2.enter_context(tc.tile_pool(name="x", bufs=3))
        e_p = es2.enter_context(tc.tile_pool(name="emat", bufs=2))
        es1 = ExitStack()   # closed after phase B: xT, W, proj
        xT_p = es1.enter_context(tc.tile_pool(name="xT", bufs=1))
        w_p = es1.enter_context(tc.tile_pool(name="w", bufs=2))
        proj_p = es1.enter_context(tc.tile_pool(name="proj", bufs=1))

        ones_row = const_p.tile([1, L], bf16)
        nc.vector.memset(ones_row[:], 1.0)
        ones_f = const_p.tile([1, 128], f32)
        nc.vector.memset(ones_f[:], 1.0)
        ones_col = const_p.tile([128, 1], f32)
        nc.vector.memset(ones_col[:], 1.0)
        iot_t = const_p.tile([128, 128], mybir.dt.int32)
        nc.gpsimd.iota(iot_t[:], pattern=[[1, 128]], base=0, channel_multiplier=-1)
        ident_t = const_p.tile([128, 128], f32)
        nc.vector.tensor_scalar(ident_t[:], iot_t[:], scalar1=0, scalar2=None,
                                op0=ALU.is_equal)

        wanom_t = const_p.tile([128, ND], bf16)
        nc.scalar.dma_start(wanom_t[:], wanom_d[:, 0].rearrange("(k p) -> p k", p=128))
        brow_t = {}
        for p in Brow_d:
            brow_t[p] = const_p.tile([1, D], bf16, name=f"brow{p}")
            nc.sync.dma_start(brow_t[p][:], Brow_d[p][:])
        bf1_t = const_p.tile([128, ND], f32)
        nc.scalar.dma_start(bf1_t[:], bf1_d[:])
        bf2_t = const_p.tile([128, ND], f32)
        nc.scalar.dma_start(bf2_t[:], bf2_d[:])
        lng_t = const_p.tile([128, ND], f32)
        nc.scalar.dma_start(lng_t[:], lng_d[:])
        lnb_t = const_p.tile([128, ND], f32)
        nc.scalar.dma_start(lnb_t[:], lnb_d[:])

        fusedT = tail_p.tile([128, NC3, BC], bf16)

        geo = []
        for s in range(BC):
            F, J0 = bounds[s]
            geo.append((F, J0, F * 128, J0 * 128, L - J0 * 128,
                        F > 0 and L - J0 * 128 > 0))

        xT_t = xT_p.tile([128, BC * ND, L], bf16)
        fm_ts, ob_ts, x_ts = [], [], []
        for s in range(BC):
            nc.sync.dma_start(
                xT_t[:, s * ND : (s + 1) * ND, :],
                xT_d[s].rearrange("(k p) i -> p k i", p=128),
            )
            fm_t = sm_p.tile([128, NL], f32, tag="fm", bufs=BC, name=f"fm{s}")
            nc.scalar.dma_start(fm_t[:], fmask_d[s].rearrange("(t p) -> p t", p=128))
            fm_ts.append(fm_t)
            ob_t = sm_p.tile([1, L], bf16, tag="ob", bufs=2, name=f"ob{s}")
            nc.scalar.dma_start(ob_t[:], obias_d[s : s + 1, :])
            ob_ts.append(ob_t)

        gate_ts = []
        for s in range(BC):
            F, J0, CQ, OJ, NO, have_attn = geo[s]
            gate_t = sm_p.tile([128, NL], f32, tag="gate", bufs=BC, name=f"gate{s}")
            gate_ts.append(gate_t)
            if F == 0:
                continue
            ghat_t = sm_p.tile([128, NL], f32, tag="ghat")
            for it in range(F):
                al_ps = ps_s.tile([128, 1], f32, tag="pss")
                for k in range(ND):
                    nc.tensor.matmul(
                        al_ps[:],
                        lhsT=xT_t[:, s * ND + k, it * 128 : (it + 1) * 128],
                        rhs=wanom_t[:, k : k + 1],
                        start=(k == 0), stop=(k == ND - 1),
                    )
                eg_t = sm_p.tile([128, 1], f32, tag="eg")
                nc.scalar.activation(eg_t[:], al_ps[:], AF.Exp)
                nc.vector.tensor_mul(
                    ghat_t[:, it : it + 1], eg_t[:], fm_ts[s][:, it : it + 1]
                )
            gsum_t = sm_p.tile([128, 1], f32, tag="gsum")
            nc.vector.tensor_reduce(
                gsum_t[:], ghat_t[:, 0:F], axis=mybir.AxisListType.X, op=ALU.add
            )
            S_ps = ps_s.tile([1, 1], f32, tag="pss")
            nc.tensor.matmul(S_ps[:], lhsT=gsum_t[:], rhs=ones_col[:],
                             start=True, stop=True)
            Smax_t = sm_p.tile([1, 1], f32, tag="Smax")
            nc.vector.tensor_scalar_max(Smax_t[:], S_ps[:], 1e-8)
            Sb_ps = ps_s.tile([128, 1], f32, tag="pss")
            nc.tensor.matmul(Sb_ps[:], lhsT=ones_f[:], rhs=Smax_t[:],
                             start=True, stop=True)
            recipS_t = sm_p.tile([128, 1], f32, tag="recipS")
            nc.vector.reciprocal(recipS_t[:], Sb_ps[:])
            nc.vector.tensor_scalar_mul(gate_t[:, 0:F], ghat_t[:, 0:F],
                                        recipS_t[:])

        projs = [[None] * BC for _ in range(6)]
        for p in range(6):
            qside = p in QPROJ
            widths = [
                ((g[2] if qside else g[4]) if g[5] else 0) for g in geo
            ]
            wmax = max(widths)
            if wmax == 0:
                continue
            wt = w_p.tile([128, ND, D], bf16, tag="w", name=f"w{p}")
            nc.gpsimd.dma_start(wt[:], W_d[p].rearrange("(k p) c -> p k c", p=128))
            pt = proj_p.tile([128, BC, ND, wmax], bf16, tag=f"proj{p}")
            for m in range(ND):
                for s in range(BC):
                    width = widths[s]
                    if width == 0:
                        continue
                    lo = 0 if qside else geo[s][3]
                    ps = ps_big.tile([128, width], f32, tag="ps")
                    for k in range(ND):
                        nc.tensor.matmul(
                            ps[:], lhsT=wt[:, k, m * 128 : (m + 1) * 128],
                            rhs=xT_t[:, s * ND + k, lo : lo + width],
                            start=(k == 0), stop=False,
                        )
                    nc.tensor.matmul(
                        ps[:], lhsT=brow_t[p][:, m * 128 : (m + 1) * 128],
                        rhs=ones_row[:, 0:width], start=False, stop=True,
                    )
                    nc.vector.tensor_copy(pt[:, s, m, :], ps[:])
            for s in range(BC):
                if widths[s]:
                    projs[p][s] = pt

        for s in range(BC):
            x_t = x_p.tile([128, NL, D], f32, tag="x", name=f"x{s}")
            nc.sync.dma_start(x_t[:], x_d[s].rearrange("(t p) d -> p t d", p=128))
            x_ts.append(x_t)

        E_sups, E_reps, co_sups, co_reps = {}, {}, {}, {}
        for s in range(BC):
            F, J0, CQ, OJ, NO, have_attn = geo[s]
            if not have_attn:
                continue
            E_sup = e_p.tile([128, max(F, 1), NO], f32, tag="esup", bufs=BC,
                             name=f"esup{s}")
            E_rep = e_p.tile([128, max(F, 1), NO], f32, tag="erep", bufs=BC,
                             name=f"erep{s}")
            co_sup = sm_p.tile([128, NL], f32, tag="cosup", bufs=BC,
                               name=f"cosup{s}")
            co_rep = sm_p.tile([128, NL], f32, tag="corep", bufs=BC,
                               name=f"corep{s}")
            E_sups[s], E_reps[s] = E_sup, E_rep
            co_sups[s], co_reps[s] = co_sup, co_rep
            gate_t = gate_ts[s]
            ob_t = ob_ts[s]
            for it in range(F):
                isl = slice(it * 128, (it + 1) * 128)
                ps_sup = ps_big.tile([128, NO], f32, tag="ps")
                for k in range(ND):
                    nc.tensor.matmul(
                        ps_sup[:], lhsT=projs[QS][s][:, s, k, isl],
                        rhs=projs[KS][s][:, s, k, 0:NO],
                        start=(k == 0), stop=False,
                    )
                nc.tensor.matmul(ps_sup[:], lhsT=ones_row[:, 0:128],
                                 rhs=ob_t[:, OJ:L], start=False, stop=True)
                ps_con = ps_big.tile([128, NO], f32, tag="ps")
                for k in range(ND):
                    nc.tensor.matmul(
                        ps_con[:], lhsT=projs[QC][s][:, s, k, isl],
                        rhs=projs[KC][s][:, s, k, 0:NO],
                        start=(k == 0), stop=(k == ND - 1),
                    )
                ps_rep = ps_big.tile([128, NO], f32, tag="ps")
                for k in range(ND):
                    nc.tensor.matmul(
                        ps_rep[:], lhsT=projs[QR][s][:, s, k, isl],
                        rhs=projs[KR][s][:, s, k, 0:NO],
                        start=(k == 0), stop=False,
                    )
                nc.tensor.matmul(ps_rep[:], lhsT=ones_row[:, 0:128],
                                 rhs=ob_t[:, OJ:L], start=False, stop=True)

                T_t = tmp_p.tile([128, NO], f32, tag="T")
                nc.scalar.activation(T_t[:], ps_con[:], AF.Tanh, scale=SCALE)
                A_t = tmp_p.tile([128, NO], f32, tag="A")
                nc.vector.scalar_tensor_tensor(
                    A_t[:], in0=ps_rep[:], scalar=SCALE, in1=T_t[:],
                    op0=ALU.mult, op1=ALU.add,
                )
                rs_sup = sm_p.tile([128, 1], f32, tag="rssup")
                nc.scalar.activation(E_sup[:, it, :], ps_sup[:], AF.Exp,
                                     scale=SCALE, accum_out=rs_sup[:])
                rs_rep = sm_p.tile([128, 1], f32, tag="rsrep")
                nc.scalar.activation(E_rep[:, it, :], A_t[:], AF.Exp,
                                     accum_out=rs_rep[:])
                rc_sup = sm_p.tile([128, 1], f32, tag="rcsup")
                nc.vector.reciprocal(rc_sup[:], rs_sup[:])
                nc.vector.tensor_mul(co_sup[:, it : it + 1],
                                     gate_t[:, it : it + 1], rc_sup[:])
                rc_rep = sm_p.tile([128, 1], f32, tag="rcrep")
                nc.vector.reciprocal(rc_rep[:], rs_rep[:])
                nc.vector.tensor_mul(co_rep[:, it : it + 1],
                                     gate_t[:, it : it + 1], rc_rep[:])

        es1.close()

        for s in range(BC):
            F, J0, CQ, OJ, NO, have_attn = geo[s]
            x_t = x_ts[s]

            G_t = sm_p.tile([128, NL, 3], f32, tag="G")
            nc.vector.memset(G_t[:], 0.0)
            if F > 0:
                for it in range(F):
                    nc.vector.tensor_copy(G_t[:, it, 0:1],
                                          gate_ts[s][:, it : it + 1])
            if have_attn:
                E_sup, E_rep = E_sups[s], E_reps[s]
                co_sup, co_rep = co_sups[s], co_reps[s]
                for jt in range(J0, NL):
                    jsl = slice(jt * 128 - OJ, jt * 128 - OJ + 128)
                    r_ps = ps_s.tile([128, 2], f32, tag="pss")
                    for it in range(F):
                        nc.tensor.matmul(
                            r_ps[:, 0:1], lhsT=E_rep[:, it, jsl],
                            rhs=co_rep[:, it : it + 1],
                            start=(it == 0), stop=(it == F - 1),
                        )
                    for it in range(F):
                        nc.tensor.matmul(
                            r_ps[:, 1:2], lhsT=E_sup[:, it, jsl],
                            rhs=co_sup[:, it : it + 1],
                            start=(it == 0), stop=(it == F - 1),
                        )
                    nc.vector.tensor_copy(G_t[:, jt, 1:3], r_ps[:, 0:2])

            rts = sorted(set(range(F)) | (set(range(J0, NL)) if have_attn else set()))
            if not rts:
                rts = [0]
            for m in range(ND):
                pool_ps = ps_s.tile([128, 3], f32, tag="pss")
                for i, rt in enumerate(rts):
                    nc.tensor.matmul(
                        pool_ps[:], lhsT=x_t[:, rt, m * 128 : (m + 1) * 128],
                        rhs=G_t[:, rt, :],
                        start=(i == 0), stop=(i == len(rts) - 1),
                    )
                for t in range(3):
                    nc.vector.tensor_copy(
                        fusedT[:, t * ND + m, s : s + 1], pool_ps[:, t : t + 1]
                    )

        es2.close()

        wf1_p = ctx.enter_context(tc.tile_pool(name="wf1", bufs=8))
        hT_t = tail_p.tile([128, ND, BC], bf16)
        for m in range(ND):
            wt = wf1_p.tile([128, NC3, 128], bf16, tag="wf1")
            nc.gpsimd.dma_start(wt[:], wf1_d[m].rearrange("p (k c) -> p k c", c=128))
            h_ps = ps_s.tile([128, BC], f32, tag="pss")
            for k in range(NC3):
                nc.tensor.matmul(h_ps[:], lhsT=wt[:, k, :], rhs=fusedT[:, k, :],
                                 start=(k == 0), stop=(k == NC3 - 1))
            nc.scalar.activation(hT_t[:, m, :], h_ps[:], AF.Relu,
                                 bias=bf1_t[:, m : m + 1])

        yT_t = tail_p.tile([128, ND, BC], f32)
        sq_t = tail_p.tile([128, ND, BC], f32)
        for m in range(ND):
            wt = wf1_p.tile([128, ND, 128], bf16, tag="wf2")
            nc.gpsimd.dma_start(wt[:], wf2_d[m].rearrange("p (k c) -> p k c", c=128))
            y_ps = ps_s.tile([128, BC], f32, tag="pss")
            for k in range(ND):
                nc.tensor.matmul(y_ps[:], lhsT=wt[:, k, :], rhs=hT_t[:, k, :],
                                 start=(k == 0), stop=(k == ND - 1))
            nc.vector.tensor_scalar_add(yT_t[:, m, :], y_ps[:], bf2_t[:, m : m + 1])
            nc.scalar.square(sq_t[:, m, :], yT_t[:, m, :])

        sum_ps = ps_s.tile([1, BC], f32, tag="pss")
        for m in range(ND):
            nc.tensor.matmul(sum_ps[:], lhsT=ones_col[:], rhs=yT_t[:, m, :],
                             start=(m == 0), stop=(m == ND - 1))
        ssq_ps = ps_s.tile([1, BC], f32, tag="pss")
        for m in range(ND):
            nc.tensor.matmul(ssq_ps[:], lhsT=ones_col[:], rhs=sq_t[:, m, :],
                             start=(m == 0), stop=(m == ND - 1))
        mean_t = sm_p.tile([1, BC], f32, tag="mean")
        nc.scalar.mul(mean_t[:], sum_ps[:], 1.0 / D)
        msq_t = sm_p.tile([1, BC], f32, tag="msq")
        nc.scalar.mul(msq_t[:], ssq_ps[:], 1.0 / D)
        m2_t = sm_p.tile([1, BC], f32, tag="m2")
        nc.vector.tensor_mul(m2_t[:], mean_t[:], mean_t[:])
        var_t = sm_p.tile([1, BC], f32, tag="var")
        nc.vector.tensor_sub(var_t[:], msq_t[:], m2_t[:])
        nc.vector.tensor_scalar_add(var_t[:], var_t[:], LN_EPS)
        sd_t = sm_p.tile([1, BC], f32, tag="sd")
        nc.scalar.sqrt(sd_t[:], var_t[:])
        rstd_t = sm_p.tile([1, BC], f32, tag="rstd")
        nc.vector.reciprocal(rstd_t[:], sd_t[:])

        mb_ps = ps_s.tile([128, BC], f32, tag="pss")
        nc.tensor.matmul(mb_ps[:], lhsT=ones_f[:], rhs=mean_t[:],
                         start=True, stop=True)
        mb_t = sm_p.tile([128, BC], f32, tag="mbt")
        nc.vector.tensor_copy(mb_t[:], mb_ps[:])
        rb_ps = ps_s.tile([128, BC], f32, tag="pss")
        nc.tensor.matmul(rb_ps[:], lhsT=ones_f[:], rhs=rstd_t[:],
                         start=True, stop=True)
        rb_t = sm_p.tile([128, BC], f32, tag="rbt")
        nc.vector.tensor_copy(rb_t[:], rb_ps[:])

        zrow_t = tail_p.tile([BC, D], f32)
        for m in range(ND):
            z_t = tmp_p.tile([128, BC], f32, tag="z")
            nc.vector.tensor_sub(z_t[:], yT_t[:, m, :], mb_t[:])
            nc.vector.tensor_mul(z_t[:], z_t[:], rb_t[:])
            z2_t = tmp_p.tile([128, BC], f32, tag="z2")
            nc.vector.tensor_scalar(
                z2_t[:], z_t[:], scalar1=lng_t[:, m : m + 1],
                scalar2=lnb_t[:, m : m + 1], op0=ALU.mult, op1=ALU.add,
            )
            tr_ps = ps_s.tile([BC, 128], f32, tag="pss")
            nc.tensor.transpose(tr_ps[:], z2_t[:], ident_t[:])
            nc.vector.tensor_copy(zrow_t[:, m * 128 : (m + 1) * 128], tr_ps[:])
        nc.sync.dma_start(out_d[:, :], zrow_t[:, :])

    nc.compile()
    return nc


def _run_legacy(trace=False, **inputs):
    x = np.asarray(inputs["x"], dtype=np.float32)
    x_ids = np.asarray(inputs["x_ids"])
    pad_idx = int(np.asarray(inputs["pad_idx"]))
    sep_idx = int(np.asarray(inputs["sep_idx"]))

    sp, fmask_b, omask = _masks(x_ids, pad_idx, sep_idx)
    fmask = fmask_b.astype(np.float32)
    obias = np.where(omask, 0.0, OBIAS_RAW).astype(np.float32)

    F_all = np.ceil(sp / 128).astype(int)
    J0_all = np.minimum((sp + 1) // 128, NL)
    bounds = tuple(
        (int(F_all.reshape(NCORES, BC)[:, s].max()),
         int(J0_all.reshape(NCORES, BC)[:, s].min()))
        for s in range(BC)
    )

    xT = np.ascontiguousarray(x.transpose(0, 2, 1))

    def w(name):
        return np.ascontiguousarray(np.asarray(inputs[name], dtype=np.float32))

    def ppart(name):
        return np.ascontiguousarray(np.asarray(inputs[name], dtype=np.float32)
                                    .reshape(ND, 128).T)

    shared = {}
    for p in range(6):
        shared[PROJ_NAMES[p]] = w(PROJ_NAMES[p]).astype(np_bf16)
        shared[PBIAS_NAMES[p]] = w(PBIAS_NAMES[p]).reshape(1, D).astype(np_bf16)
    shared["w_anom"] = w("w_anom").reshape(D, 1).astype(np_bf16)

    def mpack(name, nk):
        a = w(name)
        a = a.reshape(nk, 128, ND, 128).transpose(2, 1, 0, 3).reshape(ND, 128, nk * 128)
        return np.ascontiguousarray(a).astype(np_bf16)

    shared["w_f1"] = mpack("w_f1", NC3)
    shared["w_f2"] = mpack("w_f2", ND)
    shared["b_f1"] = ppart("b_f1")
    shared["b_f2"] = ppart("b_f2")
    shared["ln_g"] = ppart("ln_g")
    shared["ln_b"] = ppart("ln_b")

    in_maps = []
    for c in range(NCORES):
        sl = slice(c * BC, (c + 1) * BC)
        m = dict(shared)
        m["x"] = np.ascontiguousarray(x[sl])
        m["xT"] = np.ascontiguousarray(xT[sl]).astype(np_bf16)
        m["fmask"] = np.ascontiguousarray(fmask[sl])
        m["obias"] = np.ascontiguousarray(obias[sl]).astype(np_bf16)
        in_maps.append(m)

    if bounds not in _LEGACY_CACHE:
        _LEGACY_CACHE[bounds] = build_program_legacy(bounds)
    nc = _LEGACY_CACHE[bounds]
    res = bass_utils.run_bass_kernel_spmd(
        nc, in_maps, core_ids=list(range(NCORES)), trace=trace
    )
    out = np.concatenate([res.results[c]["out"] for c in range(NCORES)], axis=0)
    return out.astype(np.float32), res
